# revision 1
# baseline (speedup 1.0000x reference)
"""Trainium2 Bass kernel for nn_Attention1 (channel attention transformer block).

Reference computation (per batch):
  kv = W_kv @ x ; k, v = split(kv)                    # pointwise conv over m=3072
  q  = conv3x3(W_q @ y, W_dw)                         # 1x1 then full 3x3, 64x64 image
  q  = linear_interp(snake(q.flatten(HW)), 4096->3072)
  q, k = l2norm over m ; attn = softmax(q @ k^T * temp) per 32-channel head
  out = W_po @ (attn @ v)

Sharding: data-parallel over batch, 16 batches / 8 cores = 2 per core. SPMD,
no collectives; per-core outputs are concatenated on host.

Per-core kernel layout strategy (all heavy matmuls in float32r, which streams
1 row/cycle on the PE when the moving free dim is >= 256):
  - v        : W_vT as stationary -> v natural (channels on partitions)
  - kT       : x columns as stationary -> k transposed (m on partitions)
  - q path   : q1 natural -> zero-padded 66x66 image -> 3x3 conv as 9 shifted
               matmuls with the *image* stationary -> conv output transposed
               (spatial on partitions)
  - snake+interp : fused into a sparse (4096,3072) matrix S applied on the
               partition axis via 24 two-tile matmuls; S blocks repeat with
               period 3, so only 6 distinct 128x128 blocks are uploaded
  - scores   : one matmul stream computes q-gram (for l2 norms, diag extracted
               with an eye mask) and q@kT scores together; k-gram separately
  - softmax  : per-head masking via additive -30 mask over the full 256-wide
               score rows; exp on ScalarE with fused row-sum (accum_out);
               1/Z folded into the attnV PSUM->SBUF copy as a per-row scale
  - attn@v   : exp-scores transposed via PE transpose, then matmul against v
  - po       : W_poT stationary over the attention output
"""
import numpy as np

HEADS = 8
B, DIM, M = 16, 256, 3072
HW = 64
L = HW * HW          # 4096 flattened conv spatial size
NCORES = 8
BL = B // NCORES     # batches per core
C128 = DIM // 128    # channel 128-tiles (2)
NM512 = M // 512     # m-dim 512-tiles (6)
NMT = M // 128       # m-dim 128-tiles (24)
NST = L // 128       # conv-spatial 128-tiles (32)

_CACHE = {}


def _s_blocks():
    """Snake+interp as a sparse matrix; 6 distinct 128x128 blocks (period 3)."""
    mask = np.arange(L).reshape(HW, HW)
    mask[1::2] = mask[1::2][:, ::-1]
    mask = mask.reshape(-1)
    src = (np.arange(M) + 0.5) * (L / M) - 0.5
    src = np.maximum(src, 0.0)
    i0 = np.minimum(np.floor(src).astype(np.int64), L - 1)
    i1 = np.minimum(i0 + 1, L - 1)
    lam = (src - i0).astype(np.float32)
    S = np.zeros((L, M), np.float32)
    S[mask[i0], np.arange(M)] += (1 - lam)
    S[mask[i1], np.arange(M)] += lam
    blocks = np.zeros((6, 128, 128), np.float32)
    for j in range(3):
        s0 = (4 * j) // 3
        for t in range(2):
            blocks[j * 2 + t] = S[128 * (s0 + t):128 * (s0 + t + 1), 128 * j:128 * (j + 1)]
    return blocks.transpose(1, 0, 2).reshape(128, 6 * 128).copy()


def _host_consts(W_kv, W_q, W_dw, W_po, temperature):
    c = np.arange(DIM)
    mask = np.where((c[:, None] // 32) == (c[None, :] // 32), 0.0, -30.0).astype(np.float32)
    tv = np.repeat(np.asarray(temperature, np.float32).reshape(HEADS), DIM // HEADS)
    return {
        "w_kT": np.ascontiguousarray(W_kv[:DIM].T, np.float16),
        "w_v2": np.ascontiguousarray(W_kv[DIM:], np.float16),
        "w_poT": np.ascontiguousarray(W_po.T, np.float32),
        "w_dwT": np.ascontiguousarray(
            np.einsum("oayx,ab->byxo", np.asarray(W_dw, np.float32),
                      np.asarray(W_q, np.float32)).reshape(DIM, 9 * DIM),
            np.float16),
        "s_mat": _s_blocks(),
        "mask": np.ascontiguousarray(mask.reshape(2, 128, DIM)),
        "tempv": np.ascontiguousarray(tv.reshape(2, 128, 1)),
    }


def _make_tc_class():
    """TileContext subclass splitting the end-of-kernel drain waits.

    This container's walrus rejects >1 sem wait on CTRL-encoded instructions
    (Drain/NoOp). The stock Tile epilogue hangs every semaphore's final value
    on one Drain. Emit a chain of SP NoOps with one wait each instead, then a
    waitless drain: SP reaches it only after all sems hit their final values.
    """
    import bass_rust
    import concourse.mybir as mybir
    import concourse.tile as tile

    class SplitDrainTileContext(tile.TileContext):
        def _drain_and_barrier(self, tick_clock, wait_clock):
            probe = self.nc.sync.nop()
            wait_clock.add_sem_waits(
                probe.ins, bass_rust.ScopedClock({None: tick_clock.global_clock})
            )
            waits = list(probe.ins.sync_info.on_wait or [])
            probe.ins.sync_info.on_wait = waits[:1]
            for w in waits[1:]:
                n2 = self.nc.sync.nop()
                n2.ins.sync_info = mybir.SyncInfo(on_wait=[w], on_update=[])
            self.nc.sync.drain()
            self.nc.all_engine_barrier()
            assert self.sems is not None
            popped = self.nc._tile_sem_poison_stack.pop()
            assert popped is self._sem_poison
            self.nc.clear_and_free_semaphores(list(self.sems.allocated().values()))
            self.nc.all_engine_barrier()

    return SplitDrainTileContext


def _split_waits(nc):
    """Walrus in this container allows only one sem wait per instruction.
    Move extra waits onto same-engine NoOps inserted just before."""
    import concourse.mybir as mybir
    n = 0
    for f in nc.m.functions:
        for bb in f.blocks:
            out = []
            changed = False
            for inst in bb.instructions:
                si = inst.sync_info
                waits = list(si.on_wait) if si and si.on_wait else []
                if len(waits) > 1:
                    for w in waits[:-1]:
                        n += 1
                        nop = mybir.InstNoOp(name=f"I-sw{n}-{inst.name}", ins=[], outs=[])
                        nop.engine = inst.engine
                        nop.sync_info = mybir.SyncInfo(on_wait=[w], on_update=[])
                        out.append(nop)
                    si.on_wait = [waits[-1]]
                    changed = True
                out.append(inst)
            if changed:
                bb.instructions = out
    return n


def build_nc(split_waits=True, n_batches=BL, seq=None):
    from contextlib import ExitStack
    from collections import defaultdict
    import concourse.bass as bass
    import concourse.mybir as mybir
    from concourse.masks import make_identity

    f32 = mybir.dt.float32
    f32r = mybir.dt.float32r
    u32 = mybir.dt.uint32
    u16 = mybir.dt.uint16
    f16 = mybir.dt.float16
    Exp = mybir.ActivationFunctionType.Exp
    Square = mybir.ActivationFunctionType.Square
    Sqrt = mybir.ActivationFunctionType.Sqrt
    X = mybir.AxisListType.X

    def r(ap):
        return ap.bitcast(f32r)

    TC = _make_tc_class()
    nc = bass.Bass("TRN2", target_bir_lowering=False, debug=False)

    xd = nc.dram_tensor("x_sh", [BL, DIM, M], f16, kind="ExternalInput").ap()
    yd = nc.dram_tensor("y_sh", [BL, DIM, L], f16, kind="ExternalInput").ap()
    wkd = nc.dram_tensor("w_kT", [DIM, DIM], f16, kind="ExternalInput").ap()
    wvd = nc.dram_tensor("w_v2", [DIM, DIM], f16, kind="ExternalInput").ap()
    wpd = nc.dram_tensor("w_poT", [DIM, DIM], f32r, kind="ExternalInput").ap()
    wdd = nc.dram_tensor("w_dwT", [DIM, 9 * DIM], f16, kind="ExternalInput").ap()
    sd = nc.dram_tensor("s_mat", [128, 6 * 128], f32r, kind="ExternalInput").ap()
    md = nc.dram_tensor("mask", [2, 128, DIM], f32, kind="ExternalInput").ap()
    td = nc.dram_tensor("tempv", [2, 128, 1], f32, kind="ExternalInput").ap()
    od = nc.dram_tensor("out", [BL, DIM, M], f32, kind="ExternalOutput").ap()

    with TC(nc) as tc, ExitStack() as ctx:
        P = lambda **kw: ctx.enter_context(tc.tile_pool(**kw))
        consts = P(name="consts", bufs=1)
        p_qp = P(name="p_qp", bufs=2)
        p_ct = P(name="p_ct", bufs=4)
        p_qk = P(name="p_qk", bufs=4)
        p_sq = P(name="p_sq", bufs=3)
        p_x = P(name="p_x", bufs=4)
        p_big = P(name="p_big", bufs=3)
        p_sm = P(name="p_sm", bufs=2)
        p_tn = P(name="p_tn", bufs=4)
        # global PSUM pools: 2+2+1+3 = 8 banks exactly
        pp_pq = P(name="pp_pq", bufs=3, space="PSUM")
        pp_pc = P(name="pp_pc", bufs=2, space="PSUM")
        pp_ik = P(name="pp_ik", bufs=1, space="PSUM")
        pp_at = P(name="pp_at", bufs=1, space="PSUM")

        # ---- constants ----
        wk = [consts.tile([128, DIM], f16, tag=f"wk{k}", name=f"wk{k}") for k in range(C128)]
        wv2 = [consts.tile([128, DIM], f16, tag=f"wv2{k}", name=f"wv2{k}") for k in range(C128)]
        wp = [consts.tile([128, DIM], f32r, tag=f"wp{k}", name=f"wp{k}") for k in range(C128)]
        wdw = [consts.tile([128, 9 * DIM], f16, tag=f"wdw{k}", name=f"wdw{k}") for k in range(C128)]
        for c3 in range(3):
            for k in range(C128):
                sl = slice(128 * k, 128 * (k + 1))
                c0, c1 = 3 * DIM * c3, 3 * DIM * (c3 + 1)
                nc.scalar.dma_start(out=wdw[k][:, c0:c1], in_=wdd[sl, c0:c1])
        for k in range(C128):
            sl = slice(128 * k, 128 * (k + 1))
            nc.scalar.dma_start(out=wk[k][:], in_=wkd[sl, :])
            nc.scalar.dma_start(out=wv2[k][:], in_=wvd[sl, :])
            nc.scalar.dma_start(out=wp[k][:], in_=wpd[sl, :])
        smat = consts.tile([128, 6, 128], f32r, tag="smat", name="smat")
        nc.scalar.dma_start(out=smat[:], in_=sd.rearrange("p (i m) -> p i m", i=6))
        msk = [consts.tile([128, DIM], f32, tag=f"msk{k}", name=f"msk{k}") for k in range(2)]
        tmpv = [consts.tile([128, 1], f32, tag=f"tmpv{k}", name=f"tmpv{k}") for k in range(2)]
        for rr in range(2):
            nc.scalar.dma_start(out=msk[rr][:], in_=md[rr])
            nc.scalar.dma_start(out=tmpv[rr][:], in_=td[rr])
        ident = consts.tile([128, 128], f32, tag="ident", name="ident")
        make_identity(nc, ident[:])
        ones_row = consts.tile([1, 128], f32, tag="ones", name="ones")
        nc.vector.memset(ones_row[:], 1.0)
        ones_c16 = consts.tile([128, 1], f16, tag="ones16", name="ones16")
        nc.vector.memset(ones_c16[:], 1.0)

        state = defaultdict(dict)

        def emit_load_q1(vk, b):
            s = state[vk]
            s["b"] = b
            s["x"] = [p_x.tile([128, M], f16, tag="x", name="x") for _ in range(C128)]
            for k in range(C128):
                nc.scalar.dma_start(out=s["x"][k][:],
                                    in_=xd[b, 128 * k:128 * (k + 1), :])
            # W_q is folded into the conv weights on the host, so the conv
            # input is y itself: DMA it straight into three horizontally
            # shifted fp16 images, each (128, 66, 64) with zeroed borders and
            # rows contiguous at stride 64 (conv windows must be 1D slices).
            qsh = [[p_qp.tile([128, HW + 2, HW], f16, tag=f"qsh{k}{dx}",
                              name=f"qsh{k}{dx}", bufs=2) for dx in range(3)]
                   for k in range(C128)]
            s["qsh"] = qsh
            # contiguous flat DMAs shifted by dx-1, split into 4 chunks so
            # the first conv tiles can start early; the row-wrap artifacts
            # land exactly on the edge columns the memsets zero afterwards
            NCH = 8
            for c in range(NCH):
                for k in range(C128):
                    yk = yd[b, 128 * k:128 * (k + 1), :]
                    for dx in range(3):
                        tf = qsh[k][dx].rearrange("p a b -> p (a b)")
                        lo, hi = (L * c) // NCH, (L * (c + 1)) // NCH
                        if dx == 0:
                            so, do = 0, HW + 1
                        elif dx == 1:
                            so, do = 0, HW
                        else:
                            so, do = 1, HW - 1 + (0 if True else 0)
                            so, do = 1, HW
                        slo = min(lo + so, L)
                        shi = min(hi + so, L)
                        if shi > slo:
                            nc.sync.dma_start(out=tf[:, do + lo:do + lo + (shi - slo)],
                                              in_=yk[:, slo:shi])
            for k in range(C128):
                for dx in range(3):
                    t = qsh[k][dx]
                    nc.vector.memset(t[:, 0:1, :].bitcast(u32), 0)
                    nc.vector.memset(t[:, HW + 1:HW + 2, :].bitcast(u32), 0)
                    # edge columns per chunk so early conv tiles don't wait
                    # for the whole image
                    for c in range(NCH):
                        r0 = 1 + (L * c) // NCH // HW
                        r1 = 1 + (L * (c + 1)) // NCH // HW
                        if dx == 0:
                            nc.vector.memset(t[:, r0:r1, 0:1].bitcast(u16), 0)
                        elif dx == 2:
                            nc.vector.memset(t[:, r0:r1, HW - 1:HW].bitcast(u16), 0)

        def emit_stream(vk):
            s = state[vk]
            qsh, x_sb = s["qsh"], s["x"]
            ps_scc = pp_at.tile([128, 512], f32, tag="pscc", name="pscc")
            ps_sc = [ps_scc[:, 0:DIM], ps_scc[:, DIM:512]]
            ps_nqk = pp_at.tile([1, 512], f32, tag="nqk", name="nqk")
            s["sc"], s["nqk"] = ps_sc, ps_nqk
            ct_tiles = {}

            def emit_conv(j2):
                ps = pp_pc.tile([128, DIM], f32, tag="pc", name="pc")
                for k in range(C128):
                    flats = [qsh[k][dx].rearrange("p a b -> p (a b)") for dx in range(3)]
                    for dy in range(3):
                        for dx in range(3):
                            off = (2 * j2 + dy) * HW
                            nc.tensor.matmul(
                                ps[:], flats[dx][:, off:off + 128],
                                wdw[k][:, (dy * 3 + dx) * DIM:(dy * 3 + dx + 1) * DIM],
                                start=(k == 0 and dy == 0 and dx == 0),
                                stop=(k == C128 - 1 and dy == 2 and dx == 2))
                ct = p_ct.tile([128, DIM], f32r, tag="ct", name="ct")
                nc.any.tensor_copy(ct[:], ps[:])
                ct_tiles[j2] = ct

            def emit_mtile(j):
                s0 = (4 * j) // 3
                qk = p_qk.tile([128, 512], f32r, tag="qk", name="qk")
                # interp into [0:256] and kT into [256:512] of one PSUM bank,
                # one accumulation group (per-element has_written drives
                # overwrite-then-accumulate), one combined copy.
                psik = pp_ik.tile([128, 512], f32, tag="pik", name="pik")
                for t in range(2):
                    nc.tensor.matmul(
                        psik[:, 0:DIM], r(smat[:, (j % 3) * 2 + t, :]),
                        r(ct_tiles[s0 + t][:]),
                        start=(t == 0), stop=False, skip_group_check=True)
                for k in range(C128):
                    nc.tensor.matmul(
                        psik[:, DIM:512], x_sb[k][:, 128 * j:128 * (j + 1)],
                        wk[k][:],
                        start=False, stop=(k == C128 - 1), skip_group_check=True)
                nc.any.tensor_copy(qk[:], psik[:])
                for rr in range(2):
                    nc.tensor.matmul(
                        ps_sc[rr][:], r(qk[:, 128 * rr:128 * (rr + 1)]),
                        r(qk[:, DIM:512]),
                        start=(j == 0 and rr == 0),
                        stop=(j == NMT - 1 and rr == 1), skip_group_check=True)
                # l2 norms: fp16 squares + ones-matmul accumulation into (1,512)
                sq = p_sq.tile([128, 512], f16, tag="sq", name="sq")
                nc.scalar.activation(sq[:], qk[:].bitcast(f32), Square)
                nc.tensor.matmul(
                    ps_nqk[:], ones_c16[:], sq[:],
                    start=(j == 0), stop=(j == NMT - 1), skip_group_check=True)

            for jj in range(8):
                emit_conv(4 * jj)
                emit_conv(4 * jj + 1)
                emit_mtile(3 * jj)
                emit_conv(4 * jj + 2)
                emit_mtile(3 * jj + 1)
                emit_conv(4 * jj + 3)
                emit_mtile(3 * jj + 2)

        def emit_softmax(vk):
            s = state[vk]
            ps_sc, ps_nqk = s["sc"], s["nqk"]
            rqT, rZ = [], []
            E = [p_sm.tile([128, DIM], f32r, tag="e", name="e") for _ in range(2)]
            # 1/sqrt of the packed [nq | nk] row
            rrow = p_sm.tile([1, 512], f32, tag="rrow", name="rrow", bufs=2)
            nc.vector.reciprocal(rrow[:], ps_nqk[:])
            nc.scalar.activation(rrow[:], rrow[:], Sqrt)
            # rnq back to per-partition columns (+ temperature)
            for rr in range(2):
                pst = pp_pq.tile([128, 1], f32, tag="pq", name="pq")
                nc.tensor.transpose(pst[:], rrow[:, 128 * rr:128 * (rr + 1)], ident[0:1, 0:1])
                rqt = p_tn.tile([128, 1], f32, tag="rqt", name="rqt")
                nc.any.tensor_mul(rqt[:], pst[:], tmpv[rr][:])
                rqT.append(rqt)
            # rnk broadcast down partitions via outer product
            psb = pp_pq.tile([128, DIM], f32, tag="pq", name="pq")
            nc.tensor.matmul(psb[:], ones_row[:], rrow[:, DIM:512], start=True, stop=True)
            rkb = p_sm.tile([128, DIM], f32, tag="rkb", name="rkb", bufs=1)
            nc.any.tensor_copy(rkb[:], psb[:])
            # masked softmax, exp with fused row-sum
            for rr in range(2):
                sc = p_sm.tile([128, DIM], f32, tag="sc", name="sc")
                nc.any.tensor_scalar_mul(sc[:], ps_sc[rr][:], rqT[rr][:])
                nc.any.tensor_mul(sc[:], sc[:], rkb[:])
                nc.any.tensor_add(sc[:], sc[:], msk[rr][:])
                z = p_tn.tile([128, 1], f32, tag="z", name="z")
                nc.scalar.activation(E[rr][:], sc[:], Exp, accum_out=z[:])
                rz = p_tn.tile([128, 1], f32, tag="rz", name="rz")
                nc.vector.reciprocal(rz[:], z[:])
                rZ.append(rz)
            # Ahat = E / Z (rows)
            Ahat = [p_sm.tile([128, DIM], f32r, tag="ah", name="ah", bufs=3) for _ in range(2)]
            for rr in range(2):
                nc.any.tensor_scalar_mul(Ahat[rr][:], E[rr][:], rZ[rr][:])
            s["Ahat"] = Ahat

        def emit_out(vk):
            s = state[vk]
            b = s["b"]
            x_sb, Ahat = s["x"], s["Ahat"]
            # m1t[d,o] = (W_po @ Ahat)^T ; wch[c,o] = (W_po @ Ahat @ W_v)^T
            m1t = [p_sm.tile([128, DIM], f16, tag="m1t", name="m1t") for _ in range(2)]
            for d in range(2):
                ps = pp_pq.tile([128, DIM], f32, tag="pq", name="pq")
                for k in range(C128):
                    nc.tensor.matmul(
                        ps[:], r(Ahat[k][:, 128 * d:128 * (d + 1)]), r(wp[k][:]),
                        start=(k == 0), stop=(k == C128 - 1))
                nc.any.tensor_copy(m1t[d][:], ps[:])
            wch = [p_sm.tile([128, DIM], f16, tag="wch", name="wch") for _ in range(2)]
            for cb in range(2):
                ps = pp_pq.tile([128, DIM], f32, tag="pq", name="pq")
                for d in range(2):
                    nc.tensor.matmul(
                        ps[:], wv2[d][:, 128 * cb:128 * (cb + 1)], m1t[d][:],
                        start=(d == 0), stop=(d == 1))
                nc.any.tensor_copy(wch[cb][:], ps[:])
            # final = W_chain @ x, streamed out
            fin = [p_big.tile([128, M], f32, tag="big", name="big") for _ in range(C128)]
            for o in range(C128):
                for n in range(NM512):
                    ps = pp_pq.tile([128, 512], f32, tag="pq", name="pq")
                    for k in range(C128):
                        nc.tensor.matmul(
                            ps[:], wch[k][:, 128 * o:128 * (o + 1)],
                            x_sb[k][:, 512 * n:512 * (n + 1)],
                            start=(k == 0), stop=(k == C128 - 1))
                    nc.any.tensor_copy(fin[o][:, 512 * n:512 * (n + 1)], ps[:])
                    eng = nc.gpsimd if n % 2 == 0 else nc.sync
                    eng.dma_start(
                        out=od[b, 128 * o:128 * (o + 1), 512 * n:512 * (n + 1)],
                        in_=fin[o][:, 512 * n:512 * (n + 1)])

        # software pipeline: q1(b+1) fills the PE while batch b's softmax
        # chain runs; attnV/po of batch b are emitted after stream(b+1) so
        # they fill the next batch's softmax-chain PE idle
        sq_ = list(range(n_batches)) if seq is None else list(seq)
        vis = [(i, b) for i, b in enumerate(sq_)]
        emit_load_q1(0, vis[0][1])
        emit_stream(0)
        for i in range(1, len(vis) - 1):
            emit_load_q1(i, vis[i][1])
            emit_stream(i)
            emit_softmax(i - 1)
            emit_out(i - 1)
        n = len(vis)
        if n > 1:
            emit_load_q1(n - 1, vis[n - 1][1])
            emit_stream(n - 1)
            emit_softmax(n - 2)
            # both tail softmax chains run back-to-back on ACT/DVE while the
            # PE chews out(n-2); out copies no longer delay the last chain
            emit_softmax(n - 1)
            emit_out(n - 2)
            emit_out(n - 1)
        else:
            emit_softmax(0)
            emit_out(0)

    if split_waits:
        _split_waits(nc)
    return nc


def _get_nc():
    if "nc" not in _CACHE:
        _CACHE["nc"] = build_nc()
    return _CACHE["nc"]


def run(inputs, trace=False, trace_kwargs=None):
    from concourse.bass_utils import run_bass_kernel_spmd

    nc = _get_nc()
    consts = _host_consts(inputs["W_kv"], inputs["W_q"], inputs["W_dw"],
                          inputs["W_po"], inputs["temperature"])
    x = np.asarray(inputs["x"], np.float16)
    y = np.asarray(inputs["y"], np.float16).reshape(B, DIM, L)
    in_maps = []
    for i in range(NCORES):
        m = dict(consts)
        m["x_sh"] = np.ascontiguousarray(x[BL * i:BL * (i + 1)])
        m["y_sh"] = np.ascontiguousarray(y[BL * i:BL * (i + 1)])
        in_maps.append(m)
    res = run_bass_kernel_spmd(
        nc, in_maps, core_ids=list(range(NCORES)), trace=trace,
        trace_kwargs=trace_kwargs or {})
    out = np.concatenate([res.results[i]["out"] for i in range(NCORES)], axis=0)
    return out, res


def kernel(**inputs) -> np.ndarray:
    out, _ = run(inputs, trace=False)
    return out



# revision 49
# speedup vs baseline: 2.5068x; 2.5068x over previous
"""Trainium2 Bass kernel for nn_Attention1 (channel attention transformer block).

Reference computation (per batch):
  kv = W_kv @ x ; k, v = split(kv)                    # pointwise conv over m=3072
  q  = conv3x3(W_q @ y, W_dw)                         # 1x1 then full 3x3, 64x64 image
  q  = linear_interp(snake(q.flatten(HW)), 4096->3072)
  q, k = l2norm over m ; attn = softmax(q @ k^T * temp) per 32-channel head
  out = W_po @ (attn @ v)

Sharding: data-parallel over batch, 16 batches / 8 cores = 2 per core. SPMD,
no collectives; per-core outputs are concatenated on host.

Per-core kernel layout strategy. All heavy matmuls run in fp8(e4m3) with
perf_mode=DoubleRow: operands are laid out [128 partitions, 2, cols] so one
matmul contracts 256 channels (the pair dim sums in the PE), doubling PE
throughput vs fp16. fp8 is safe for everything that feeds the l2-normalized
q/k (scale and elementwise quantization wash out to ~0.1% at score level);
the final W_chain @ x matmul stays fp16 since it writes the output directly.
  - q path   : y kept as fp8 chan-pair images, zero-padded 66x64, three
               horizontally shifted copies; 3x3 conv as 9 DoubleRow matmuls
               (image stationary), both 128-chan blocks contracted at once.
               conv outputs for two row-tiles share one PSUM bank, one copy
               into a contiguous fp8 ct arena (32 x [128,256] slots)
  - snake+interp : fused sparse matrix S applied via one DoubleRow matmul
               per m-tile (S blocks repeat with period 3; the two
               contributing 128x128 blocks ride the pair dim)
  - kT       : x chan-pair fp8 stationary, W_k^T pair fp8 moving -> kT
               (m on partitions), same PSUM bank as interp (one copy)
  - scores   : qk tiles copied to fp8 pair arenas (two m-tiles per pair);
               q@kT via DoubleRow contracting 256 m-rows per matmul
  - norms    : gram blocks qk8[s]^T qk8[s] accumulate in PSUM; diagonal
               extracted with an eye mask (DVE) + ones-matmul -> packed
               [1,512] row of |q|^2,|k|^2 (replaces explicit squares)
  - softmax  : per-head masking via additive -30 mask over the full 256-wide
               score rows; exp on ScalarE with fused row-sum (accum_out);
               1/Z folded into Ahat as a per-row scale
  - out      : W_po @ A @ W_v folded into a 256x256 chain (f32r/f16 small
               matmuls), then W_chain @ x streamed in fp16; output DMA'd
               as f16 and upcast on host
"""
import numpy as np

HEADS = 8
B, DIM, M = 16, 256, 3072
HW = 64
L = HW * HW          # 4096 flattened conv spatial size
NCORES = 8
BL = B // NCORES     # batches per core
C128 = DIM // 128    # channel 128-tiles (2)
NM512 = M // 512     # m-dim 512-tiles (6)
NMT = M // 128       # m-dim 128-tiles (24)
NST = L // 128       # conv-spatial 128-tiles (32)

_CACHE = {}


def _f8(a):
    import ml_dtypes
    return np.asarray(a, np.float32).astype(ml_dtypes.float8_e4m3)


def _s_blocks():
    """Snake+interp as a sparse matrix; 6 distinct 128x128 blocks (period 3)."""
    mask = np.arange(L).reshape(HW, HW)
    mask[1::2] = mask[1::2][:, ::-1]
    mask = mask.reshape(-1)
    src = (np.arange(M) + 0.5) * (L / M) - 0.5
    src = np.maximum(src, 0.0)
    i0 = np.minimum(np.floor(src).astype(np.int64), L - 1)
    i1 = np.minimum(i0 + 1, L - 1)
    lam = (src - i0).astype(np.float32)
    S = np.zeros((L, M), np.float32)
    S[mask[i0], np.arange(M)] += (1 - lam)
    S[mask[i1], np.arange(M)] += lam
    blocks = np.zeros((6, 128, 128), np.float32)
    for j in range(3):
        s0 = (4 * j) // 3
        for t in range(2):
            blocks[j * 2 + t] = S[128 * (s0 + t):128 * (s0 + t + 1), 128 * j:128 * (j + 1)]
    # [128 l-part, 3 (j%3), 2 (pair t), 128 m]
    return blocks.transpose(1, 0, 2).reshape(128, 3, 2, 128).copy()


def _host_consts(W_kv, W_q, W_dw, W_po, temperature):
    c = np.arange(DIM)
    mask = np.where((c[:, None] // 32) == (c[None, :] // 32), 0.0, -30.0).astype(np.float32)
    tv = np.repeat(np.asarray(temperature, np.float32).reshape(HEADS), DIM // HEADS)
    # folded conv weights: (W_dw . W_q) -> [in-chan b, dy, dx, out o], then
    # chan-pair layout [128 p, 2 t, 9*256] with in-chan = p + 128 t
    fold = np.einsum("oayx,ab->byxo", np.asarray(W_dw, np.float32),
                     np.asarray(W_q, np.float32))
    wdw8 = fold.reshape(2, 128, 9 * DIM).transpose(1, 0, 2)
    # W_k^T chan-pair: [128 p, 2 t, 256 o] with in-chan = p + 128 t
    wk8 = W_kv[:DIM].T.reshape(2, 128, DIM).transpose(1, 0, 2)
    eye4 = np.concatenate([np.eye(128, dtype=np.float32)] * 4, axis=1)
    return {
        "w_k8": _f8(wk8),
        "w_v2": np.ascontiguousarray(W_kv[DIM:], np.float16),
        "w_poT": np.ascontiguousarray(W_po.T, np.float32),
        "w_dw8": _f8(wdw8),
        "s_mat8": _f8(_s_blocks()),
        "eye4": np.ascontiguousarray(eye4),
        "mask": np.ascontiguousarray(mask.reshape(2, 128, DIM)),
        "tempv": np.ascontiguousarray(tv.reshape(2, 128, 1)),
    }


def _make_tc_class():
    """TileContext subclass splitting the end-of-kernel drain waits.

    This container's walrus rejects >1 sem wait on CTRL-encoded instructions
    (Drain/NoOp). The stock Tile epilogue hangs every semaphore's final value
    on one Drain. Emit a chain of SP NoOps with one wait each instead, then a
    waitless drain: SP reaches it only after all sems hit their final values.
    """
    import bass_rust
    import concourse.mybir as mybir
    import concourse.tile as tile

    class SplitDrainTileContext(tile.TileContext):
        def _drain_and_barrier(self, tick_clock, wait_clock):
            probe = self.nc.sync.nop()
            wait_clock.add_sem_waits(
                probe.ins, bass_rust.ScopedClock({None: tick_clock.global_clock})
            )
            waits = list(probe.ins.sync_info.on_wait or [])
            probe.ins.sync_info.on_wait = waits[:1]
            for w in waits[1:]:
                n2 = self.nc.sync.nop()
                n2.ins.sync_info = mybir.SyncInfo(on_wait=[w], on_update=[])
            self.nc.sync.drain()
            self.nc.all_engine_barrier()
            assert self.sems is not None
            popped = self.nc._tile_sem_poison_stack.pop()
            assert popped is self._sem_poison
            self.nc.clear_and_free_semaphores(list(self.sems.allocated().values()))
            self.nc.all_engine_barrier()

    return SplitDrainTileContext


def _split_waits(nc):
    """Walrus in this container allows only one sem wait per instruction.
    Move extra waits onto same-engine NoOps inserted just before."""
    import concourse.mybir as mybir
    n = 0
    for f in nc.m.functions:
        for bb in f.blocks:
            out = []
            changed = False
            for inst in bb.instructions:
                si = inst.sync_info
                waits = list(si.on_wait) if si and si.on_wait else []
                if len(waits) > 1:
                    for w in waits[:-1]:
                        n += 1
                        nop = mybir.InstNoOp(name=f"I-sw{n}-{inst.name}", ins=[], outs=[])
                        nop.engine = inst.engine
                        nop.sync_info = mybir.SyncInfo(on_wait=[w], on_update=[])
                        out.append(nop)
                    si.on_wait = [waits[-1]]
                    changed = True
                out.append(inst)
            if changed:
                bb.instructions = out
    return n


def build_nc(split_waits=True, n_batches=BL, seq=None):
    from contextlib import ExitStack
    from collections import defaultdict
    import concourse.bass as bass
    import concourse.mybir as mybir
    from concourse.masks import make_identity

    f32 = mybir.dt.float32
    f32r = mybir.dt.float32r
    u32 = mybir.dt.uint32
    u8 = mybir.dt.uint8
    f16 = mybir.dt.float16
    f8 = mybir.dt.float8e4
    DR = mybir.MatmulPerfMode.DoubleRow
    Exp = mybir.ActivationFunctionType.Exp
    Sqrt = mybir.ActivationFunctionType.Sqrt

    def r(ap):
        return ap.bitcast(f32r)

    TC = _make_tc_class()
    nc = bass.Bass("TRN2", target_bir_lowering=False, debug=False)

    xd = nc.dram_tensor("x_sh", [BL, DIM, M], f16, kind="ExternalInput").ap()
    x8d = nc.dram_tensor("x8_sh", [BL, 128, 2, M], f8, kind="ExternalInput").ap()
    y8d = nc.dram_tensor("y8_sh", [BL, 128, 2, L], f8, kind="ExternalInput").ap()
    wkd = nc.dram_tensor("w_k8", [128, 2, DIM], f8, kind="ExternalInput").ap()
    wvd = nc.dram_tensor("w_v2", [DIM, DIM], f16, kind="ExternalInput").ap()
    wpd = nc.dram_tensor("w_poT", [DIM, DIM], f32r, kind="ExternalInput").ap()
    wdd = nc.dram_tensor("w_dw8", [128, 2, 9 * DIM], f8, kind="ExternalInput").ap()
    sd = nc.dram_tensor("s_mat8", [128, 3, 2, 128], f8, kind="ExternalInput").ap()
    eyd = nc.dram_tensor("eye4", [128, 512], f32, kind="ExternalInput").ap()
    md = nc.dram_tensor("mask", [2, 128, DIM], f32, kind="ExternalInput").ap()
    td = nc.dram_tensor("tempv", [2, 128, 1], f32, kind="ExternalInput").ap()
    od = nc.dram_tensor("out", [BL, DIM, M], f16, kind="ExternalOutput").ap()

    with TC(nc) as tc, ExitStack() as ctx:
        P = lambda **kw: ctx.enter_context(tc.tile_pool(**kw))
        consts = P(name="consts", bufs=1)
        p_qp = P(name="p_qp", bufs=2)
        p_x = P(name="p_x", bufs=2)
        p_ct = P(name="p_ct", bufs=2)
        p_qk = P(name="p_qk", bufs=4)
        p_sm = P(name="p_sm", bufs=2)
        p_tn = P(name="p_tn", bufs=4)
        p_fin = P(name="p_fin", bufs=6)
        # global PSUM pools: 2+2+1+1+2 = 8 banks exactly
        pp_pc = P(name="pp_pc", bufs=2, space="PSUM")
        pp_ik = P(name="pp_ik", bufs=2, space="PSUM")
        pp_sc = P(name="pp_sc", bufs=1, space="PSUM")
        pp_g4 = P(name="pp_g4", bufs=1, space="PSUM")
        pp_pq = P(name="pp_pq", bufs=2, space="PSUM")

        # ---- constants; the conv weights are split across all three DMA
        # queues (behind each queue's first image chunk) so the first conv
        # tile can start ~2.5us in; softmax/out-chain consts are emitted
        # after batch 0's loads ----
        wdw8 = consts.tile([128, 2, 9 * DIM], f8, tag="wdw8", name="wdw8")
        smat8 = consts.tile([128, 3, 2, 128], f8, tag="smat8", name="smat8")
        wk8 = consts.tile([128, 2, DIM], f8, tag="wk8", name="wk8")

        def early_consts():
            for dy, eng in ((0, nc.scalar), (1, nc.sync), (2, nc.gpsimd)):
                c0, c1 = 3 * DIM * dy, 3 * DIM * (dy + 1)
                eng.dma_start(out=wdw8[:, :, c0:c1], in_=wdd[:, :, c0:c1])
            nc.scalar.dma_start(out=smat8[:], in_=sd)
            nc.scalar.dma_start(out=wk8[:], in_=wkd)
        wv2 = [consts.tile([128, DIM], f16, tag=f"wv2{k}", name=f"wv2{k}") for k in range(C128)]
        wp = [consts.tile([128, DIM], f32r, tag=f"wp{k}", name=f"wp{k}") for k in range(C128)]
        eye4 = consts.tile([128, 512], f32, tag="eye4", name="eye4")
        msk = [consts.tile([128, DIM], f32, tag=f"msk{k}", name=f"msk{k}") for k in range(2)]
        tmpv = [consts.tile([128, 1], f32, tag=f"tmpv{k}", name=f"tmpv{k}") for k in range(2)]

        def late_consts():
            for k in range(C128):
                sl = slice(128 * k, 128 * (k + 1))
                nc.sync.dma_start(out=wv2[k][:], in_=wvd[sl, :])
                nc.sync.dma_start(out=wp[k][:], in_=wpd[sl, :])
            nc.sync.dma_start(out=eye4[:], in_=eyd)
            for rr in range(2):
                nc.sync.dma_start(out=msk[rr][:], in_=md[rr])
                nc.sync.dma_start(out=tmpv[rr][:], in_=td[rr])
        ident = consts.tile([128, 128], f32, tag="ident", name="ident")
        make_identity(nc, ident[:])
        ones_row = consts.tile([1, 128], f32, tag="ones", name="ones")
        nc.vector.memset(ones_row[:], 1.0)
        ones_c16 = consts.tile([128, 1], f16, tag="ones16", name="ones16")
        nc.vector.memset(ones_c16[:], 1.0)

        state = defaultdict(dict)

        def emit_load_q1(vk, b, after_c0=None):
            s = state[vk]
            s["b"] = b
            s["x8"] = p_x.tile([128, 2, M], f8, tag="x8", name="x8")
            # W_q is folded into the conv weights on the host, so the conv
            # input is y itself: one DRAM DMA for the centered fp8 chan-pair
            # image (128, 2, 66, 64) with zeroed border rows, then two
            # on-chip shifted copies (dx-1 = -1/+1) built by SBUF->SBUF DMA;
            # the row-wrap artifacts land exactly on the edge columns the
            # memsets zero afterwards. Chunked x2 so early conv tiles start
            # before the whole image lands.
            qsh = [p_qp.tile([128, 2, HW + 2, HW], f8, tag=f"qsh{dx}",
                             name=f"qsh{dx}") for dx in range(3)]
            s["qsh"] = qsh
            tf = [qsh[dx].rearrange("p t a b -> p t (a b)") for dx in range(3)]
            # two half-image DMAs per shifted copy straight from DRAM, one
            # copy per queue, all three queues in parallel (subtile deps let
            # conv groups start as soon as the rows they read have landed)
            HC = L // 2
            nc.sync.dma_start(out=tf[1][:, :, HW:HW + HC], in_=y8d[b][:, :, 0:HC])
            # qsh0[65+l] = y[l]
            nc.scalar.dma_start(out=tf[0][:, :, HW + 1:HW + 1 + HC],
                                in_=y8d[b][:, :, 0:HC])
            # qsh2[64+l] = y[l+1]  (tail col is memset)
            nc.gpsimd.dma_start(out=tf[2][:, :, HW:HW + HC],
                                in_=y8d[b][:, :, 1:HC + 1])
            if after_c0 is not None:
                after_c0()
            # second halves + x8: keep the hwdge ACT queue clear (it carries
            # most PSUM->SBUF copies); late image halves ride Pool's swdge
            nc.sync.dma_start(out=tf[1][:, :, HW + HC:HW + L],
                              in_=y8d[b][:, :, HC:L])
            for cc in range(2):
                nc.gpsimd.dma_start(
                    out=s["x8"][:, :, 1536 * cc:1536 * (cc + 1)],
                    in_=x8d[b][:, :, 1536 * cc:1536 * (cc + 1)])
            nc.gpsimd.dma_start(out=tf[0][:, :, HW + 1 + HC:HW + 1 + L],
                                in_=y8d[b][:, :, HC:L])
            nc.gpsimd.dma_start(out=tf[2][:, :, HW + HC:HW + L - 1],
                                in_=y8d[b][:, :, HC + 1:L])
            # memset order matters: the DVE FIFO is in-order, so everything
            # the first conv groups need (row 0, first-half edge columns)
            # is emitted before anything gated on the second-half DMAs
            for dx in range(3):
                nc.vector.memset(qsh[dx][:, :, 0:1, :].bitcast(u32), 0)
            for dx in (0, 2):
                col = 0 if dx == 0 else HW - 1
                nc.vector.memset(
                    qsh[dx].bitcast(u8)[:, :, 1:1 + HC // HW, col:col + 1], 0)
            for dx in range(3):
                t = qsh[dx]
                nc.vector.memset(t[:, :, HW + 1:HW + 2, :].bitcast(u32), 0)
                if dx != 1:
                    col = 0 if dx == 0 else HW - 1
                    nc.vector.memset(
                        t.bitcast(u8)[:, :, 1 + HC // HW:HW + 1, col:col + 1], 0)
            s["x"] = [p_x.tile([128, M], f16, tag=f"x{k}", name="x") for k in range(C128)]
            for k in range(C128):
                nc.sync.dma_start(out=s["x"][k][:],
                                  in_=xd[b, 128 * k:128 * (k + 1), :])

        def emit_stream(vk, hooks=()):
            s = state[vk]
            qsh, x8t = s["qsh"], s["x8"]
            ps_scc = pp_sc.tile([128, 512], f32, tag="pscc", name="pscc")
            ps_sc = [ps_scc[:, 0:DIM], ps_scc[:, DIM:512]]
            ps_g4 = pp_g4.tile([128, 512], f32, tag="g4", name="g4")
            s["sc"], s["scc"], s["g4"] = ps_sc, ps_scc, ps_g4
            ct8 = p_ct.tile([128, NST, DIM], f8, tag="ct8", name="ct8")
            flats = [qsh[dx].rearrange("p t a b -> p t (a b)") for dx in range(3)]
            qps = s["qps"] = []

            def emit_conv(j2):
                half = j2 % 2
                if half == 0:
                    s["pc"] = pp_pc.tile([128, 512], f32, tag="pc", name="pc")
                ps = s["pc"][:, DIM * half:DIM * (half + 1)]
                for dy in range(3):
                    for dx in range(3):
                        off = (2 * j2 + dy) * HW
                        nc.tensor.matmul(
                            ps, flats[dx][:, :, off:off + 128],
                            wdw8[:, :, (dy * 3 + dx) * DIM:(dy * 3 + dx + 1) * DIM],
                            start=(dy == 0 and dx == 0),
                            stop=(dy == 2 and dx == 2),
                            perf_mode=DR, skip_group_check=True)
                if half == 1:
                    # GPSIMD can't read PSUM on hw: alternate DVE/ACT
                    dst = ct8[:, j2 - 1:j2 + 1, :].rearrange("p a b -> p (a b)")
                    if (j2 // 2) % 2 == 0:
                        nc.vector.tensor_copy(dst, s["pc"][:])
                    else:
                        nc.scalar.copy(dst, s["pc"][:])

            def emit_mtile(j):
                s0 = (4 * j) // 3
                psik = pp_ik.tile([128, 512], f32, tag="pik", name="pik")
                # interp into [0:256] and kT into [256:512] of one PSUM bank
                # (per-element has_written drives overwrite-then-accumulate)
                nc.tensor.matmul(
                    psik[:, 0:DIM], smat8[:, j % 3], ct8[:, s0:s0 + 2, :],
                    start=True, stop=False, perf_mode=DR, skip_group_check=True)
                nc.tensor.matmul(
                    psik[:, DIM:512], x8t[:, :, 128 * j:128 * (j + 1)], wk8[:],
                    start=False, stop=True, perf_mode=DR, skip_group_check=True)
                if j % 2 == 0:
                    qps.append(p_qk.tile([128, 2, 512], f8, tag="qk8", name="qk8"))
                    nc.vector.tensor_copy(qps[-1][:, 0, :], psik[:])
                else:
                    nc.scalar.copy(qps[-1][:, 1, :], psik[:])

            def emit_scores(p):
                # lagged by a conv group so the qk8 copies are long done
                qp = qps[p]
                for rr in range(2):
                    nc.tensor.matmul(
                        ps_sc[rr], qp[:, :, 128 * rr:128 * (rr + 1)],
                        qp[:, :, DIM:512],
                        start=(p == 0 and rr == 0),
                        stop=(p == NMT // 2 - 1 and rr == 1),
                        perf_mode=DR, skip_group_check=True)
                # gram blocks for l2 norms (diag extracted in softmax)
                for g in range(4):
                    nc.tensor.matmul(
                        ps_g4[:, 128 * g:128 * (g + 1)],
                        qp[:, :, 128 * g:128 * (g + 1)],
                        qp[:, :, 128 * g:128 * (g + 1)],
                        start=(p == 0), stop=(p == NMT // 2 - 1),
                        perf_mode=DR, skip_group_check=True)

            # conv group jj; mtiles lag one group; scores lag mtiles so every
            # PE instruction's copy-dependencies resolved a whole group ago
            ndone = 0
            for jj in range(8):
                for q in range(4):
                    emit_conv(4 * jj + q)
                if jj < len(hooks):
                    hooks[jj]()
                if jj >= 1:
                    # pairs whose last mtile was emitted a full group ago
                    ready = max(0, (3 * jj - 3) // 2)
                    while ndone < ready:
                        emit_scores(ndone)
                        ndone += 1
                    for t in range(3):
                        emit_mtile(3 * (jj - 1) + t)
            for t in range(3):
                emit_mtile(21 + t)

            def tail_scores(nd=ndone):
                for p in range(nd, NMT // 2):
                    emit_scores(p)

            # defer the last score pairs: their qk8 copies were just issued,
            # so the caller schedules them under later PE work
            s["tail_fn"] = tail_scores

        def softmax_parts(vk):
            """Softmax chain as three emission hooks, interleavable with the
            next batch's conv groups so the chain latency hides under them.
            Part 1 frees the pscc/g4 PSUM banks early (SBUF copies)."""
            s = state[vk]
            rqT, rZ = [], []

            def part1():
                s["tail_fn"]()  # this batch's deferred score pairs
                ps_g4 = s["g4"]
                # scores to SBUF, freeing the bank for the next stream
                scp = p_sm.tile([128, 512], f32, tag="scp", name="scp")
                s["scp"] = scp
                nc.vector.tensor_copy(scp[:], s["scc"][:])
                # gram diagonals -> packed [1,512] row [nq0 nq1 nk0 nk1]
                geye = p_sm.tile([128, 512], f16, tag="geye", name="geye")
                nc.vector.tensor_mul(geye[:], ps_g4[:], eye4[:])
                ps_nqk = pp_pq.tile([1, 512], f32, tag="pq", name="pq")
                nc.tensor.matmul(ps_nqk[:], ones_c16[:], geye[:], start=True, stop=True)
                # 1/sqrt of the packed [nq | nk] row
                rrow = p_sm.tile([1, 512], f32, tag="rrow", name="rrow", bufs=2)
                s["rrow"] = rrow
                nc.vector.reciprocal(rrow[:], ps_nqk[:])
                nc.scalar.activation(rrow[:], rrow[:], Sqrt)

            def part2():
                rrow = s["rrow"]
                # rnq back to per-partition columns (+ temperature)
                for rr in range(2):
                    pst = pp_pq.tile([128, 1], f32, tag="pq", name="pq")
                    nc.tensor.transpose(pst[:], rrow[:, 128 * rr:128 * (rr + 1)],
                                        ident[0:1, 0:1])
                    rqt = p_tn.tile([128, 1], f32, tag="rqt", name="rqt")
                    nc.vector.tensor_mul(rqt[:], pst[:], tmpv[rr][:])
                    rqT.append(rqt)
                # rnk broadcast down partitions via outer product
                psb = pp_pq.tile([128, DIM], f32, tag="pq", name="pq")
                nc.tensor.matmul(psb[:], ones_row[:], rrow[:, DIM:512],
                                 start=True, stop=True)
                rkb = p_sm.tile([128, DIM], f32, tag="rkb", name="rkb")
                s["rkb"] = rkb
                nc.scalar.copy(rkb[:], psb[:])

            def part3():
                scp, rkb = s["scp"], s["rkb"]
                E = [p_sm.tile([128, DIM], f32, tag="e", name="e") for _ in range(2)]
                # masked softmax, exp with fused row-sum
                for rr in range(2):
                    sc = p_sm.tile([128, DIM], f32, tag="sc", name="sc")
                    nc.vector.tensor_scalar_mul(
                        sc[:], scp[:, DIM * rr:DIM * (rr + 1)], rqT[rr][:])
                    nc.vector.tensor_mul(sc[:], sc[:], rkb[:])
                    nc.vector.tensor_add(sc[:], sc[:], msk[rr][:])
                    z = p_tn.tile([128, 1], f32, tag="z", name="z")
                    nc.scalar.activation(E[rr][:], sc[:], Exp, accum_out=z[:])
                    rz = p_tn.tile([128, 1], f32, tag="rz", name="rz")
                    nc.vector.reciprocal(rz[:], z[:])
                    rZ.append(rz)
                # Ahat = E / Z (rows)
                Ahat = [p_sm.tile([128, DIM], f32r, tag="ah", name="ah", bufs=3)
                        for _ in range(2)]
                for rr in range(2):
                    nc.vector.tensor_scalar_mul(Ahat[rr][:], E[rr][:], rZ[rr][:])
                s["Ahat"] = Ahat

            return (part1, part2, part3)

        def emit_softmax(vk):
            for p in softmax_parts(vk):
                p()

        def emit_out(vk, hooks=()):
            s = state[vk]
            b = s["b"]
            x_sb, Ahat = s["x"], s["Ahat"]
            # m1t[d,o] = (W_po @ Ahat)^T ; wch[c,o] = (W_po @ Ahat @ W_v)^T
            m1t = [p_sm.tile([128, DIM], f16, tag="m1t", name="m1t") for _ in range(2)]
            for d in range(2):
                ps = pp_pq.tile([128, DIM], f32, tag="pq", name="pq")
                for k in range(C128):
                    nc.tensor.matmul(
                        ps[:], r(Ahat[k][:, 128 * d:128 * (d + 1)]), r(wp[k][:]),
                        start=(k == 0), stop=(k == C128 - 1))
                nc.vector.tensor_copy(m1t[d][:], ps[:])
            wch = [p_sm.tile([128, DIM], f16, tag="wch", name="wch") for _ in range(2)]
            for cb in range(2):
                ps = pp_pq.tile([128, DIM], f32, tag="pq", name="pq")
                for d in range(2):
                    nc.tensor.matmul(
                        ps[:], wv2[d][:, 128 * cb:128 * (cb + 1)], m1t[d][:],
                        start=(d == 0), stop=(d == 1))
                nc.scalar.copy(wch[cb][:], ps[:])
            if len(hooks) > 0:
                hooks[0]()
            # final = W_chain @ x, staged as f16 in 1024-wide tiles (fewer,
            # bigger out-DMAs)
            hs = [(o, h) for o in range(C128) for h in range(NM512 // 2)]
            for i, (o, h) in enumerate(hs):
                st = p_fin.tile([128, 1024], f16, tag="fin", name="fin")
                for half in range(2):
                    n = 2 * h + half
                    # alternate PSUM between the pq pool and the ik pool
                    # (idle once the stream is done) for 4-deep pipelining
                    pool = pp_pq if (2 * i + half) % 2 == 0 else pp_ik
                    tg = "pq" if pool is pp_pq else "pik"
                    ps = pool.tile([128, 512], f32, tag=tg, name=tg)
                    for k in range(C128):
                        nc.tensor.matmul(
                            ps[:], wch[k][:, 128 * o:128 * (o + 1)],
                            x_sb[k][:, 512 * n:512 * (n + 1)],
                            start=(k == 0), stop=(k == C128 - 1))
                    if half == 0:
                        nc.scalar.copy(st[:, 0:512], ps[:])
                    else:
                        nc.vector.tensor_copy(st[:, 512:1024], ps[:])
                deng = nc.sync if h % 2 == 0 else nc.gpsimd
                deng.dma_start(
                    out=od[b, 128 * o:128 * (o + 1), 1024 * h:1024 * (h + 1)],
                    in_=st[:])
                if i + 1 < len(hooks):
                    hooks[i + 1]()

        # software pipeline: q1(b+1) fills the PE while batch b's softmax
        # chain runs *inside* stream(b+1) via hooks (its tiny PE ops slot
        # between conv groups; part 1 frees the score/gram PSUM banks before
        # stream(b+1)'s first scores matmul needs them)
        sq_ = list(range(n_batches)) if seq is None else list(seq)
        vis = [(i, b) for i, b in enumerate(sq_)]
        n = len(vis)
        emit_load_q1(0, vis[0][1], after_c0=early_consts)
        emit_stream(0)
        late_consts()
        for i in range(1, n):
            emit_load_q1(i, vis[i][1])
            emit_stream(i, hooks=softmax_parts(i - 1))
            if i >= 2:
                emit_out(i - 2)
        if n > 1:
            # tail: last softmax chain hides under out(n-2)'s PE stream
            emit_out(n - 2, hooks=softmax_parts(n - 1))
            emit_out(n - 1)
        else:
            emit_softmax(0)
            emit_out(0)

    if split_waits:
        _split_waits(nc)
    return nc


def _get_nc():
    if "nc" not in _CACHE:
        _CACHE["nc"] = build_nc()
    return _CACHE["nc"]


def make_inputs(inputs):
    """Host-side prep: consts + per-core sharded activations."""
    consts = _host_consts(inputs["W_kv"], inputs["W_q"], inputs["W_dw"],
                          inputs["W_po"], inputs["temperature"])
    x = np.asarray(inputs["x"], np.float16)
    x8 = _f8(inputs["x"]).reshape(B, 2, 128, M).transpose(0, 2, 1, 3)
    y8 = _f8(inputs["y"]).reshape(B, 2, 128, L).transpose(0, 2, 1, 3)
    in_maps = []
    for i in range(NCORES):
        m = dict(consts)
        m["x_sh"] = np.ascontiguousarray(x[BL * i:BL * (i + 1)])
        m["x8_sh"] = np.ascontiguousarray(x8[BL * i:BL * (i + 1)])
        m["y8_sh"] = np.ascontiguousarray(y8[BL * i:BL * (i + 1)])
        in_maps.append(m)
    return in_maps


def run(inputs, trace=False, trace_kwargs=None):
    from concourse.bass_utils import run_bass_kernel_spmd

    nc = _get_nc()
    in_maps = make_inputs(inputs)
    res = run_bass_kernel_spmd(
        nc, in_maps, core_ids=list(range(NCORES)), trace=trace,
        trace_kwargs=trace_kwargs or {})
    out = np.concatenate(
        [np.asarray(res.results[i]["out"], np.float32) for i in range(NCORES)],
        axis=0)
    return out, res


def kernel(**inputs) -> np.ndarray:
    out, _ = run(inputs, trace=False)
    return out


# revision 74
# speedup vs baseline: 2.5954x; 1.0353x over previous
"""Trainium2 Bass kernel for nn_Attention1 (channel attention transformer block).

Reference computation (per batch):
  kv = W_kv @ x ; k, v = split(kv)                    # pointwise conv over m=3072
  q  = conv3x3(W_q @ y, W_dw)                         # 1x1 then full 3x3, 64x64 image
  q  = linear_interp(snake(q.flatten(HW)), 4096->3072)
  q, k = l2norm over m ; attn = softmax(q @ k^T * temp) per 32-channel head
  out = W_po @ (attn @ v)

Sharding: data-parallel over batch, 16 batches / 8 cores = 2 per core. SPMD,
no collectives; per-core outputs are concatenated on host.

Per-core kernel layout strategy. All heavy matmuls run in fp8(e4m3) with
perf_mode=DoubleRow: operands are laid out [128 partitions, 2, cols] so one
matmul contracts 256 channels (the pair dim sums in the PE), doubling PE
throughput vs fp16. fp8 is safe for everything that feeds the l2-normalized
q/k (scale and elementwise quantization wash out to ~0.1% at score level);
the final W_chain @ x matmul stays fp16 since it writes the output directly.
  - q path   : y kept as fp8 chan-pair images, zero-padded 66x64, three
               horizontally shifted copies; 3x3 conv as 9 DoubleRow matmuls
               (image stationary), both 128-chan blocks contracted at once.
               conv outputs for two row-tiles share one PSUM bank, one copy
               into a contiguous fp8 ct arena (32 x [128,256] slots)
  - snake+interp : fused sparse matrix S applied via one DoubleRow matmul
               per m-tile (S blocks repeat with period 3; the two
               contributing 128x128 blocks ride the pair dim)
  - kT       : x chan-pair fp8 stationary, W_k^T pair fp8 moving -> kT
               (m on partitions), same PSUM bank as interp (one copy)
  - scores   : qk tiles copied to fp8 pair arenas (two m-tiles per pair);
               q@kT via DoubleRow contracting 256 m-rows per matmul
  - norms    : gram blocks qk8[s]^T qk8[s] accumulate in PSUM; diagonal
               extracted with an eye mask (DVE) + ones-matmul -> packed
               [1,512] row of |q|^2,|k|^2 (replaces explicit squares)
  - softmax  : per-head masking via additive -30 mask over the full 256-wide
               score rows; exp on ScalarE with fused row-sum (accum_out);
               1/Z folded into Ahat as a per-row scale
  - out      : W_po @ A @ W_v folded into a 256x256 chain (f32r/f16 small
               matmuls), then W_chain @ x streamed in fp16; output DMA'd
               as f16 and upcast on host
"""
import numpy as np

HEADS = 8
B, DIM, M = 16, 256, 3072
HW = 64
L = HW * HW          # 4096 flattened conv spatial size
NCORES = 8
BL = B // NCORES     # batches per core
C128 = DIM // 128    # channel 128-tiles (2)
NM512 = M // 512     # m-dim 512-tiles (6)
NMT = M // 128       # m-dim 128-tiles (24)
NST = L // 128       # conv-spatial 128-tiles (32)

_CACHE = {}


def _f8(a):
    import ml_dtypes
    return np.asarray(a, np.float32).astype(ml_dtypes.float8_e4m3)


def _f8e5(a):
    import ml_dtypes
    return np.asarray(a, np.float32).astype(ml_dtypes.float8_e5m2)


def _s_blocks():
    """Snake+interp as a sparse matrix; 6 distinct 128x128 blocks (period 3)."""
    mask = np.arange(L).reshape(HW, HW)
    mask[1::2] = mask[1::2][:, ::-1]
    mask = mask.reshape(-1)
    src = (np.arange(M) + 0.5) * (L / M) - 0.5
    src = np.maximum(src, 0.0)
    i0 = np.minimum(np.floor(src).astype(np.int64), L - 1)
    i1 = np.minimum(i0 + 1, L - 1)
    lam = (src - i0).astype(np.float32)
    S = np.zeros((L, M), np.float32)
    S[mask[i0], np.arange(M)] += (1 - lam)
    S[mask[i1], np.arange(M)] += lam
    blocks = np.zeros((6, 128, 128), np.float32)
    for j in range(3):
        s0 = (4 * j) // 3
        for t in range(2):
            blocks[j * 2 + t] = S[128 * (s0 + t):128 * (s0 + t + 1), 128 * j:128 * (j + 1)]
    # [128 l-part, 3 (j%3), 2 (pair t), 128 m]
    return blocks.transpose(1, 0, 2).reshape(128, 3, 2, 128).copy()


def _host_consts(W_kv, W_q, W_dw, W_po, temperature):
    c = np.arange(DIM)
    mask = np.where((c[:, None] // 32) == (c[None, :] // 32), 0.0, -30.0).astype(np.float32)
    tv = np.repeat(np.asarray(temperature, np.float32).reshape(HEADS), DIM // HEADS)
    # folded conv weights: (W_dw . W_q) -> [in-chan b, dy, dx, out o], then
    # chan-pair layout [128 p, 2 t, 9*256] with in-chan = p + 128 t
    fold = np.einsum("oayx,ab->byxo", np.asarray(W_dw, np.float32),
                     np.asarray(W_q, np.float32))
    wdw8 = fold.reshape(2, 128, 9 * DIM).transpose(1, 0, 2)
    # W_k^T chan-pair: [128 p, 2 t, 256 o] with in-chan = p + 128 t
    wk8 = W_kv[:DIM].T.reshape(2, 128, DIM).transpose(1, 0, 2)
    eye4 = np.concatenate([np.eye(128, dtype=np.float32)] * 4, axis=1)
    return {
        "w_k8": _f8(wk8),
        "w_v2": np.ascontiguousarray(W_kv[DIM:], np.float16),
        "w_poT": np.ascontiguousarray(W_po.T, np.float32),
        "w_dw8": _f8(wdw8),
        "s_mat8": _f8(_s_blocks()),
        "eye4": np.ascontiguousarray(eye4),
        "mask": np.ascontiguousarray(mask.reshape(2, 128, DIM)),
        "tempv": np.ascontiguousarray(tv.reshape(2, 128, 1)),
    }


def _make_tc_class():
    """TileContext subclass splitting the end-of-kernel drain waits.

    This container's walrus rejects >1 sem wait on CTRL-encoded instructions
    (Drain/NoOp). The stock Tile epilogue hangs every semaphore's final value
    on one Drain. Emit a chain of SP NoOps with one wait each instead, then a
    waitless drain: SP reaches it only after all sems hit their final values.
    """
    import bass_rust
    import concourse.mybir as mybir
    import concourse.tile as tile

    class SplitDrainTileContext(tile.TileContext):
        def _drain_and_barrier(self, tick_clock, wait_clock):
            probe = self.nc.sync.nop()
            wait_clock.add_sem_waits(
                probe.ins, bass_rust.ScopedClock({None: tick_clock.global_clock})
            )
            waits = list(probe.ins.sync_info.on_wait or [])
            probe.ins.sync_info.on_wait = waits[:1]
            for w in waits[1:]:
                n2 = self.nc.sync.nop()
                n2.ins.sync_info = mybir.SyncInfo(on_wait=[w], on_update=[])
            self.nc.sync.drain()
            self.nc.all_engine_barrier()
            assert self.sems is not None
            popped = self.nc._tile_sem_poison_stack.pop()
            assert popped is self._sem_poison
            self.nc.clear_and_free_semaphores(list(self.sems.allocated().values()))
            self.nc.all_engine_barrier()

    return SplitDrainTileContext


def _split_waits(nc):
    """Walrus in this container allows only one sem wait per instruction.
    Move extra waits onto same-engine NoOps inserted just before."""
    import concourse.mybir as mybir
    n = 0
    for f in nc.m.functions:
        for bb in f.blocks:
            out = []
            changed = False
            for inst in bb.instructions:
                si = inst.sync_info
                waits = list(si.on_wait) if si and si.on_wait else []
                if len(waits) > 1:
                    for w in waits[:-1]:
                        n += 1
                        nop = mybir.InstNoOp(name=f"I-sw{n}-{inst.name}", ins=[], outs=[])
                        nop.engine = inst.engine
                        nop.sync_info = mybir.SyncInfo(on_wait=[w], on_update=[])
                        out.append(nop)
                    si.on_wait = [waits[-1]]
                    changed = True
                out.append(inst)
            if changed:
                bb.instructions = out
    return n


def build_nc(split_waits=True, n_batches=BL, seq=None):
    from contextlib import ExitStack
    from collections import defaultdict
    import concourse.bass as bass
    import concourse.mybir as mybir
    from concourse.masks import make_identity

    f32 = mybir.dt.float32
    f32r = mybir.dt.float32r
    u32 = mybir.dt.uint32
    u8 = mybir.dt.uint8
    f16 = mybir.dt.float16
    f8 = mybir.dt.float8e4
    DR = mybir.MatmulPerfMode.DoubleRow
    Exp = mybir.ActivationFunctionType.Exp
    Ln = mybir.ActivationFunctionType.Ln

    def r(ap):
        return ap.bitcast(f32r)

    TC = _make_tc_class()
    nc = bass.Bass("TRN2", target_bir_lowering=False, debug=False)

    f8e5 = mybir.dt.float8e5
    x8d = nc.dram_tensor("x8_sh", [BL, 128, 2, M], f8, kind="ExternalInput").ap()
    x5d = nc.dram_tensor("x5_sh", [BL, 128, 2, M], f8e5, kind="ExternalInput").ap()
    y8d = nc.dram_tensor("y8_sh", [BL, 128, 2, L], f8, kind="ExternalInput").ap()
    wkd = nc.dram_tensor("w_k8", [128, 2, DIM], f8, kind="ExternalInput").ap()
    wvd = nc.dram_tensor("w_v2", [DIM, DIM], f16, kind="ExternalInput").ap()
    wpd = nc.dram_tensor("w_poT", [DIM, DIM], f32r, kind="ExternalInput").ap()
    wdd = nc.dram_tensor("w_dw8", [128, 2, 9 * DIM], f8, kind="ExternalInput").ap()
    sd = nc.dram_tensor("s_mat8", [128, 3, 2, 128], f8, kind="ExternalInput").ap()
    eyd = nc.dram_tensor("eye4", [128, 512], f32, kind="ExternalInput").ap()
    md = nc.dram_tensor("mask", [2, 128, DIM], f32, kind="ExternalInput").ap()
    td = nc.dram_tensor("tempv", [2, 128, 1], f32, kind="ExternalInput").ap()
    od = nc.dram_tensor("out", [BL, DIM, M], f16, kind="ExternalOutput").ap()

    with TC(nc) as tc, ExitStack() as ctx:
        P = lambda **kw: ctx.enter_context(tc.tile_pool(**kw))
        consts = P(name="consts", bufs=1)
        p_qp = P(name="p_qp", bufs=2)
        p_x = P(name="p_x", bufs=2)
        p_ct = P(name="p_ct", bufs=2)
        p_qk = P(name="p_qk", bufs=4)
        p_sm = P(name="p_sm", bufs=2)
        p_tn = P(name="p_tn", bufs=4)
        p_fin = P(name="p_fin", bufs=6)
        # global PSUM pools: 2+2+1+1+2 = 8 banks exactly
        pp_pc = P(name="pp_pc", bufs=2, space="PSUM")
        pp_ik = P(name="pp_ik", bufs=2, space="PSUM")
        pp_sc = P(name="pp_sc", bufs=1, space="PSUM")
        pp_g4 = P(name="pp_g4", bufs=1, space="PSUM")
        pp_pq = P(name="pp_pq", bufs=2, space="PSUM")

        # ---- constants; the conv weights are split across all three DMA
        # queues (behind each queue's first image chunk) so the first conv
        # tile can start ~2.5us in; softmax/out-chain consts are emitted
        # after batch 0's loads ----
        wdw8 = consts.tile([128, 2, 9 * DIM], f8, tag="wdw8", name="wdw8")
        smat8 = consts.tile([128, 3, 2, 128], f8, tag="smat8", name="smat8")
        wk8 = consts.tile([128, 2, DIM], f8, tag="wk8", name="wk8")

        def early_consts():
            for dy, eng in ((0, nc.scalar), (1, nc.sync), (2, nc.gpsimd)):
                c0, c1 = 3 * DIM * dy, 3 * DIM * (dy + 1)
                eng.dma_start(out=wdw8[:, :, c0:c1], in_=wdd[:, :, c0:c1])
            nc.scalar.dma_start(out=smat8[:], in_=sd)
            nc.scalar.dma_start(out=wk8[:], in_=wkd)
        wv2 = [consts.tile([128, DIM], f16, tag=f"wv2{k}", name=f"wv2{k}") for k in range(C128)]
        wp = [consts.tile([128, DIM], f32r, tag=f"wp{k}", name=f"wp{k}") for k in range(C128)]
        eye4 = consts.tile([128, 512], f32, tag="eye4", name="eye4")
        msk = [consts.tile([128, DIM], f32, tag=f"msk{k}", name=f"msk{k}") for k in range(2)]
        tmpv = [consts.tile([128, 1], f32, tag=f"tmpv{k}", name=f"tmpv{k}") for k in range(2)]

        def late_consts():
            for k in range(C128):
                sl = slice(128 * k, 128 * (k + 1))
                nc.sync.dma_start(out=wv2[k][:], in_=wvd[sl, :])
                nc.sync.dma_start(out=wp[k][:], in_=wpd[sl, :])
            nc.sync.dma_start(out=eye4[:], in_=eyd)
            for rr in range(2):
                nc.sync.dma_start(out=msk[rr][:], in_=md[rr])
                nc.sync.dma_start(out=tmpv[rr][:], in_=td[rr])
        ident = consts.tile([128, 128], f32, tag="ident", name="ident")
        make_identity(nc, ident[:])
        ones_row = consts.tile([1, 128], f32, tag="ones", name="ones")
        nc.vector.memset(ones_row[:], 1.0)
        ones_c16 = consts.tile([128, 1], f16, tag="ones16", name="ones16")
        nc.vector.memset(ones_c16[:], 1.0)

        state = defaultdict(dict)

        def emit_load_q1(vk, b, after_c0=None):
            s = state[vk]
            s["b"] = b
            s["x8"] = p_x.tile([128, 2, M], f8, tag="x8", name="x8")
            # W_q is folded into the conv weights on the host, so the conv
            # input is y itself: one DRAM DMA for the centered fp8 chan-pair
            # image (128, 2, 66, 64) with zeroed border rows, then two
            # on-chip shifted copies (dx-1 = -1/+1) built by SBUF->SBUF DMA;
            # the row-wrap artifacts land exactly on the edge columns the
            # memsets zero afterwards. Chunked x2 so early conv tiles start
            # before the whole image lands.
            qsh = [p_qp.tile([128, 2, HW + 2, HW], f8, tag=f"qsh{dx}",
                             name=f"qsh{dx}") for dx in range(3)]
            s["qsh"] = qsh
            tf = [qsh[dx].rearrange("p t a b -> p t (a b)") for dx in range(3)]
            # two half-image DMAs per shifted copy straight from DRAM, one
            # copy per queue, all three queues in parallel (subtile deps let
            # conv groups start as soon as the rows they read have landed)
            HC = L // 2
            nc.sync.dma_start(out=tf[1][:, :, HW:HW + HC], in_=y8d[b][:, :, 0:HC])
            # qsh0[65+l] = y[l]
            nc.scalar.dma_start(out=tf[0][:, :, HW + 1:HW + 1 + HC],
                                in_=y8d[b][:, :, 0:HC])
            # qsh2[64+l] = y[l+1]  (tail col is memset)
            nc.gpsimd.dma_start(out=tf[2][:, :, HW:HW + HC],
                                in_=y8d[b][:, :, 1:HC + 1])
            if after_c0 is not None:
                after_c0()
            # second halves + x8: keep the hwdge ACT queue clear (it carries
            # most PSUM->SBUF copies); late image halves ride Pool's swdge
            nc.sync.dma_start(out=tf[1][:, :, HW + HC:HW + L],
                              in_=y8d[b][:, :, HC:L])
            for cc in range(2):
                nc.gpsimd.dma_start(
                    out=s["x8"][:, :, 1536 * cc:1536 * (cc + 1)],
                    in_=x8d[b][:, :, 1536 * cc:1536 * (cc + 1)])
            nc.gpsimd.dma_start(out=tf[0][:, :, HW + 1 + HC:HW + 1 + L],
                                in_=y8d[b][:, :, HC:L])
            nc.gpsimd.dma_start(out=tf[2][:, :, HW + HC:HW + L - 1],
                                in_=y8d[b][:, :, HC + 1:L])
            # e5m2 residual of x (for the fp8 error-feedback output matmul)
            s["x5"] = p_x.tile([128, 2, M], f8e5, tag="x5", name="x5")
            nc.sync.dma_start(out=s["x5"][:], in_=x5d[b])
            # memset order matters: the DVE FIFO is in-order, so everything
            # the first conv groups need (row 0, first-half edge columns)
            # is emitted before anything gated on the second-half DMAs
            for dx in range(3):
                nc.vector.memset(qsh[dx][:, :, 0:1, :].bitcast(u32), 0)
            for dx in (0, 2):
                col = 0 if dx == 0 else HW - 1
                nc.vector.memset(
                    qsh[dx].bitcast(u8)[:, :, 1:1 + HC // HW, col:col + 1], 0)
            for dx in range(3):
                t = qsh[dx]
                nc.vector.memset(t[:, :, HW + 1:HW + 2, :].bitcast(u32), 0)
                if dx != 1:
                    col = 0 if dx == 0 else HW - 1
                    nc.vector.memset(
                        t.bitcast(u8)[:, :, 1 + HC // HW:HW + 1, col:col + 1], 0)

        def emit_stream(vk, hooks=()):
            s = state[vk]
            qsh, x8t = s["qsh"], s["x8"]
            ps_scc = pp_sc.tile([128, 512], f32, tag="pscc", name="pscc")
            ps_sc = [ps_scc[:, 0:DIM], ps_scc[:, DIM:512]]
            ps_g4 = pp_g4.tile([128, 512], f32, tag="g4", name="g4")
            s["sc"], s["scc"], s["g4"] = ps_sc, ps_scc, ps_g4
            ct8 = p_ct.tile([128, NST, DIM], f8, tag="ct8", name="ct8")
            flats = [qsh[dx].rearrange("p t a b -> p t (a b)") for dx in range(3)]
            qps = s["qps"] = []

            def emit_conv(j2):
                half = j2 % 2
                if half == 0:
                    s["pc"] = pp_pc.tile([128, 512], f32, tag="pc", name="pc")
                ps = s["pc"][:, DIM * half:DIM * (half + 1)]
                for dy in range(3):
                    for dx in range(3):
                        off = (2 * j2 + dy) * HW
                        nc.tensor.matmul(
                            ps, flats[dx][:, :, off:off + 128],
                            wdw8[:, :, (dy * 3 + dx) * DIM:(dy * 3 + dx + 1) * DIM],
                            start=(dy == 0 and dx == 0),
                            stop=(dy == 2 and dx == 2),
                            perf_mode=DR, skip_group_check=True)
                if half == 1:
                    # GPSIMD can't read PSUM on hw: alternate DVE/ACT
                    dst = ct8[:, j2 - 1:j2 + 1, :].rearrange("p a b -> p (a b)")
                    if (j2 // 2) % 2 == 0:
                        nc.vector.tensor_copy(dst, s["pc"][:])
                    else:
                        nc.scalar.copy(dst, s["pc"][:])

            def emit_mtile(j):
                s0 = (4 * j) // 3
                psik = pp_ik.tile([128, 512], f32, tag="pik", name="pik")
                # interp into [0:256] and kT into [256:512] of one PSUM bank
                # (per-element has_written drives overwrite-then-accumulate)
                nc.tensor.matmul(
                    psik[:, 0:DIM], smat8[:, j % 3], ct8[:, s0:s0 + 2, :],
                    start=True, stop=False, perf_mode=DR, skip_group_check=True)
                nc.tensor.matmul(
                    psik[:, DIM:512], x8t[:, :, 128 * j:128 * (j + 1)], wk8[:],
                    start=False, stop=True, perf_mode=DR, skip_group_check=True)
                if j % 2 == 0:
                    qps.append(p_qk.tile([128, 2, 512], f8, tag="qk8", name="qk8"))
                    nc.vector.tensor_copy(qps[-1][:, 0, :], psik[:])
                else:
                    nc.scalar.copy(qps[-1][:, 1, :], psik[:])

            def emit_scores(p):
                # lagged by a conv group so the qk8 copies are long done
                qp = qps[p]
                for rr in range(2):
                    nc.tensor.matmul(
                        ps_sc[rr], qp[:, :, 128 * rr:128 * (rr + 1)],
                        qp[:, :, DIM:512],
                        start=(p == 0 and rr == 0),
                        stop=(p == NMT // 2 - 1 and rr == 1),
                        perf_mode=DR, skip_group_check=True)
                # gram blocks for l2 norms (diag extracted in softmax)
                for g in range(4):
                    nc.tensor.matmul(
                        ps_g4[:, 128 * g:128 * (g + 1)],
                        qp[:, :, 128 * g:128 * (g + 1)],
                        qp[:, :, 128 * g:128 * (g + 1)],
                        start=(p == 0), stop=(p == NMT // 2 - 1),
                        perf_mode=DR, skip_group_check=True)

            # conv group jj; mtiles lag one group; scores lag mtiles so every
            # PE instruction's copy-dependencies resolved a whole group ago
            ndone = 0
            for jj in range(8):
                for q in range(4):
                    emit_conv(4 * jj + q)
                if jj < len(hooks):
                    hooks[jj]()
                if jj >= 1:
                    # pairs whose last mtile was emitted a full group ago
                    ready = max(0, (3 * jj - 3) // 2)
                    while ndone < ready:
                        emit_scores(ndone)
                        ndone += 1
                    for t in range(3):
                        emit_mtile(3 * (jj - 1) + t)
            for t in range(3):
                emit_mtile(21 + t)

            def tail_scores(nd=ndone):
                for p in range(nd, NMT // 2):
                    emit_scores(p)

            # defer the last score pairs: their qk8 copies were just issued,
            # so the caller schedules them under later PE work
            s["tail_fn"] = tail_scores

        def softmax_parts(vk):
            """Softmax chain as three emission hooks, interleavable with the
            next batch's conv groups so the chain latency hides under them.
            Part 1 frees the pscc/g4 PSUM banks early (SBUF copies)."""
            s = state[vk]
            rqT, rZ = [], []

            def part1():
                s["tail_fn"]()  # this batch's deferred score pairs
                ps_g4 = s["g4"]
                # scores to SBUF, freeing the bank for the next stream
                scp = p_sm.tile([128, 512], f32, tag="scp", name="scp")
                s["scp"] = scp
                nc.vector.tensor_copy(scp[:], s["scc"][:])
                # gram diagonals -> packed [1,512] row [nq0 nq1 nk0 nk1]
                geye = p_sm.tile([128, 512], f16, tag="geye", name="geye")
                nc.vector.tensor_mul(geye[:], ps_g4[:], eye4[:])
                ps_nqk = pp_pq.tile([1, 512], f32, tag="pq", name="pq")
                nc.tensor.matmul(ps_nqk[:], ones_c16[:], geye[:], start=True, stop=True)
                # 1/sqrt of the packed [nq | nk] row as exp(-ln(n)/2): keeps
                # every ACT function (copy/exp/ln) in one activation table,
                # avoiding two 1.3us table reloads per batch that sqrt causes
                rrow = p_sm.tile([1, 512], f32, tag="rrow", name="rrow", bufs=2)
                s["rrow"] = rrow
                nc.scalar.activation(rrow[:], ps_nqk[:], Ln)
                nc.scalar.activation(rrow[:], rrow[:], Exp, scale=-0.5)

            def part2():
                rrow = s["rrow"]
                # rnq back to per-partition columns (+ temperature)
                for rr in range(2):
                    pst = pp_pq.tile([128, 1], f32, tag="pq", name="pq")
                    nc.tensor.transpose(pst[:], rrow[:, 128 * rr:128 * (rr + 1)],
                                        ident[0:1, 0:1])
                    rqt = p_tn.tile([128, 1], f32, tag="rqt", name="rqt")
                    nc.vector.tensor_mul(rqt[:], pst[:], tmpv[rr][:])
                    rqT.append(rqt)
                # rnk broadcast down partitions via outer product
                psb = pp_pq.tile([128, DIM], f32, tag="pq", name="pq")
                nc.tensor.matmul(psb[:], ones_row[:], rrow[:, DIM:512],
                                 start=True, stop=True)
                rkb = p_sm.tile([128, DIM], f32, tag="rkb", name="rkb")
                s["rkb"] = rkb
                nc.scalar.copy(rkb[:], psb[:])

            def part3():
                scp, rkb = s["scp"], s["rkb"]
                E = [p_sm.tile([128, DIM], f32, tag="e", name="e") for _ in range(2)]
                # masked softmax, exp with fused row-sum; the all-SBUF chain
                # ops legally run on GPSIMD/Pool, freeing DVE for PSUM copies
                for rr in range(2):
                    sc = p_sm.tile([128, DIM], f32, tag="sc", name="sc")
                    # fused (scp * rq) * rkb in one DVE pass
                    nc.vector.scalar_tensor_tensor(
                        sc[:], scp[:, DIM * rr:DIM * (rr + 1)], rqT[rr][:],
                        rkb[:], op0=mybir.AluOpType.mult,
                        op1=mybir.AluOpType.mult)
                    nc.vector.tensor_add(sc[:], sc[:], msk[rr][:])
                    z = p_tn.tile([128, 1], f32, tag="z", name="z")
                    nc.scalar.activation(E[rr][:], sc[:], Exp, accum_out=z[:])
                    rz = p_tn.tile([128, 1], f32, tag="rz", name="rz")
                    nc.vector.reciprocal(rz[:], z[:])
                    rZ.append(rz)
                # Ahat = E / Z (rows)
                Ahat = [p_sm.tile([128, DIM], f32r, tag="ah", name="ah", bufs=3)
                        for _ in range(2)]
                for rr in range(2):
                    nc.vector.tensor_scalar_mul(Ahat[rr][:], E[rr][:], rZ[rr][:])
                s["Ahat"] = Ahat

            return (part1, part2, part3)

        def emit_softmax(vk):
            for p in softmax_parts(vk):
                p()

        def out_head(vk):
            s = state[vk]
            Ahat = s["Ahat"]
            # m1t[d,o] = (W_po @ Ahat)^T ; wch[c,o] = (W_po @ Ahat @ W_v)^T
            m1t = [p_sm.tile([128, DIM], f16, tag="m1t", name="m1t") for _ in range(2)]
            for d in range(2):
                ps = pp_pq.tile([128, DIM], f32, tag="pq", name="pq")
                for k in range(C128):
                    nc.tensor.matmul(
                        ps[:], r(Ahat[k][:, 128 * d:128 * (d + 1)]), r(wp[k][:]),
                        start=(k == 0), stop=(k == C128 - 1))
                nc.vector.tensor_copy(m1t[d][:], ps[:])
            # wch in e4m3 + e5m2 residual: the final matmul runs as fp8
            # DoubleRow with error feedback (wch8@x8 + wch8@dx + dwch@x8),
            # adding only ~0.2% output error but halving the PE cost
            wch8 = p_sm.tile([128, 2, DIM], f8, tag="wch8", name="wch8")
            dwch5 = p_sm.tile([128, 2, DIM], f8e5, tag="dwch5", name="dwch5")
            s["wch8"], s["dwch5"] = wch8, dwch5
            for cb in range(2):
                ps = pp_pq.tile([128, DIM], f32, tag="pq", name="pq")
                for d in range(2):
                    nc.tensor.matmul(
                        ps[:], wv2[d][:, 128 * cb:128 * (cb + 1)], m1t[d][:],
                        start=(d == 0), stop=(d == 1))
                nc.scalar.copy(wch8[:, cb, :], ps[:])
                nc.vector.tensor_sub(dwch5[:, cb, :], ps[:], wch8[:, cb, :])

        def fin_tile(vk, i, mixed=True):
            """One 1024-wide output tile of W_chain @ x (fp8 DoubleRow with
            error feedback). mixed=False keeps PSUM in the pq pool so tiles
            can interleave with a stream (whose psik owns the ik pool)."""
            s = state[vk]
            b = s["b"]
            x8t, x5t = s["x8"], s["x5"]
            wch8, dwch5 = s["wch8"], s["dwch5"]
            o, h = divmod(i, NM512 // 2)
            st = p_fin.tile([128, 1024], f16, tag="fin", name="fin")
            for half in range(2):
                n = 2 * h + half
                pool = pp_pq if (not mixed or (2 * i + half) % 2 == 0) else pp_ik
                tg = "pq" if pool is pp_pq else "pik"
                ps = pool.tile([128, 512], f32, tag=tg, name=tg)
                osl = slice(128 * o, 128 * (o + 1))
                nsl = slice(512 * n, 512 * (n + 1))
                nc.tensor.matmul(ps[:], wch8[:, :, osl], x8t[:, :, nsl],
                                 start=True, stop=False, perf_mode=DR)
                nc.tensor.matmul(ps[:], wch8[:, :, osl], x5t[:, :, nsl],
                                 start=False, stop=False, perf_mode=DR)
                nc.tensor.matmul(ps[:], dwch5[:, :, osl], x8t[:, :, nsl],
                                 start=False, stop=True, perf_mode=DR)
                if half == 0:
                    nc.scalar.copy(st[:, 0:512], ps[:])
                else:
                    nc.vector.tensor_copy(st[:, 512:1024], ps[:])
            deng = nc.sync if h % 2 == 0 else nc.gpsimd
            deng.dma_start(
                out=od[b, 128 * o:128 * (o + 1), 1024 * h:1024 * (h + 1)],
                in_=st[:])

        def emit_out(vk, hooks=(), tiles=None):
            s = state[vk]
            if "wch8" not in s:
                out_head(vk)
            if len(hooks) > 0:
                hooks[0]()
            for i in (range(2 * (NM512 // 2)) if tiles is None else tiles):
                fin_tile(vk, i)
                if i + 1 < len(hooks):
                    hooks[i + 1]()

        # software pipeline: q1(b+1) fills the PE while batch b's softmax
        # chain runs *inside* stream(b+1) via hooks (its tiny PE ops slot
        # between conv groups; part 1 frees the score/gram PSUM banks before
        # stream(b+1)'s first scores matmul needs them)
        sq_ = list(range(n_batches)) if seq is None else list(seq)
        vis = [(i, b) for i, b in enumerate(sq_)]
        n = len(vis)
        emit_load_q1(0, vis[0][1], after_c0=early_consts)
        emit_stream(0)
        late_consts()
        for i in range(1, n):
            emit_load_q1(i, vis[i][1])
            # softmax(i-1) and batch i-1's out-head hide inside stream(i)'s
            # conv groups
            sm = softmax_parts(i - 1)
            emit_stream(i, hooks=(*sm, lambda vv=i - 1: out_head(vv)))
            if i >= 2:
                emit_out(i - 2)
        if n > 1:
            # tail: last softmax chain hides under out(n-2)'s fin stream
            emit_out(n - 2, hooks=softmax_parts(n - 1))
            out_head(n - 1)
            emit_out(n - 1)
        else:
            emit_softmax(0)
            emit_out(0)

    if split_waits:
        _split_waits(nc)
    return nc


def _get_nc():
    if "nc" not in _CACHE:
        _CACHE["nc"] = build_nc()
    return _CACHE["nc"]


def make_inputs(inputs):
    """Host-side prep: consts + per-core sharded activations."""
    consts = _host_consts(inputs["W_kv"], inputs["W_q"], inputs["W_dw"],
                          inputs["W_po"], inputs["temperature"])
    xf = np.asarray(inputs["x"], np.float32)
    x8f = _f8(xf)
    x5f = _f8e5(xf - np.float32(x8f))
    x8 = x8f.reshape(B, 2, 128, M).transpose(0, 2, 1, 3)
    x5 = x5f.reshape(B, 2, 128, M).transpose(0, 2, 1, 3)
    y8 = _f8(inputs["y"]).reshape(B, 2, 128, L).transpose(0, 2, 1, 3)
    in_maps = []
    for i in range(NCORES):
        m = dict(consts)
        m["x8_sh"] = np.ascontiguousarray(x8[BL * i:BL * (i + 1)])
        m["x5_sh"] = np.ascontiguousarray(x5[BL * i:BL * (i + 1)])
        m["y8_sh"] = np.ascontiguousarray(y8[BL * i:BL * (i + 1)])
        in_maps.append(m)
    return in_maps


def run(inputs, trace=False, trace_kwargs=None):
    from concourse.bass_utils import run_bass_kernel_spmd

    nc = _get_nc()
    in_maps = make_inputs(inputs)
    res = run_bass_kernel_spmd(
        nc, in_maps, core_ids=list(range(NCORES)), trace=trace,
        trace_kwargs=trace_kwargs or {})
    out = np.concatenate(
        [np.asarray(res.results[i]["out"], np.float32) for i in range(NCORES)],
        axis=0)
    return out, res


def kernel(**inputs) -> np.ndarray:
    out, _ = run(inputs, trace=False)
    return out


# revision 89
# speedup vs baseline: 2.7127x; 1.0452x over previous
"""Trainium2 Bass kernel for nn_Attention1 (channel attention transformer block).

Reference computation (per batch):
  kv = W_kv @ x ; k, v = split(kv)                    # pointwise conv over m=3072
  q  = conv3x3(W_q @ y, W_dw)                         # 1x1 then full 3x3, 64x64 image
  q  = linear_interp(snake(q.flatten(HW)), 4096->3072)
  q, k = l2norm over m ; attn = softmax(q @ k^T * temp) per 32-channel head
  out = W_po @ (attn @ v)

Sharding: data-parallel over batch, 16 batches / 8 cores = 2 per core. SPMD,
no collectives; per-core outputs are concatenated on host.

Per-core kernel layout strategy. All heavy matmuls run in fp8(e4m3) with
perf_mode=DoubleRow: operands are laid out [128 partitions, 2, cols] so one
matmul contracts 256 channels (the pair dim sums in the PE), doubling PE
throughput vs fp16. fp8 is safe for everything that feeds the l2-normalized
q/k (scale and elementwise quantization wash out to ~0.1% at score level);
the final W_chain @ x matmul stays fp16 since it writes the output directly.
  - q path   : y kept as fp8 chan-pair images, zero-padded 66x64, three
               horizontally shifted copies; 3x3 conv as 9 DoubleRow matmuls
               (image stationary), both 128-chan blocks contracted at once.
               conv outputs for two row-tiles share one PSUM bank, one copy
               into a contiguous fp8 ct arena (32 x [128,256] slots)
  - snake+interp : fused sparse matrix S applied via one DoubleRow matmul
               per m-tile (S blocks repeat with period 3; the two
               contributing 128x128 blocks ride the pair dim)
  - kT       : x chan-pair fp8 stationary, W_k^T pair fp8 moving -> kT
               (m on partitions), same PSUM bank as interp (one copy)
  - scores   : qk tiles copied to fp8 pair arenas (two m-tiles per pair);
               q@kT via DoubleRow contracting 256 m-rows per matmul
  - norms    : gram blocks qk8[s]^T qk8[s] accumulate in PSUM; diagonal
               extracted with an eye mask (DVE) + ones-matmul -> packed
               [1,512] row of |q|^2,|k|^2 (replaces explicit squares)
  - softmax  : per-head masking via additive -30 mask over the full 256-wide
               score rows; exp on ScalarE with fused row-sum (accum_out);
               1/Z folded into Ahat as a per-row scale
  - out      : W_po @ A @ W_v folded into a 256x256 chain (f32r/f16 small
               matmuls), then W_chain @ x streamed in fp16; output DMA'd
               as f16 and upcast on host
"""
import numpy as np

HEADS = 8
B, DIM, M = 16, 256, 3072
HW = 64
L = HW * HW          # 4096 flattened conv spatial size
NCORES = 8
BL = B // NCORES     # batches per core
C128 = DIM // 128    # channel 128-tiles (2)
NM512 = M // 512     # m-dim 512-tiles (6)
NMT = M // 128       # m-dim 128-tiles (24)
NST = L // 128       # conv-spatial 128-tiles (32)

_CACHE = {}


def _f8(a):
    import ml_dtypes
    return np.asarray(a, np.float32).astype(ml_dtypes.float8_e4m3)


def _f8e5(a):
    import ml_dtypes
    return np.asarray(a, np.float32).astype(ml_dtypes.float8_e5m2)


def _s_blocks():
    """Snake+interp as a sparse matrix; 6 distinct 128x128 blocks (period 3)."""
    mask = np.arange(L).reshape(HW, HW)
    mask[1::2] = mask[1::2][:, ::-1]
    mask = mask.reshape(-1)
    src = (np.arange(M) + 0.5) * (L / M) - 0.5
    src = np.maximum(src, 0.0)
    i0 = np.minimum(np.floor(src).astype(np.int64), L - 1)
    i1 = np.minimum(i0 + 1, L - 1)
    lam = (src - i0).astype(np.float32)
    S = np.zeros((L, M), np.float32)
    S[mask[i0], np.arange(M)] += (1 - lam)
    S[mask[i1], np.arange(M)] += lam
    blocks = np.zeros((6, 128, 128), np.float32)
    for j in range(3):
        s0 = (4 * j) // 3
        for t in range(2):
            blocks[j * 2 + t] = S[128 * (s0 + t):128 * (s0 + t + 1), 128 * j:128 * (j + 1)]
    # [128 l-part, 3 (j%3), 2 (pair t), 128 m]
    return blocks.transpose(1, 0, 2).reshape(128, 3, 2, 128).copy()


def _host_consts(W_kv, W_q, W_dw, W_po, temperature):
    c = np.arange(DIM)
    mask = np.where((c[:, None] // 32) == (c[None, :] // 32), 0.0, -30.0).astype(np.float32)
    tv = np.repeat(np.asarray(temperature, np.float32).reshape(HEADS), DIM // HEADS)
    # folded conv weights: (W_dw . W_q) -> [in-chan b, dy, dx, out o], then
    # chan-pair layout [128 p, 2 t, 9*256] with in-chan = p + 128 t
    fold = np.einsum("oayx,ab->byxo", np.asarray(W_dw, np.float32),
                     np.asarray(W_q, np.float32))
    wdw8 = fold.reshape(2, 128, 9 * DIM).transpose(1, 0, 2)
    # W_k^T chan-pair: [128 p, 2 t, 256 o] with in-chan = p + 128 t
    wk8 = W_kv[:DIM].T.reshape(2, 128, DIM).transpose(1, 0, 2)
    eye4 = np.concatenate([np.eye(128, dtype=np.float32)] * 4, axis=1)
    return {
        "w_k8": _f8(wk8),
        "w_v2": np.ascontiguousarray(W_kv[DIM:], np.float16),
        "w_poT": np.ascontiguousarray(W_po.T, np.float32),
        "w_dw8": _f8(wdw8),
        "s_mat8": _f8(_s_blocks()),
        "eye4": np.ascontiguousarray(eye4),
        "mask": np.ascontiguousarray(mask.reshape(2, 128, DIM)),
        "tempv": np.ascontiguousarray(tv.reshape(2, 128, 1)),
    }


def _make_tc_class():
    """TileContext subclass splitting the end-of-kernel drain waits.

    This container's walrus rejects >1 sem wait on CTRL-encoded instructions
    (Drain/NoOp). The stock Tile epilogue hangs every semaphore's final value
    on one Drain. Emit a chain of SP NoOps with one wait each instead, then a
    waitless drain: SP reaches it only after all sems hit their final values.
    """
    import bass_rust
    import concourse.mybir as mybir
    import concourse.tile as tile

    class SplitDrainTileContext(tile.TileContext):
        def _drain_and_barrier(self, tick_clock, wait_clock):
            probe = self.nc.sync.nop()
            wait_clock.add_sem_waits(
                probe.ins, bass_rust.ScopedClock({None: tick_clock.global_clock})
            )
            waits = list(probe.ins.sync_info.on_wait or [])
            probe.ins.sync_info.on_wait = waits[:1]
            for w in waits[1:]:
                n2 = self.nc.sync.nop()
                n2.ins.sync_info = mybir.SyncInfo(on_wait=[w], on_update=[])
            self.nc.sync.drain()
            self.nc.all_engine_barrier()
            assert self.sems is not None
            popped = self.nc._tile_sem_poison_stack.pop()
            assert popped is self._sem_poison
            self.nc.clear_and_free_semaphores(list(self.sems.allocated().values()))
            self.nc.all_engine_barrier()

    return SplitDrainTileContext


def _split_waits(nc):
    """Walrus in this container allows only one sem wait per instruction.
    Move extra waits onto same-engine NoOps inserted just before."""
    import concourse.mybir as mybir
    n = 0
    for f in nc.m.functions:
        for bb in f.blocks:
            out = []
            changed = False
            for inst in bb.instructions:
                si = inst.sync_info
                waits = list(si.on_wait) if si and si.on_wait else []
                if len(waits) > 1:
                    for w in waits[:-1]:
                        n += 1
                        nop = mybir.InstNoOp(name=f"I-sw{n}-{inst.name}", ins=[], outs=[])
                        nop.engine = inst.engine
                        nop.sync_info = mybir.SyncInfo(on_wait=[w], on_update=[])
                        out.append(nop)
                    si.on_wait = [waits[-1]]
                    changed = True
                out.append(inst)
            if changed:
                bb.instructions = out
    return n


def build_nc(split_waits=True, n_batches=BL, seq=None):
    from contextlib import ExitStack
    from collections import defaultdict
    import concourse.bass as bass
    import concourse.mybir as mybir
    from concourse.masks import make_identity

    f32 = mybir.dt.float32
    f32r = mybir.dt.float32r
    u32 = mybir.dt.uint32
    u8 = mybir.dt.uint8
    f16 = mybir.dt.float16
    f8 = mybir.dt.float8e4
    DR = mybir.MatmulPerfMode.DoubleRow
    Exp = mybir.ActivationFunctionType.Exp
    Ln = mybir.ActivationFunctionType.Ln

    def r(ap):
        return ap.bitcast(f32r)

    TC = _make_tc_class()
    nc = bass.Bass("TRN2", target_bir_lowering=False, debug=False)

    f8e5 = mybir.dt.float8e5
    x8d = nc.dram_tensor("x8_sh", [BL, 128, 2, M], f8, kind="ExternalInput").ap()
    x5d = nc.dram_tensor("x5_sh", [BL, 128, 2, M], f8e5, kind="ExternalInput").ap()
    y8d = nc.dram_tensor("y8_sh", [BL, 128, 2, L], f8, kind="ExternalInput").ap()
    wkd = nc.dram_tensor("w_k8", [128, 2, DIM], f8, kind="ExternalInput").ap()
    wvd = nc.dram_tensor("w_v2", [DIM, DIM], f16, kind="ExternalInput").ap()
    wpd = nc.dram_tensor("w_poT", [DIM, DIM], f32r, kind="ExternalInput").ap()
    wdd = nc.dram_tensor("w_dw8", [128, 2, 9 * DIM], f8, kind="ExternalInput").ap()
    sd = nc.dram_tensor("s_mat8", [128, 3, 2, 128], f8, kind="ExternalInput").ap()
    eyd = nc.dram_tensor("eye4", [128, 512], f32, kind="ExternalInput").ap()
    md = nc.dram_tensor("mask", [2, 128, DIM], f32, kind="ExternalInput").ap()
    td = nc.dram_tensor("tempv", [2, 128, 1], f32, kind="ExternalInput").ap()
    od = nc.dram_tensor("out", [BL, DIM, M], f16, kind="ExternalOutput").ap()

    with TC(nc) as tc, ExitStack() as ctx:
        P = lambda **kw: ctx.enter_context(tc.tile_pool(**kw))
        consts = P(name="consts", bufs=1)
        p_qp = P(name="p_qp", bufs=2)
        p_x = P(name="p_x", bufs=2)
        p_ct = P(name="p_ct", bufs=2)
        p_qk = P(name="p_qk", bufs=4)
        p_sm = P(name="p_sm", bufs=2)
        p_tn = P(name="p_tn", bufs=4)
        p_fin = P(name="p_fin", bufs=12)
        # global PSUM pools: 2+2+1+1+2 = 8 banks exactly
        pp_pc = P(name="pp_pc", bufs=2, space="PSUM")
        pp_ik = P(name="pp_ik", bufs=2, space="PSUM")
        pp_sc = P(name="pp_sc", bufs=1, space="PSUM")
        pp_g4 = P(name="pp_g4", bufs=1, space="PSUM")
        pp_pq = P(name="pp_pq", bufs=2, space="PSUM")

        # ---- constants; the conv weights are split across all three DMA
        # queues (behind each queue's first image chunk) so the first conv
        # tile can start ~2.5us in; softmax/out-chain consts are emitted
        # after batch 0's loads ----
        wdw8 = consts.tile([128, 2, 9 * DIM], f8, tag="wdw8", name="wdw8")
        smat8 = consts.tile([128, 3, 2, 128], f8, tag="smat8", name="smat8")
        wk8 = consts.tile([128, 2, DIM], f8, tag="wk8", name="wk8")

        def early_consts():
            for dy, eng in ((0, nc.scalar), (1, nc.sync), (2, nc.gpsimd)):
                c0, c1 = 3 * DIM * dy, 3 * DIM * (dy + 1)
                eng.dma_start(out=wdw8[:, :, c0:c1], in_=wdd[:, :, c0:c1])
            nc.scalar.dma_start(out=smat8[:], in_=sd)
            nc.scalar.dma_start(out=wk8[:], in_=wkd)
        wv2 = [consts.tile([128, DIM], f16, tag=f"wv2{k}", name=f"wv2{k}") for k in range(C128)]
        wp = [consts.tile([128, DIM], f32r, tag=f"wp{k}", name=f"wp{k}") for k in range(C128)]
        eye4 = consts.tile([128, 512], f32, tag="eye4", name="eye4")
        msk = [consts.tile([128, DIM], f32, tag=f"msk{k}", name=f"msk{k}") for k in range(2)]
        tmpv = [consts.tile([128, 1], f32, tag=f"tmpv{k}", name=f"tmpv{k}") for k in range(2)]

        def late_consts():
            for k in range(C128):
                sl = slice(128 * k, 128 * (k + 1))
                nc.sync.dma_start(out=wv2[k][:], in_=wvd[sl, :])
                nc.sync.dma_start(out=wp[k][:], in_=wpd[sl, :])
            nc.sync.dma_start(out=eye4[:], in_=eyd)
            for rr in range(2):
                nc.sync.dma_start(out=msk[rr][:], in_=md[rr])
                nc.sync.dma_start(out=tmpv[rr][:], in_=td[rr])
        ident = consts.tile([128, 128], f32, tag="ident", name="ident")
        make_identity(nc, ident[:])
        ones_row = consts.tile([1, 128], f32, tag="ones", name="ones")
        nc.vector.memset(ones_row[:], 1.0)
        ones_c16 = consts.tile([128, 1], f16, tag="ones16", name="ones16")
        nc.vector.memset(ones_c16[:], 1.0)

        state = defaultdict(dict)

        def emit_load_q1(vk, b, after_c0=None):
            s = state[vk]
            s["b"] = b
            s["x8"] = p_x.tile([128, 2, M], f8, tag="x8", name="x8")
            # W_q is folded into the conv weights on the host, so the conv
            # input is y itself: one DRAM DMA for the centered fp8 chan-pair
            # image (128, 2, 66, 64) with zeroed border rows, then two
            # on-chip shifted copies (dx-1 = -1/+1) built by SBUF->SBUF DMA;
            # the row-wrap artifacts land exactly on the edge columns the
            # memsets zero afterwards. Chunked x2 so early conv tiles start
            # before the whole image lands.
            qsh = [p_qp.tile([128, 2, HW + 2, HW], f8, tag=f"qsh{dx}",
                             name=f"qsh{dx}") for dx in range(3)]
            s["qsh"] = qsh
            tf = [qsh[dx].rearrange("p t a b -> p t (a b)") for dx in range(3)]
            # one shifted image copy per DMA queue, nothing queued ahead of
            # them: subtile tracking can't see through the pair-dim views, so
            # every conv waits for the LAST write to its qsh tile -- minimize
            # the makespan of the three copies instead of chunking cleverly
            HC = L // 2
            nc.sync.dma_start(out=tf[1][:, :, HW:HW + HC], in_=y8d[b][:, :, 0:HC])
            # qsh0[65+l] = y[l]
            nc.scalar.dma_start(out=tf[0][:, :, HW + 1:HW + 1 + HC],
                                in_=y8d[b][:, :, 0:HC])
            # qsh2[64+l] = y[l+1]  (tail col is memset)
            nc.gpsimd.dma_start(out=tf[2][:, :, HW:HW + HC],
                                in_=y8d[b][:, :, 1:HC + 1])
            nc.sync.dma_start(out=tf[1][:, :, HW + HC:HW + L],
                              in_=y8d[b][:, :, HC:L])
            nc.scalar.dma_start(out=tf[0][:, :, HW + 1 + HC:HW + 1 + L],
                                in_=y8d[b][:, :, HC:L])
            nc.gpsimd.dma_start(out=tf[2][:, :, HW + HC:HW + L - 1],
                                in_=y8d[b][:, :, HC + 1:L])
            if after_c0 is not None:
                after_c0()
            for cc in range(2):
                nc.sync.dma_start(
                    out=s["x8"][:, :, 1536 * cc:1536 * (cc + 1)],
                    in_=x8d[b][:, :, 1536 * cc:1536 * (cc + 1)])
            # e5m2 residual of x (for the fp8 error-feedback output matmul)
            s["x5"] = p_x.tile([128, 2, M], f8e5, tag="x5", name="x5")
            nc.gpsimd.dma_start(out=s["x5"][:], in_=x5d[b])
            # batch-0 memsets ride DVE (free at t=0); later batches use the
            # Pool queue so a memset waiting on its own batch's image can
            # never block the previous stream's PSUM copies on DVE
            meng = nc.vector if vk == 0 else nc.gpsimd
            for dx in range(3):
                meng.memset(qsh[dx][:, :, 0:1, :].bitcast(u32), 0)
            for dx in (0, 2):
                col = 0 if dx == 0 else HW - 1
                meng.memset(qsh[dx].bitcast(u8)[:, :, 1:1 + HC // HW, col:col + 1], 0)
            for dx in range(3):
                t = qsh[dx]
                meng.memset(t[:, :, HW + 1:HW + 2, :].bitcast(u32), 0)
                if dx != 1:
                    col = 0 if dx == 0 else HW - 1
                    meng.memset(
                        t.bitcast(u8)[:, :, 1 + HC // HW:HW + 1, col:col + 1], 0)

        def emit_stream(vk, hooks=(), defer_mtiles=False):
            s = state[vk]
            qsh, x8t = s["qsh"], s["x8"]
            ps_scc = pp_sc.tile([128, 512], f32, tag="pscc", name="pscc")
            ps_sc = [ps_scc[:, 0:DIM], ps_scc[:, DIM:512]]
            ps_g4 = pp_g4.tile([128, 512], f32, tag="g4", name="g4")
            s["sc"], s["scc"], s["g4"] = ps_sc, ps_scc, ps_g4
            ct8 = p_ct.tile([128, NST, DIM], f8, tag="ct8", name="ct8")
            flats = [qsh[dx].rearrange("p t a b -> p t (a b)") for dx in range(3)]
            qps = s["qps"] = []

            def emit_conv(j2):
                half = j2 % 2
                if half == 0:
                    s["pc"] = pp_pc.tile([128, 512], f32, tag="pc", name="pc")
                ps = s["pc"][:, DIM * half:DIM * (half + 1)]
                for dy in range(3):
                    for dx in range(3):
                        off = (2 * j2 + dy) * HW
                        nc.tensor.matmul(
                            ps, flats[dx][:, :, off:off + 128],
                            wdw8[:, :, (dy * 3 + dx) * DIM:(dy * 3 + dx + 1) * DIM],
                            start=(dy == 0 and dx == 0),
                            stop=(dy == 2 and dx == 2),
                            perf_mode=DR, skip_group_check=True)
                if half == 1:
                    # GPSIMD can't read PSUM on hw: alternate DVE/ACT
                    dst = ct8[:, j2 - 1:j2 + 1, :].rearrange("p a b -> p (a b)")
                    if (j2 // 2) % 2 == 0:
                        nc.vector.tensor_copy(dst, s["pc"][:])
                    else:
                        nc.scalar.copy(dst, s["pc"][:])

            def emit_mtile(j):
                s0 = (4 * j) // 3
                psik = pp_ik.tile([128, 512], f32, tag="pik", name="pik")
                # interp into [0:256] and kT into [256:512] of one PSUM bank
                # (per-element has_written drives overwrite-then-accumulate)
                nc.tensor.matmul(
                    psik[:, 0:DIM], smat8[:, j % 3], ct8[:, s0:s0 + 2, :],
                    start=True, stop=False, perf_mode=DR, skip_group_check=True)
                nc.tensor.matmul(
                    psik[:, DIM:512], x8t[:, :, 128 * j:128 * (j + 1)], wk8[:],
                    start=False, stop=True, perf_mode=DR, skip_group_check=True)
                if j % 2 == 0:
                    qps.append(p_qk.tile([128, 2, 512], f8, tag="qk8", name="qk8"))
                    nc.vector.tensor_copy(qps[-1][:, 0, :], psik[:])
                else:
                    nc.scalar.copy(qps[-1][:, 1, :], psik[:])

            def emit_scores(p):
                # lagged by a conv group so the qk8 copies are long done
                qp = qps[p]
                for rr in range(2):
                    nc.tensor.matmul(
                        ps_sc[rr], qp[:, :, 128 * rr:128 * (rr + 1)],
                        qp[:, :, DIM:512],
                        start=(p == 0 and rr == 0),
                        stop=(p == NMT // 2 - 1 and rr == 1),
                        perf_mode=DR, skip_group_check=True)
                # gram blocks for l2 norms (diag extracted in softmax)
                for g in range(4):
                    nc.tensor.matmul(
                        ps_g4[:, 128 * g:128 * (g + 1)],
                        qp[:, :, 128 * g:128 * (g + 1)],
                        qp[:, :, 128 * g:128 * (g + 1)],
                        start=(p == 0), stop=(p == NMT // 2 - 1),
                        perf_mode=DR, skip_group_check=True)

            # conv group jj; mtiles lag one group; scores lag mtiles so every
            # PE instruction's copy-dependencies resolved a whole group ago
            ndone = 0
            for jj in range(8):
                for q in range(4):
                    emit_conv(4 * jj + q)
                if jj < len(hooks):
                    hooks[jj]()
                if jj >= 1:
                    # pairs whose last mtile was emitted a full group ago
                    ready = max(0, (3 * jj - 3) // 2)
                    while ndone < ready:
                        emit_scores(ndone)
                        ndone += 1
                    for t in range(3):
                        emit_mtile(3 * (jj - 1) + t)
            if defer_mtiles:
                # hand the last three mtiles to the caller so it can
                # interleave them with fin tiles (no conv cover remains)
                s["tail_mtiles"] = [lambda t=t: emit_mtile(21 + t)
                                    for t in range(3)]
            else:
                for t in range(3):
                    emit_mtile(21 + t)

            def tail_scores(nd=ndone):
                for p in range(nd, NMT // 2):
                    emit_scores(p)

            # defer the last score pairs: their qk8 copies were just issued,
            # so the caller schedules them under later PE work
            s["tail_fn"] = tail_scores

        def softmax_parts(vk):
            """Softmax chain as three emission hooks, interleavable with the
            next batch's conv groups so the chain latency hides under them.
            Part 1 frees the pscc/g4 PSUM banks early (SBUF copies)."""
            s = state[vk]
            rqT, rZ = [], []

            def part1():
                s["tail_fn"]()  # this batch's deferred score pairs
                ps_g4 = s["g4"]
                # scores to SBUF, freeing the bank for the next stream
                scp = p_sm.tile([128, 512], f32, tag="scp", name="scp")
                s["scp"] = scp
                nc.vector.tensor_copy(scp[:], s["scc"][:])
                # gram diagonals -> packed [1,512] row [nq0 nq1 nk0 nk1]
                geye = p_sm.tile([128, 512], f16, tag="geye", name="geye")
                nc.vector.tensor_mul(geye[:], ps_g4[:], eye4[:])
                ps_nqk = pp_pq.tile([1, 512], f32, tag="pq", name="pq")
                nc.tensor.matmul(ps_nqk[:], ones_c16[:], geye[:], start=True, stop=True)
                # 1/sqrt of the packed [nq | nk] row as exp(-ln(n)/2): keeps
                # every ACT function (copy/exp/ln) in one activation table,
                # avoiding two 1.3us table reloads per batch that sqrt causes
                rrow = p_sm.tile([1, 512], f32, tag="rrow", name="rrow", bufs=2)
                s["rrow"] = rrow
                nc.scalar.activation(rrow[:], ps_nqk[:], Ln)
                nc.scalar.activation(rrow[:], rrow[:], Exp, scale=-0.5)

            def part2():
                rrow = s["rrow"]
                # rnq back to per-partition columns (+ temperature)
                for rr in range(2):
                    pst = pp_pq.tile([128, 1], f32, tag="pq", name="pq")
                    nc.tensor.transpose(pst[:], rrow[:, 128 * rr:128 * (rr + 1)],
                                        ident[0:1, 0:1])
                    rqt = p_tn.tile([128, 1], f32, tag="rqt", name="rqt")
                    nc.vector.tensor_mul(rqt[:], pst[:], tmpv[rr][:])
                    rqT.append(rqt)
                # rnk broadcast down partitions via outer product
                psb = pp_pq.tile([128, DIM], f32, tag="pq", name="pq")
                nc.tensor.matmul(psb[:], ones_row[:], rrow[:, DIM:512],
                                 start=True, stop=True)
                rkb = p_sm.tile([128, DIM], f32, tag="rkb", name="rkb")
                s["rkb"] = rkb
                nc.scalar.copy(rkb[:], psb[:])

            def part3():
                scp, rkb = s["scp"], s["rkb"]
                E = [p_sm.tile([128, DIM], f32, tag="e", name="e") for _ in range(2)]
                # masked softmax, exp with fused row-sum; the all-SBUF chain
                # ops legally run on GPSIMD/Pool, freeing DVE for PSUM copies
                for rr in range(2):
                    sc = p_sm.tile([128, DIM], f32, tag="sc", name="sc")
                    # fused (scp * rq) * rkb in one DVE pass
                    nc.vector.scalar_tensor_tensor(
                        sc[:], scp[:, DIM * rr:DIM * (rr + 1)], rqT[rr][:],
                        rkb[:], op0=mybir.AluOpType.mult,
                        op1=mybir.AluOpType.mult)
                    nc.vector.tensor_add(sc[:], sc[:], msk[rr][:])
                    z = p_tn.tile([128, 1], f32, tag="z", name="z")
                    nc.scalar.activation(E[rr][:], sc[:], Exp, accum_out=z[:])
                    rz = p_tn.tile([128, 1], f32, tag="rz", name="rz")
                    nc.vector.reciprocal(rz[:], z[:])
                    rZ.append(rz)
                # Ahat = E / Z (rows)
                Ahat = [p_sm.tile([128, DIM], f32r, tag="ah", name="ah", bufs=3)
                        for _ in range(2)]
                for rr in range(2):
                    nc.vector.tensor_scalar_mul(Ahat[rr][:], E[rr][:], rZ[rr][:])
                s["Ahat"] = Ahat

            return (part1, part2, part3)

        def emit_softmax(vk):
            for p in softmax_parts(vk):
                p()

        def out_head(vk):
            s = state[vk]
            Ahat = s["Ahat"]
            # m1t[d,o] = (W_po @ Ahat)^T ; wch[c,o] = (W_po @ Ahat @ W_v)^T
            m1t = [p_sm.tile([128, DIM], f16, tag="m1t", name="m1t") for _ in range(2)]
            for d in range(2):
                ps = pp_pq.tile([128, DIM], f32, tag="pq", name="pq")
                for k in range(C128):
                    nc.tensor.matmul(
                        ps[:], r(Ahat[k][:, 128 * d:128 * (d + 1)]), r(wp[k][:]),
                        start=(k == 0), stop=(k == C128 - 1))
                nc.vector.tensor_copy(m1t[d][:], ps[:])
            # wch in e4m3 + e5m2 residual: the final matmul runs as fp8
            # DoubleRow with error feedback (wch8@x8 + wch8@dx + dwch@x8),
            # adding only ~0.2% output error but halving the PE cost
            wch8 = p_sm.tile([128, 2, DIM], f8, tag="wch8", name="wch8")
            dwch5 = p_sm.tile([128, 2, DIM], f8e5, tag="dwch5", name="dwch5")
            s["wch8"], s["dwch5"] = wch8, dwch5
            for cb in range(2):
                ps = pp_pq.tile([128, DIM], f32, tag="pq", name="pq")
                for d in range(2):
                    nc.tensor.matmul(
                        ps[:], wv2[d][:, 128 * cb:128 * (cb + 1)], m1t[d][:],
                        start=(d == 0), stop=(d == 1))
                nc.scalar.copy(wch8[:, cb, :], ps[:])
                nc.vector.tensor_sub(dwch5[:, cb, :], ps[:], wch8[:, cb, :])

        def fin_tile(vk, i, mixed=True):
            """One 1024-wide output tile of W_chain @ x (fp8 DoubleRow with
            error feedback). mixed=False keeps PSUM in the pq pool so tiles
            can interleave with a stream (whose psik owns the ik pool)."""
            s = state[vk]
            b = s["b"]
            x8t, x5t = s["x8"], s["x5"]
            wch8, dwch5 = s["wch8"], s["dwch5"]
            o, h = divmod(i, NM512 // 2)
            st = p_fin.tile([128, 1024], f16, tag="fin", name="fin")
            for half in range(2):
                n = 2 * h + half
                pool = pp_pq if (not mixed or (2 * i + half) % 2 == 0) else pp_ik
                tg = "pq" if pool is pp_pq else "pik"
                ps = pool.tile([128, 512], f32, tag=tg, name=tg)
                osl = slice(128 * o, 128 * (o + 1))
                nsl = slice(512 * n, 512 * (n + 1))
                nc.tensor.matmul(ps[:], wch8[:, :, osl], x8t[:, :, nsl],
                                 start=True, stop=False, perf_mode=DR)
                nc.tensor.matmul(ps[:], wch8[:, :, osl], x5t[:, :, nsl],
                                 start=False, stop=False, perf_mode=DR)
                nc.tensor.matmul(ps[:], dwch5[:, :, osl], x8t[:, :, nsl],
                                 start=False, stop=True, perf_mode=DR)
                if half == 0:
                    nc.scalar.copy(st[:, 0:512], ps[:])
                else:
                    nc.vector.tensor_copy(st[:, 512:1024], ps[:])
            deng = nc.sync if h % 2 == 0 else nc.gpsimd
            deng.dma_start(
                out=od[b, 128 * o:128 * (o + 1), 1024 * h:1024 * (h + 1)],
                in_=st[:])

        def emit_out(vk, hooks=(), tiles=None):
            s = state[vk]
            if "wch8" not in s:
                out_head(vk)
            if len(hooks) > 0:
                hooks[0]()
            for i in (range(2 * (NM512 // 2)) if tiles is None else tiles):
                fin_tile(vk, i)
                if i + 1 < len(hooks):
                    hooks[i + 1]()

        # software pipeline: q1(b+1) fills the PE while batch b's softmax
        # chain runs *inside* stream(b+1) via hooks (its tiny PE ops slot
        # between conv groups; part 1 frees the score/gram PSUM banks before
        # stream(b+1)'s first scores matmul needs them)
        sq_ = list(range(n_batches)) if seq is None else list(seq)
        vis = [(i, b) for i, b in enumerate(sq_)]
        n = len(vis)
        emit_load_q1(0, vis[0][1], after_c0=early_consts)
        emit_stream(0)
        late_consts()
        for i in range(1, n):
            emit_load_q1(i, vis[i][1])
            # softmax(i-1) and batch i-1's out-head hide inside stream(i)'s
            # conv groups
            sm = softmax_parts(i - 1)
            emit_stream(i, hooks=(*sm, lambda vv=i - 1: out_head(vv)))
            if i >= 2:
                emit_out(i - 2)
        if n > 1:
            # tail: last softmax chain hides under out(n-2)'s fin stream
            emit_out(n - 2, hooks=softmax_parts(n - 1))
            out_head(n - 1)
            emit_out(n - 1)
        else:
            emit_softmax(0)
            emit_out(0)

    if split_waits:
        _split_waits(nc)
    return nc


def _get_nc():
    if "nc" not in _CACHE:
        _CACHE["nc"] = build_nc()
    return _CACHE["nc"]


def make_inputs(inputs):
    """Host-side prep: consts + per-core sharded activations."""
    consts = _host_consts(inputs["W_kv"], inputs["W_q"], inputs["W_dw"],
                          inputs["W_po"], inputs["temperature"])
    xf = np.asarray(inputs["x"], np.float32)
    x8f = _f8(xf)
    x5f = _f8e5(xf - np.float32(x8f))
    x8 = x8f.reshape(B, 2, 128, M).transpose(0, 2, 1, 3)
    x5 = x5f.reshape(B, 2, 128, M).transpose(0, 2, 1, 3)
    y8 = _f8(inputs["y"]).reshape(B, 2, 128, L).transpose(0, 2, 1, 3)
    in_maps = []
    for i in range(NCORES):
        m = dict(consts)
        m["x8_sh"] = np.ascontiguousarray(x8[BL * i:BL * (i + 1)])
        m["x5_sh"] = np.ascontiguousarray(x5[BL * i:BL * (i + 1)])
        m["y8_sh"] = np.ascontiguousarray(y8[BL * i:BL * (i + 1)])
        in_maps.append(m)
    return in_maps


def run(inputs, trace=False, trace_kwargs=None):
    from concourse.bass_utils import run_bass_kernel_spmd

    nc = _get_nc()
    in_maps = make_inputs(inputs)
    res = run_bass_kernel_spmd(
        nc, in_maps, core_ids=list(range(NCORES)), trace=trace,
        trace_kwargs=trace_kwargs or {})
    out = np.concatenate(
        [np.asarray(res.results[i]["out"], np.float32) for i in range(NCORES)],
        axis=0)
    return out, res


def kernel(**inputs) -> np.ndarray:
    out, _ = run(inputs, trace=False)
    return out


# revision 94
# speedup vs baseline: 2.7513x; 1.0143x over previous
"""Trainium2 Bass kernel for nn_Attention1 (channel attention transformer block).

Reference computation (per batch):
  kv = W_kv @ x ; k, v = split(kv)                    # pointwise conv over m=3072
  q  = conv3x3(W_q @ y, W_dw)                         # 1x1 then full 3x3, 64x64 image
  q  = linear_interp(snake(q.flatten(HW)), 4096->3072)
  q, k = l2norm over m ; attn = softmax(q @ k^T * temp) per 32-channel head
  out = W_po @ (attn @ v)

Sharding: data-parallel over batch, 16 batches / 8 cores = 2 per core. SPMD,
no collectives; per-core outputs are concatenated on host.

Per-core kernel layout strategy. All heavy matmuls run in fp8(e4m3) with
perf_mode=DoubleRow: operands are laid out [128 partitions, 2, cols] so one
matmul contracts 256 channels (the pair dim sums in the PE), doubling PE
throughput vs fp16. fp8 is safe for everything that feeds the l2-normalized
q/k (scale and elementwise quantization wash out to ~0.1% at score level);
the final W_chain @ x matmul stays fp16 since it writes the output directly.
  - q path   : y kept as fp8 chan-pair images, zero-padded 66x64, three
               horizontally shifted copies; 3x3 conv as 9 DoubleRow matmuls
               (image stationary), both 128-chan blocks contracted at once.
               conv outputs for two row-tiles share one PSUM bank, one copy
               into a contiguous fp8 ct arena (32 x [128,256] slots)
  - snake+interp : fused sparse matrix S applied via one DoubleRow matmul
               per m-tile (S blocks repeat with period 3; the two
               contributing 128x128 blocks ride the pair dim)
  - kT       : x chan-pair fp8 stationary, W_k^T pair fp8 moving -> kT
               (m on partitions), same PSUM bank as interp (one copy)
  - scores   : qk tiles copied to fp8 pair arenas (two m-tiles per pair);
               q@kT via DoubleRow contracting 256 m-rows per matmul
  - norms    : gram blocks qk8[s]^T qk8[s] accumulate in PSUM; diagonal
               extracted with an eye mask (DVE) + ones-matmul -> packed
               [1,512] row of |q|^2,|k|^2 (replaces explicit squares)
  - softmax  : per-head masking via additive -30 mask over the full 256-wide
               score rows; exp on ScalarE with fused row-sum (accum_out);
               1/Z folded into Ahat as a per-row scale
  - out      : W_po @ A @ W_v folded into a 256x256 chain (f32r/f16 small
               matmuls), then W_chain @ x streamed in fp16; output DMA'd
               as f16 and upcast on host
"""
import numpy as np

HEADS = 8
B, DIM, M = 16, 256, 3072
HW = 64
L = HW * HW          # 4096 flattened conv spatial size
NCORES = 8
BL = B // NCORES     # batches per core
C128 = DIM // 128    # channel 128-tiles (2)
NM512 = M // 512     # m-dim 512-tiles (6)
NMT = M // 128       # m-dim 128-tiles (24)
NST = L // 128       # conv-spatial 128-tiles (32)

_CACHE = {}


def _f8(a):
    import ml_dtypes
    return np.asarray(a, np.float32).astype(ml_dtypes.float8_e4m3)


def _f8e5(a):
    import ml_dtypes
    return np.asarray(a, np.float32).astype(ml_dtypes.float8_e5m2)


def _s_blocks():
    """Snake+interp as a sparse matrix; 6 distinct 128x128 blocks (period 3)."""
    mask = np.arange(L).reshape(HW, HW)
    mask[1::2] = mask[1::2][:, ::-1]
    mask = mask.reshape(-1)
    src = (np.arange(M) + 0.5) * (L / M) - 0.5
    src = np.maximum(src, 0.0)
    i0 = np.minimum(np.floor(src).astype(np.int64), L - 1)
    i1 = np.minimum(i0 + 1, L - 1)
    lam = (src - i0).astype(np.float32)
    S = np.zeros((L, M), np.float32)
    S[mask[i0], np.arange(M)] += (1 - lam)
    S[mask[i1], np.arange(M)] += lam
    blocks = np.zeros((6, 128, 128), np.float32)
    for j in range(3):
        s0 = (4 * j) // 3
        for t in range(2):
            blocks[j * 2 + t] = S[128 * (s0 + t):128 * (s0 + t + 1), 128 * j:128 * (j + 1)]
    # [128 l-part, 3 (j%3), 2 (pair t), 128 m]
    return blocks.transpose(1, 0, 2).reshape(128, 3, 2, 128).copy()


def _host_consts(W_kv, W_q, W_dw, W_po, temperature):
    c = np.arange(DIM)
    mask = np.where((c[:, None] // 32) == (c[None, :] // 32), 0.0, -30.0).astype(np.float32)
    tv = np.repeat(np.asarray(temperature, np.float32).reshape(HEADS), DIM // HEADS)
    # folded conv weights: (W_dw . W_q) -> [in-chan b, dy, dx, out o], then
    # chan-pair layout [128 p, 2 t, 9*256] with in-chan = p + 128 t
    fold = np.einsum("oayx,ab->byxo", np.asarray(W_dw, np.float32),
                     np.asarray(W_q, np.float32))
    wdw8 = fold.reshape(2, 128, 9 * DIM).transpose(1, 0, 2)
    # W_k^T chan-pair: [128 p, 2 t, 256 o] with in-chan = p + 128 t
    wk8 = W_kv[:DIM].T.reshape(2, 128, DIM).transpose(1, 0, 2)
    eye4 = np.concatenate([np.eye(128, dtype=np.float32)] * 4, axis=1)
    return {
        "w_k8": _f8(wk8),
        "w_v2": np.ascontiguousarray(W_kv[DIM:], np.float16),
        "w_poT": np.ascontiguousarray(W_po.T, np.float32),
        "w_dw8": _f8(wdw8),
        "s_mat8": _f8(_s_blocks()),
        "eye4": np.ascontiguousarray(eye4),
        "mask": np.ascontiguousarray(mask.reshape(2, 128, DIM)),
        "tempv": np.ascontiguousarray(tv.reshape(2, 128, 1)),
    }


def _make_tc_class():
    """TileContext subclass splitting the end-of-kernel drain waits.

    This container's walrus rejects >1 sem wait on CTRL-encoded instructions
    (Drain/NoOp). The stock Tile epilogue hangs every semaphore's final value
    on one Drain. Emit a chain of SP NoOps with one wait each instead, then a
    waitless drain: SP reaches it only after all sems hit their final values.
    """
    import bass_rust
    import concourse.mybir as mybir
    import concourse.tile as tile

    class SplitDrainTileContext(tile.TileContext):
        def _drain_and_barrier(self, tick_clock, wait_clock):
            probe = self.nc.sync.nop()
            wait_clock.add_sem_waits(
                probe.ins, bass_rust.ScopedClock({None: tick_clock.global_clock})
            )
            waits = list(probe.ins.sync_info.on_wait or [])
            probe.ins.sync_info.on_wait = waits[:1]
            for w in waits[1:]:
                n2 = self.nc.sync.nop()
                n2.ins.sync_info = mybir.SyncInfo(on_wait=[w], on_update=[])
            self.nc.sync.drain()
            self.nc.all_engine_barrier()
            assert self.sems is not None
            popped = self.nc._tile_sem_poison_stack.pop()
            assert popped is self._sem_poison
            self.nc.clear_and_free_semaphores(list(self.sems.allocated().values()))
            self.nc.all_engine_barrier()

    return SplitDrainTileContext


def _split_waits(nc):
    """Walrus in this container allows only one sem wait per instruction.
    Move extra waits onto same-engine NoOps inserted just before."""
    import concourse.mybir as mybir
    n = 0
    for f in nc.m.functions:
        for bb in f.blocks:
            out = []
            changed = False
            for inst in bb.instructions:
                si = inst.sync_info
                waits = list(si.on_wait) if si and si.on_wait else []
                if len(waits) > 1:
                    for w in waits[:-1]:
                        n += 1
                        nop = mybir.InstNoOp(name=f"I-sw{n}-{inst.name}", ins=[], outs=[])
                        nop.engine = inst.engine
                        nop.sync_info = mybir.SyncInfo(on_wait=[w], on_update=[])
                        out.append(nop)
                    si.on_wait = [waits[-1]]
                    changed = True
                out.append(inst)
            if changed:
                bb.instructions = out
    return n


def build_nc(split_waits=True, n_batches=BL, seq=None):
    from contextlib import ExitStack
    from collections import defaultdict
    import concourse.bass as bass
    import concourse.mybir as mybir
    from concourse.masks import make_identity

    f32 = mybir.dt.float32
    f32r = mybir.dt.float32r
    u32 = mybir.dt.uint32
    u8 = mybir.dt.uint8
    f16 = mybir.dt.float16
    f8 = mybir.dt.float8e4
    DR = mybir.MatmulPerfMode.DoubleRow
    Exp = mybir.ActivationFunctionType.Exp
    Ln = mybir.ActivationFunctionType.Ln

    def r(ap):
        return ap.bitcast(f32r)

    TC = _make_tc_class()
    nc = bass.Bass("TRN2", target_bir_lowering=False, debug=False)

    f8e5 = mybir.dt.float8e5
    x8d = nc.dram_tensor("x8_sh", [BL, 128, 2, M], f8, kind="ExternalInput").ap()
    x5d = nc.dram_tensor("x5_sh", [BL, 128, 2, M], f8e5, kind="ExternalInput").ap()
    y8d = nc.dram_tensor("y8_sh", [BL, 128, 2, L], f8, kind="ExternalInput").ap()
    wkd = nc.dram_tensor("w_k8", [128, 2, DIM], f8, kind="ExternalInput").ap()
    wvd = nc.dram_tensor("w_v2", [DIM, DIM], f16, kind="ExternalInput").ap()
    wpd = nc.dram_tensor("w_poT", [DIM, DIM], f32r, kind="ExternalInput").ap()
    wdd = nc.dram_tensor("w_dw8", [128, 2, 9 * DIM], f8, kind="ExternalInput").ap()
    sd = nc.dram_tensor("s_mat8", [128, 3, 2, 128], f8, kind="ExternalInput").ap()
    eyd = nc.dram_tensor("eye4", [128, 512], f32, kind="ExternalInput").ap()
    md = nc.dram_tensor("mask", [2, 128, DIM], f32, kind="ExternalInput").ap()
    td = nc.dram_tensor("tempv", [2, 128, 1], f32, kind="ExternalInput").ap()
    od = nc.dram_tensor("out", [BL, DIM, M], f16, kind="ExternalOutput").ap()

    with TC(nc) as tc, ExitStack() as ctx:
        P = lambda **kw: ctx.enter_context(tc.tile_pool(**kw))
        consts = P(name="consts", bufs=1)
        p_qp = P(name="p_qp", bufs=2)
        p_x = P(name="p_x", bufs=2)
        p_ct = P(name="p_ct", bufs=2)
        p_qk = P(name="p_qk", bufs=4)
        p_sm = P(name="p_sm", bufs=2)
        p_tn = P(name="p_tn", bufs=4)
        p_fin = P(name="p_fin", bufs=12)
        # global PSUM pools: 2+2+1+1+2 = 8 banks exactly
        pp_pc = P(name="pp_pc", bufs=2, space="PSUM")
        pp_ik = P(name="pp_ik", bufs=2, space="PSUM")
        pp_sc = P(name="pp_sc", bufs=1, space="PSUM")
        pp_g4 = P(name="pp_g4", bufs=1, space="PSUM")
        pp_pq = P(name="pp_pq", bufs=2, space="PSUM")

        # ---- constants; the conv weights are split across all three DMA
        # queues (behind each queue's first image chunk) so the first conv
        # tile can start ~2.5us in; softmax/out-chain consts are emitted
        # after batch 0's loads ----
        wdw8 = consts.tile([128, 2, 9 * DIM], f8, tag="wdw8", name="wdw8")
        smat8 = consts.tile([128, 3, 2, 128], f8, tag="smat8", name="smat8")
        wk8 = consts.tile([128, 2, DIM], f8, tag="wk8", name="wk8")

        def early_consts():
            for dy, eng in ((0, nc.scalar), (1, nc.sync), (2, nc.gpsimd)):
                c0, c1 = 3 * DIM * dy, 3 * DIM * (dy + 1)
                eng.dma_start(out=wdw8[:, :, c0:c1], in_=wdd[:, :, c0:c1])
            nc.scalar.dma_start(out=smat8[:], in_=sd)
            nc.scalar.dma_start(out=wk8[:], in_=wkd)
        wv2 = [consts.tile([128, DIM], f16, tag=f"wv2{k}", name=f"wv2{k}") for k in range(C128)]
        wp = [consts.tile([128, DIM], f32r, tag=f"wp{k}", name=f"wp{k}") for k in range(C128)]
        eye4 = consts.tile([128, 512], f32, tag="eye4", name="eye4")
        msk = [consts.tile([128, DIM], f32, tag=f"msk{k}", name=f"msk{k}") for k in range(2)]
        tmpv = [consts.tile([128, 1], f32, tag=f"tmpv{k}", name=f"tmpv{k}") for k in range(2)]

        def late_consts():
            for k in range(C128):
                sl = slice(128 * k, 128 * (k + 1))
                nc.sync.dma_start(out=wv2[k][:], in_=wvd[sl, :])
                nc.sync.dma_start(out=wp[k][:], in_=wpd[sl, :])
            nc.sync.dma_start(out=eye4[:], in_=eyd)
            for rr in range(2):
                nc.sync.dma_start(out=msk[rr][:], in_=md[rr])
                nc.sync.dma_start(out=tmpv[rr][:], in_=td[rr])
        ident = consts.tile([128, 128], f32, tag="ident", name="ident")
        make_identity(nc, ident[:])
        ones_row = consts.tile([1, 128], f32, tag="ones", name="ones")
        nc.vector.memset(ones_row[:], 1.0)
        ones_c16 = consts.tile([128, 1], f16, tag="ones16", name="ones16")
        nc.vector.memset(ones_c16[:], 1.0)

        state = defaultdict(dict)

        def emit_load_q1(vk, b, after_c0=None):
            s = state[vk]
            s["b"] = b
            s["x8"] = p_x.tile([128, 2, M], f8, tag="x8", name="x8")
            # W_q is folded into the conv weights on the host, so the conv
            # input is y itself: one DRAM DMA for the centered fp8 chan-pair
            # image (128, 2, 66, 64) with zeroed border rows, then two
            # on-chip shifted copies (dx-1 = -1/+1) built by SBUF->SBUF DMA;
            # the row-wrap artifacts land exactly on the edge columns the
            # memsets zero afterwards. Chunked x2 so early conv tiles start
            # before the whole image lands.
            qsh = [p_qp.tile([128, 2, HW + 2, HW], f8, tag=f"qsh{dx}",
                             name=f"qsh{dx}") for dx in range(3)]
            s["qsh"] = qsh
            tf = [qsh[dx].rearrange("p t a b -> p t (a b)") for dx in range(3)]
            # one shifted image copy per DMA queue, nothing queued ahead of
            # them: subtile tracking can't see through the pair-dim views, so
            # every conv waits for the LAST write to its qsh tile -- minimize
            # the makespan of the three copies instead of chunking cleverly
            HC = L // 2
            nc.sync.dma_start(out=tf[1][:, :, HW:HW + HC], in_=y8d[b][:, :, 0:HC])
            # qsh0[65+l] = y[l]
            nc.scalar.dma_start(out=tf[0][:, :, HW + 1:HW + 1 + HC],
                                in_=y8d[b][:, :, 0:HC])
            # qsh2[64+l] = y[l+1]  (tail col is memset)
            nc.gpsimd.dma_start(out=tf[2][:, :, HW:HW + HC],
                                in_=y8d[b][:, :, 1:HC + 1])
            nc.sync.dma_start(out=tf[1][:, :, HW + HC:HW + L],
                              in_=y8d[b][:, :, HC:L])
            nc.scalar.dma_start(out=tf[0][:, :, HW + 1 + HC:HW + 1 + L],
                                in_=y8d[b][:, :, HC:L])
            nc.gpsimd.dma_start(out=tf[2][:, :, HW + HC:HW + L - 1],
                                in_=y8d[b][:, :, HC + 1:L])
            if after_c0 is not None:
                after_c0()
            for cc in range(2):
                nc.sync.dma_start(
                    out=s["x8"][:, :, 1536 * cc:1536 * (cc + 1)],
                    in_=x8d[b][:, :, 1536 * cc:1536 * (cc + 1)])
            # e5m2 residual of x (for the fp8 error-feedback output matmul)
            s["x5"] = p_x.tile([128, 2, M], f8e5, tag="x5", name="x5")
            nc.gpsimd.dma_start(out=s["x5"][:], in_=x5d[b])
            # batch-0 memsets ride DVE (free at t=0); later batches use the
            # Pool queue so a memset waiting on its own batch's image can
            # never block the previous stream's PSUM copies on DVE
            meng = nc.vector if vk == 0 else nc.gpsimd
            for dx in range(3):
                meng.memset(qsh[dx][:, :, 0:1, :].bitcast(u32), 0)
            for dx in (0, 2):
                col = 0 if dx == 0 else HW - 1
                meng.memset(qsh[dx].bitcast(u8)[:, :, 1:1 + HC // HW, col:col + 1], 0)
            for dx in range(3):
                t = qsh[dx]
                meng.memset(t[:, :, HW + 1:HW + 2, :].bitcast(u32), 0)
                if dx != 1:
                    col = 0 if dx == 0 else HW - 1
                    meng.memset(
                        t.bitcast(u8)[:, :, 1 + HC // HW:HW + 1, col:col + 1], 0)

        def emit_stream(vk, hooks=(), defer_mtiles=False):
            s = state[vk]
            qsh, x8t = s["qsh"], s["x8"]
            ps_scc = pp_sc.tile([128, 512], f32, tag="pscc", name="pscc")
            ps_sc = [ps_scc[:, 0:DIM], ps_scc[:, DIM:512]]
            ps_g4 = pp_g4.tile([128, 512], f32, tag="g4", name="g4")
            s["sc"], s["scc"], s["g4"] = ps_sc, ps_scc, ps_g4
            ct8 = p_ct.tile([128, NST, DIM], f8, tag="ct8", name="ct8")
            flats = [qsh[dx].rearrange("p t a b -> p t (a b)") for dx in range(3)]
            qps = s["qps"] = []

            def emit_conv(j2):
                half = j2 % 2
                if half == 0:
                    s["pc"] = pp_pc.tile([128, 512], f32, tag="pc", name="pc")
                ps = s["pc"][:, DIM * half:DIM * (half + 1)]
                for dy in range(3):
                    for dx in range(3):
                        off = (2 * j2 + dy) * HW
                        nc.tensor.matmul(
                            ps, flats[dx][:, :, off:off + 128],
                            wdw8[:, :, (dy * 3 + dx) * DIM:(dy * 3 + dx + 1) * DIM],
                            start=(dy == 0 and dx == 0),
                            stop=(dy == 2 and dx == 2),
                            perf_mode=DR, skip_group_check=True)
                if half == 1:
                    # GPSIMD can't read PSUM on hw: alternate DVE/ACT
                    dst = ct8[:, j2 - 1:j2 + 1, :].rearrange("p a b -> p (a b)")
                    if (j2 // 2) % 2 == 0:
                        nc.vector.tensor_copy(dst, s["pc"][:])
                    else:
                        nc.scalar.copy(dst, s["pc"][:])

            def emit_mtile(j):
                s0 = (4 * j) // 3
                psik = pp_ik.tile([128, 512], f32, tag="pik", name="pik")
                # interp into [0:256] and kT into [256:512] of one PSUM bank
                # (per-element has_written drives overwrite-then-accumulate)
                nc.tensor.matmul(
                    psik[:, 0:DIM], smat8[:, j % 3], ct8[:, s0:s0 + 2, :],
                    start=True, stop=False, perf_mode=DR, skip_group_check=True)
                nc.tensor.matmul(
                    psik[:, DIM:512], x8t[:, :, 128 * j:128 * (j + 1)], wk8[:],
                    start=False, stop=True, perf_mode=DR, skip_group_check=True)
                if j % 2 == 0:
                    qps.append(p_qk.tile([128, 2, 512], f8, tag="qk8", name="qk8"))
                    nc.vector.tensor_copy(qps[-1][:, 0, :], psik[:])
                else:
                    nc.scalar.copy(qps[-1][:, 1, :], psik[:])

            def emit_scores(p):
                # lagged by a conv group so the qk8 copies are long done
                qp = qps[p]
                for rr in range(2):
                    nc.tensor.matmul(
                        ps_sc[rr], qp[:, :, 128 * rr:128 * (rr + 1)],
                        qp[:, :, DIM:512],
                        start=(p == 0 and rr == 0),
                        stop=(p == NMT // 2 - 1 and rr == 1),
                        perf_mode=DR, skip_group_check=True)
                # gram blocks for l2 norms (diag extracted in softmax)
                for g in range(4):
                    nc.tensor.matmul(
                        ps_g4[:, 128 * g:128 * (g + 1)],
                        qp[:, :, 128 * g:128 * (g + 1)],
                        qp[:, :, 128 * g:128 * (g + 1)],
                        start=(p == 0), stop=(p == NMT // 2 - 1),
                        perf_mode=DR, skip_group_check=True)

            # conv group jj; mtiles lag one group; scores lag mtiles so every
            # PE instruction's copy-dependencies resolved a whole group ago
            ndone = 0
            for jj in range(8):
                for q in range(4):
                    emit_conv(4 * jj + q)
                if jj < len(hooks):
                    hooks[jj]()
                if jj >= 1:
                    # pairs whose last mtile was emitted a full group ago
                    ready = max(0, (3 * jj - 3) // 2)
                    while ndone < ready:
                        emit_scores(ndone)
                        ndone += 1
                    for t in range(3):
                        emit_mtile(3 * (jj - 1) + t)
            if defer_mtiles:
                # hand the last three mtiles to the caller so it can
                # interleave them with fin tiles (no conv cover remains)
                s["tail_mtiles"] = [lambda t=t: emit_mtile(21 + t)
                                    for t in range(3)]
            else:
                for t in range(3):
                    emit_mtile(21 + t)

            def tail_scores(nd=ndone):
                for p in range(nd, NMT // 2):
                    emit_scores(p)

            # defer the last score pairs: their qk8 copies were just issued,
            # so the caller schedules them under later PE work
            s["tail_fn"] = tail_scores

        def softmax_parts(vk):
            """Softmax chain as three emission hooks, interleavable with the
            next batch's conv groups so the chain latency hides under them.
            Part 1 frees the pscc/g4 PSUM banks early (SBUF copies)."""
            s = state[vk]
            rqT, rZ = [], []

            def part1():
                s["tail_fn"]()  # this batch's deferred score pairs
                ps_g4 = s["g4"]
                # scores to SBUF, freeing the bank for the next stream
                scp = p_sm.tile([128, 512], f32, tag="scp", name="scp")
                s["scp"] = scp
                nc.scalar.copy(scp[:], s["scc"][:])
                # gram diagonals -> packed [1,512] row [nq0 nq1 nk0 nk1]
                geye = p_sm.tile([128, 512], f16, tag="geye", name="geye")
                nc.vector.tensor_mul(geye[:], ps_g4[:], eye4[:])
                ps_nqk = pp_pq.tile([1, 512], f32, tag="pq", name="pq")
                nc.tensor.matmul(ps_nqk[:], ones_c16[:], geye[:], start=True, stop=True)
                # 1/sqrt of the packed [nq | nk] row as exp(-ln(n)/2): keeps
                # every ACT function (copy/exp/ln) in one activation table,
                # avoiding two 1.3us table reloads per batch that sqrt causes
                rrow = p_sm.tile([1, 512], f32, tag="rrow", name="rrow", bufs=2)
                s["rrow"] = rrow
                nc.scalar.activation(rrow[:], ps_nqk[:], Ln)
                nc.scalar.activation(rrow[:], rrow[:], Exp, scale=-0.5)

            def part2():
                rrow = s["rrow"]
                # rnq back to per-partition columns (+ temperature)
                for rr in range(2):
                    pst = pp_pq.tile([128, 1], f32, tag="pq", name="pq")
                    nc.tensor.transpose(pst[:], rrow[:, 128 * rr:128 * (rr + 1)],
                                        ident[0:1, 0:1])
                    rqt = p_tn.tile([128, 1], f32, tag="rqt", name="rqt")
                    nc.vector.tensor_mul(rqt[:], pst[:], tmpv[rr][:])
                    rqT.append(rqt)
                # rnk broadcast down partitions via outer product
                psb = pp_pq.tile([128, DIM], f32, tag="pq", name="pq")
                nc.tensor.matmul(psb[:], ones_row[:], rrow[:, DIM:512],
                                 start=True, stop=True)
                rkb = p_sm.tile([128, DIM], f32, tag="rkb", name="rkb")
                s["rkb"] = rkb
                nc.scalar.copy(rkb[:], psb[:])

            def part3():
                scp, rkb = s["scp"], s["rkb"]
                E = [p_sm.tile([128, DIM], f32, tag="e", name="e") for _ in range(2)]
                # masked softmax, exp with fused row-sum; the all-SBUF chain
                # ops legally run on GPSIMD/Pool, freeing DVE for PSUM copies
                for rr in range(2):
                    sc = p_sm.tile([128, DIM], f32, tag="sc", name="sc")
                    # fused (scp * rq) * rkb in one DVE pass
                    nc.vector.scalar_tensor_tensor(
                        sc[:], scp[:, DIM * rr:DIM * (rr + 1)], rqT[rr][:],
                        rkb[:], op0=mybir.AluOpType.mult,
                        op1=mybir.AluOpType.mult)
                    nc.gpsimd.tensor_add(sc[:], sc[:], msk[rr][:])
                    z = p_tn.tile([128, 1], f32, tag="z", name="z")
                    nc.scalar.activation(E[rr][:], sc[:], Exp, accum_out=z[:])
                    rz = p_tn.tile([128, 1], f32, tag="rz", name="rz")
                    nc.vector.reciprocal(rz[:], z[:])
                    rZ.append(rz)
                # Ahat = E / Z (rows)
                Ahat = [p_sm.tile([128, DIM], f32r, tag="ah", name="ah", bufs=3)
                        for _ in range(2)]
                for rr in range(2):
                    nc.vector.tensor_scalar_mul(Ahat[rr][:], E[rr][:], rZ[rr][:])
                s["Ahat"] = Ahat

            return (part1, part2, part3)

        def emit_softmax(vk):
            for p in softmax_parts(vk):
                p()

        def out_head(vk):
            s = state[vk]
            Ahat = s["Ahat"]
            # m1t[d,o] = (W_po @ Ahat)^T ; wch[c,o] = (W_po @ Ahat @ W_v)^T
            m1t = [p_sm.tile([128, DIM], f16, tag="m1t", name="m1t") for _ in range(2)]
            for d in range(2):
                ps = pp_pq.tile([128, DIM], f32, tag="pq", name="pq")
                for k in range(C128):
                    nc.tensor.matmul(
                        ps[:], r(Ahat[k][:, 128 * d:128 * (d + 1)]), r(wp[k][:]),
                        start=(k == 0), stop=(k == C128 - 1))
                nc.scalar.copy(m1t[d][:], ps[:])
            # wch in e4m3 + e5m2 residual: the final matmul runs as fp8
            # DoubleRow with error feedback (wch8@x8 + wch8@dx + dwch@x8),
            # adding only ~0.2% output error but halving the PE cost
            wch8 = p_sm.tile([128, 2, DIM], f8, tag="wch8", name="wch8")
            dwch5 = p_sm.tile([128, 2, DIM], f8e5, tag="dwch5", name="dwch5")
            s["wch8"], s["dwch5"] = wch8, dwch5
            for cb in range(2):
                ps = pp_pq.tile([128, DIM], f32, tag="pq", name="pq")
                for d in range(2):
                    nc.tensor.matmul(
                        ps[:], wv2[d][:, 128 * cb:128 * (cb + 1)], m1t[d][:],
                        start=(d == 0), stop=(d == 1))
                nc.scalar.copy(wch8[:, cb, :], ps[:])
                nc.vector.tensor_sub(dwch5[:, cb, :], ps[:], wch8[:, cb, :])

        def fin_tile(vk, i, mixed=True):
            """One 1024-wide output tile of W_chain @ x (fp8 DoubleRow with
            error feedback). mixed=False keeps PSUM in the pq pool so tiles
            can interleave with a stream (whose psik owns the ik pool)."""
            s = state[vk]
            b = s["b"]
            x8t, x5t = s["x8"], s["x5"]
            wch8, dwch5 = s["wch8"], s["dwch5"]
            o, h = divmod(i, NM512 // 2)
            st = p_fin.tile([128, 1024], f16, tag="fin", name="fin")
            for half in range(2):
                n = 2 * h + half
                pool = pp_pq if (not mixed or (2 * i + half) % 2 == 0) else pp_ik
                tg = "pq" if pool is pp_pq else "pik"
                ps = pool.tile([128, 512], f32, tag=tg, name=tg)
                osl = slice(128 * o, 128 * (o + 1))
                nsl = slice(512 * n, 512 * (n + 1))
                nc.tensor.matmul(ps[:], wch8[:, :, osl], x8t[:, :, nsl],
                                 start=True, stop=False, perf_mode=DR)
                nc.tensor.matmul(ps[:], wch8[:, :, osl], x5t[:, :, nsl],
                                 start=False, stop=False, perf_mode=DR)
                nc.tensor.matmul(ps[:], dwch5[:, :, osl], x8t[:, :, nsl],
                                 start=False, stop=True, perf_mode=DR)
                if half == 0:
                    nc.scalar.copy(st[:, 0:512], ps[:])
                else:
                    nc.vector.tensor_copy(st[:, 512:1024], ps[:])
            deng = nc.sync if h % 2 == 0 else nc.gpsimd
            deng.dma_start(
                out=od[b, 128 * o:128 * (o + 1), 1024 * h:1024 * (h + 1)],
                in_=st[:])

        def emit_out(vk, hooks=(), tiles=None):
            s = state[vk]
            if "wch8" not in s:
                out_head(vk)
            if len(hooks) > 0:
                hooks[0]()
            for i in (range(2 * (NM512 // 2)) if tiles is None else tiles):
                fin_tile(vk, i)
                if i + 1 < len(hooks):
                    hooks[i + 1]()

        # software pipeline: q1(b+1) fills the PE while batch b's softmax
        # chain runs *inside* stream(b+1) via hooks (its tiny PE ops slot
        # between conv groups; part 1 frees the score/gram PSUM banks before
        # stream(b+1)'s first scores matmul needs them)
        sq_ = list(range(n_batches)) if seq is None else list(seq)
        vis = [(i, b) for i, b in enumerate(sq_)]
        n = len(vis)
        emit_load_q1(0, vis[0][1], after_c0=early_consts)
        emit_stream(0)
        late_consts()
        for i in range(1, n):
            emit_load_q1(i, vis[i][1])
            # softmax(i-1) and batch i-1's out-head hide inside stream(i)'s
            # conv groups
            sm = softmax_parts(i - 1)
            emit_stream(i, hooks=(*sm, lambda vv=i - 1: out_head(vv)))
            if i >= 2:
                emit_out(i - 2)
        if n > 1:
            # tail: last softmax chain hides under out(n-2)'s fin stream
            emit_out(n - 2, hooks=softmax_parts(n - 1))
            out_head(n - 1)
            emit_out(n - 1)
        else:
            emit_softmax(0)
            emit_out(0)

    if split_waits:
        _split_waits(nc)
    return nc


def _get_nc():
    if "nc" not in _CACHE:
        _CACHE["nc"] = build_nc()
    return _CACHE["nc"]


def make_inputs(inputs):
    """Host-side prep: consts + per-core sharded activations."""
    consts = _host_consts(inputs["W_kv"], inputs["W_q"], inputs["W_dw"],
                          inputs["W_po"], inputs["temperature"])
    xf = np.asarray(inputs["x"], np.float32)
    x8f = _f8(xf)
    x5f = _f8e5(xf - np.float32(x8f))
    x8 = x8f.reshape(B, 2, 128, M).transpose(0, 2, 1, 3)
    x5 = x5f.reshape(B, 2, 128, M).transpose(0, 2, 1, 3)
    y8 = _f8(inputs["y"]).reshape(B, 2, 128, L).transpose(0, 2, 1, 3)
    in_maps = []
    for i in range(NCORES):
        m = dict(consts)
        m["x8_sh"] = np.ascontiguousarray(x8[BL * i:BL * (i + 1)])
        m["x5_sh"] = np.ascontiguousarray(x5[BL * i:BL * (i + 1)])
        m["y8_sh"] = np.ascontiguousarray(y8[BL * i:BL * (i + 1)])
        in_maps.append(m)
    return in_maps


def run(inputs, trace=False, trace_kwargs=None):
    from concourse.bass_utils import run_bass_kernel_spmd

    nc = _get_nc()
    in_maps = make_inputs(inputs)
    res = run_bass_kernel_spmd(
        nc, in_maps, core_ids=list(range(NCORES)), trace=trace,
        trace_kwargs=trace_kwargs or {})
    out = np.concatenate(
        [np.asarray(res.results[i]["out"], np.float32) for i in range(NCORES)],
        axis=0)
    return out, res


def kernel(**inputs) -> np.ndarray:
    out, _ = run(inputs, trace=False)
    return out


# revision 98
# speedup vs baseline: 2.7549x; 1.0013x over previous
"""Trainium2 Bass kernel for nn_Attention1 (channel attention transformer block).

Reference computation (per batch):
  kv = W_kv @ x ; k, v = split(kv)                    # pointwise conv over m=3072
  q  = conv3x3(W_q @ y, W_dw)                         # 1x1 then full 3x3, 64x64 image
  q  = linear_interp(snake(q.flatten(HW)), 4096->3072)
  q, k = l2norm over m ; attn = softmax(q @ k^T * temp) per 32-channel head
  out = W_po @ (attn @ v)

Sharding: data-parallel over batch, 16 batches / 8 cores = 2 per core. SPMD,
no collectives; per-core outputs are concatenated on host.

Per-core kernel layout strategy. All heavy matmuls run in fp8(e4m3) with
perf_mode=DoubleRow: operands are laid out [128 partitions, 2, cols] so one
matmul contracts 256 channels (the pair dim sums in the PE), doubling PE
throughput vs fp16. fp8 is safe for everything that feeds the l2-normalized
q/k (scale and elementwise quantization wash out to ~0.1% at score level);
the final W_chain @ x matmul stays fp16 since it writes the output directly.
  - q path   : y kept as fp8 chan-pair images, zero-padded 66x64, three
               horizontally shifted copies; 3x3 conv as 9 DoubleRow matmuls
               (image stationary), both 128-chan blocks contracted at once.
               conv outputs for two row-tiles share one PSUM bank, one copy
               into a contiguous fp8 ct arena (32 x [128,256] slots)
  - snake+interp : fused sparse matrix S applied via one DoubleRow matmul
               per m-tile (S blocks repeat with period 3; the two
               contributing 128x128 blocks ride the pair dim)
  - kT       : x chan-pair fp8 stationary, W_k^T pair fp8 moving -> kT
               (m on partitions), same PSUM bank as interp (one copy)
  - scores   : qk tiles copied to fp8 pair arenas (two m-tiles per pair);
               q@kT via DoubleRow contracting 256 m-rows per matmul
  - norms    : gram blocks qk8[s]^T qk8[s] accumulate in PSUM; diagonal
               extracted with an eye mask (DVE) + ones-matmul -> packed
               [1,512] row of |q|^2,|k|^2 (replaces explicit squares)
  - softmax  : per-head masking via additive -30 mask over the full 256-wide
               score rows; exp on ScalarE with fused row-sum (accum_out);
               1/Z folded into Ahat as a per-row scale
  - out      : W_po @ A @ W_v folded into a 256x256 chain (f32r/f16 small
               matmuls), then W_chain @ x streamed in fp16; output DMA'd
               as f16 and upcast on host
"""
import numpy as np

HEADS = 8
B, DIM, M = 16, 256, 3072
HW = 64
L = HW * HW          # 4096 flattened conv spatial size
NCORES = 8
BL = B // NCORES     # batches per core
C128 = DIM // 128    # channel 128-tiles (2)
NM512 = M // 512     # m-dim 512-tiles (6)
NMT = M // 128       # m-dim 128-tiles (24)
NST = L // 128       # conv-spatial 128-tiles (32)

_CACHE = {}


def _f8(a):
    import ml_dtypes
    return np.asarray(a, np.float32).astype(ml_dtypes.float8_e4m3)


def _f8e5(a):
    import ml_dtypes
    return np.asarray(a, np.float32).astype(ml_dtypes.float8_e5m2)


def _s_blocks():
    """Snake+interp as a sparse matrix; 6 distinct 128x128 blocks (period 3)."""
    mask = np.arange(L).reshape(HW, HW)
    mask[1::2] = mask[1::2][:, ::-1]
    mask = mask.reshape(-1)
    src = (np.arange(M) + 0.5) * (L / M) - 0.5
    src = np.maximum(src, 0.0)
    i0 = np.minimum(np.floor(src).astype(np.int64), L - 1)
    i1 = np.minimum(i0 + 1, L - 1)
    lam = (src - i0).astype(np.float32)
    S = np.zeros((L, M), np.float32)
    S[mask[i0], np.arange(M)] += (1 - lam)
    S[mask[i1], np.arange(M)] += lam
    blocks = np.zeros((6, 128, 128), np.float32)
    for j in range(3):
        s0 = (4 * j) // 3
        for t in range(2):
            blocks[j * 2 + t] = S[128 * (s0 + t):128 * (s0 + t + 1), 128 * j:128 * (j + 1)]
    # [128 l-part, 3 (j%3), 2 (pair t), 128 m]
    return blocks.transpose(1, 0, 2).reshape(128, 3, 2, 128).copy()


def _host_consts(W_kv, W_q, W_dw, W_po, temperature):
    c = np.arange(DIM)
    mask = np.where((c[:, None] // 32) == (c[None, :] // 32), 0.0, -30.0).astype(np.float32)
    tv = np.repeat(np.asarray(temperature, np.float32).reshape(HEADS), DIM // HEADS)
    # folded conv weights: (W_dw . W_q) -> [in-chan b, dy, dx, out o], then
    # chan-pair layout [128 p, 2 t, 9*256] with in-chan = p + 128 t
    fold = np.einsum("oayx,ab->byxo", np.asarray(W_dw, np.float32),
                     np.asarray(W_q, np.float32))
    wdw8 = fold.reshape(2, 128, 9 * DIM).transpose(1, 0, 2)
    # W_k^T chan-pair: [128 p, 2 t, 256 o] with in-chan = p + 128 t
    wk8 = W_kv[:DIM].T.reshape(2, 128, DIM).transpose(1, 0, 2)
    eye4 = np.concatenate([np.eye(128, dtype=np.float32)] * 4, axis=1)
    return {
        "w_k8": _f8(wk8),
        "w_v2": np.ascontiguousarray(W_kv[DIM:], np.float16),
        "w_poT": np.ascontiguousarray(W_po.T, np.float32),
        "w_dw8": _f8(wdw8),
        "s_mat8": _f8(_s_blocks()),
        "eye4": np.ascontiguousarray(eye4),
        "mask": np.ascontiguousarray(mask.reshape(2, 128, DIM)),
        "tempv": np.ascontiguousarray(tv.reshape(2, 128, 1)),
    }


def _make_tc_class():
    """TileContext subclass splitting the end-of-kernel drain waits.

    This container's walrus rejects >1 sem wait on CTRL-encoded instructions
    (Drain/NoOp). The stock Tile epilogue hangs every semaphore's final value
    on one Drain. Emit a chain of SP NoOps with one wait each instead, then a
    waitless drain: SP reaches it only after all sems hit their final values.
    """
    import bass_rust
    import concourse.mybir as mybir
    import concourse.tile as tile

    class SplitDrainTileContext(tile.TileContext):
        def _drain_and_barrier(self, tick_clock, wait_clock):
            probe = self.nc.sync.nop()
            wait_clock.add_sem_waits(
                probe.ins, bass_rust.ScopedClock({None: tick_clock.global_clock})
            )
            waits = list(probe.ins.sync_info.on_wait or [])
            probe.ins.sync_info.on_wait = waits[:1]
            for w in waits[1:]:
                n2 = self.nc.sync.nop()
                n2.ins.sync_info = mybir.SyncInfo(on_wait=[w], on_update=[])
            self.nc.sync.drain()
            self.nc.all_engine_barrier()
            assert self.sems is not None
            popped = self.nc._tile_sem_poison_stack.pop()
            assert popped is self._sem_poison
            self.nc.clear_and_free_semaphores(list(self.sems.allocated().values()))
            self.nc.all_engine_barrier()

    return SplitDrainTileContext


def _split_waits(nc):
    """Walrus in this container allows only one sem wait per instruction.
    Move extra waits onto same-engine NoOps inserted just before."""
    import concourse.mybir as mybir
    n = 0
    for f in nc.m.functions:
        for bb in f.blocks:
            out = []
            changed = False
            for inst in bb.instructions:
                si = inst.sync_info
                waits = list(si.on_wait) if si and si.on_wait else []
                if len(waits) > 1:
                    for w in waits[:-1]:
                        n += 1
                        nop = mybir.InstNoOp(name=f"I-sw{n}-{inst.name}", ins=[], outs=[])
                        nop.engine = inst.engine
                        nop.sync_info = mybir.SyncInfo(on_wait=[w], on_update=[])
                        out.append(nop)
                    si.on_wait = [waits[-1]]
                    changed = True
                out.append(inst)
            if changed:
                bb.instructions = out
    return n


def build_nc(split_waits=True, n_batches=BL, seq=None):
    from contextlib import ExitStack
    from collections import defaultdict
    import concourse.bass as bass
    import concourse.mybir as mybir
    from concourse.masks import make_identity

    f32 = mybir.dt.float32
    f32r = mybir.dt.float32r
    u32 = mybir.dt.uint32
    u8 = mybir.dt.uint8
    f16 = mybir.dt.float16
    f8 = mybir.dt.float8e4
    DR = mybir.MatmulPerfMode.DoubleRow
    Exp = mybir.ActivationFunctionType.Exp
    Ln = mybir.ActivationFunctionType.Ln

    def r(ap):
        return ap.bitcast(f32r)

    TC = _make_tc_class()
    nc = bass.Bass("TRN2", target_bir_lowering=False, debug=False)

    f8e5 = mybir.dt.float8e5
    x8d = nc.dram_tensor("x8_sh", [BL, 128, 2, M], f8, kind="ExternalInput").ap()
    x5d = nc.dram_tensor("x5_sh", [BL, 128, 2, M], f8e5, kind="ExternalInput").ap()
    y8d = nc.dram_tensor("y8_sh", [BL, 128, 2, L], f8, kind="ExternalInput").ap()
    wkd = nc.dram_tensor("w_k8", [128, 2, DIM], f8, kind="ExternalInput").ap()
    wvd = nc.dram_tensor("w_v2", [DIM, DIM], f16, kind="ExternalInput").ap()
    wpd = nc.dram_tensor("w_poT", [DIM, DIM], f32r, kind="ExternalInput").ap()
    wdd = nc.dram_tensor("w_dw8", [128, 2, 9 * DIM], f8, kind="ExternalInput").ap()
    sd = nc.dram_tensor("s_mat8", [128, 3, 2, 128], f8, kind="ExternalInput").ap()
    eyd = nc.dram_tensor("eye4", [128, 512], f32, kind="ExternalInput").ap()
    md = nc.dram_tensor("mask", [2, 128, DIM], f32, kind="ExternalInput").ap()
    td = nc.dram_tensor("tempv", [2, 128, 1], f32, kind="ExternalInput").ap()
    od = nc.dram_tensor("out", [BL, DIM, M], f16, kind="ExternalOutput").ap()

    with TC(nc) as tc, ExitStack() as ctx:
        P = lambda **kw: ctx.enter_context(tc.tile_pool(**kw))
        consts = P(name="consts", bufs=1)
        p_qp = P(name="p_qp", bufs=2)
        p_x = P(name="p_x", bufs=2)
        p_ct = P(name="p_ct", bufs=2)
        p_qk = P(name="p_qk", bufs=4)
        p_sm = P(name="p_sm", bufs=2)
        p_tn = P(name="p_tn", bufs=4)
        p_fin = P(name="p_fin", bufs=12)
        # global PSUM pools: 2+2+1+1+2 = 8 banks exactly
        pp_pc = P(name="pp_pc", bufs=2, space="PSUM")
        pp_ik = P(name="pp_ik", bufs=2, space="PSUM")
        pp_sc = P(name="pp_sc", bufs=1, space="PSUM")
        pp_g4 = P(name="pp_g4", bufs=1, space="PSUM")
        pp_pq = P(name="pp_pq", bufs=2, space="PSUM")

        # ---- constants; the conv weights are split across all three DMA
        # queues (behind each queue's first image chunk) so the first conv
        # tile can start ~2.5us in; softmax/out-chain consts are emitted
        # after batch 0's loads ----
        wdw8 = consts.tile([128, 2, 9 * DIM], f8, tag="wdw8", name="wdw8")
        smat8 = consts.tile([128, 3, 2, 128], f8, tag="smat8", name="smat8")
        wk8 = consts.tile([128, 2, DIM], f8, tag="wk8", name="wk8")

        def early_consts():
            for dy, eng in ((0, nc.scalar), (1, nc.sync), (2, nc.gpsimd)):
                c0, c1 = 3 * DIM * dy, 3 * DIM * (dy + 1)
                eng.dma_start(out=wdw8[:, :, c0:c1], in_=wdd[:, :, c0:c1])
            nc.scalar.dma_start(out=smat8[:], in_=sd)
            nc.scalar.dma_start(out=wk8[:], in_=wkd)
        wv2 = [consts.tile([128, DIM], f16, tag=f"wv2{k}", name=f"wv2{k}") for k in range(C128)]
        wp = [consts.tile([128, DIM], f32r, tag=f"wp{k}", name=f"wp{k}") for k in range(C128)]
        eye4 = consts.tile([128, 512], f32, tag="eye4", name="eye4")
        msk = [consts.tile([128, DIM], f32, tag=f"msk{k}", name=f"msk{k}") for k in range(2)]
        tmpv = [consts.tile([128, 1], f32, tag=f"tmpv{k}", name=f"tmpv{k}") for k in range(2)]

        def late_consts():
            for k in range(C128):
                sl = slice(128 * k, 128 * (k + 1))
                nc.sync.dma_start(out=wv2[k][:], in_=wvd[sl, :])
                nc.sync.dma_start(out=wp[k][:], in_=wpd[sl, :])
            nc.sync.dma_start(out=eye4[:], in_=eyd)
            for rr in range(2):
                nc.sync.dma_start(out=msk[rr][:], in_=md[rr])
                nc.sync.dma_start(out=tmpv[rr][:], in_=td[rr])
        ident = consts.tile([128, 128], f32, tag="ident", name="ident")
        make_identity(nc, ident[:])
        ones_row = consts.tile([1, 128], f32, tag="ones", name="ones")
        nc.vector.memset(ones_row[:], 1.0)
        ones_c16 = consts.tile([128, 1], f16, tag="ones16", name="ones16")
        nc.vector.memset(ones_c16[:], 1.0)

        state = defaultdict(dict)

        def emit_load_q1(vk, b, after_c0=None):
            s = state[vk]
            s["b"] = b
            s["x8"] = p_x.tile([128, 2, M], f8, tag="x8", name="x8")
            # W_q is folded into the conv weights on the host, so the conv
            # input is y itself: one DRAM DMA for the centered fp8 chan-pair
            # image (128, 2, 66, 64) with zeroed border rows, then two
            # on-chip shifted copies (dx-1 = -1/+1) built by SBUF->SBUF DMA;
            # the row-wrap artifacts land exactly on the edge columns the
            # memsets zero afterwards. Chunked x2 so early conv tiles start
            # before the whole image lands.
            qsh = [p_qp.tile([128, 2, HW + 2, HW], f8, tag=f"qsh{dx}",
                             name=f"qsh{dx}") for dx in range(3)]
            s["qsh"] = qsh
            tf = [qsh[dx].rearrange("p t a b -> p t (a b)") for dx in range(3)]
            # one shifted image copy per DMA queue, nothing queued ahead of
            # them: subtile tracking can't see through the pair-dim views, so
            # every conv waits for the LAST write to its qsh tile -- minimize
            # the makespan of the three copies instead of chunking cleverly
            HC = L // 2
            # batch 0's dx0 copy must ride the ACT hwdge queue to parallelize
            # the startup; later batches' dx0 goes on SP so ACT stays free
            # for PSUM copies during the busy stream window
            d0eng = nc.scalar if vk == 0 else nc.sync
            nc.sync.dma_start(out=tf[1][:, :, HW:HW + HC], in_=y8d[b][:, :, 0:HC])
            # qsh0[65+l] = y[l]
            d0eng.dma_start(out=tf[0][:, :, HW + 1:HW + 1 + HC],
                            in_=y8d[b][:, :, 0:HC])
            # qsh2[64+l] = y[l+1]  (tail col is memset)
            nc.gpsimd.dma_start(out=tf[2][:, :, HW:HW + HC],
                                in_=y8d[b][:, :, 1:HC + 1])
            nc.sync.dma_start(out=tf[1][:, :, HW + HC:HW + L],
                              in_=y8d[b][:, :, HC:L])
            d0eng.dma_start(out=tf[0][:, :, HW + 1 + HC:HW + 1 + L],
                            in_=y8d[b][:, :, HC:L])
            nc.gpsimd.dma_start(out=tf[2][:, :, HW + HC:HW + L - 1],
                                in_=y8d[b][:, :, HC + 1:L])
            if after_c0 is not None:
                after_c0()
            for cc in range(2):
                nc.sync.dma_start(
                    out=s["x8"][:, :, 1536 * cc:1536 * (cc + 1)],
                    in_=x8d[b][:, :, 1536 * cc:1536 * (cc + 1)])
            # e5m2 residual of x (for the fp8 error-feedback output matmul)
            s["x5"] = p_x.tile([128, 2, M], f8e5, tag="x5", name="x5")
            nc.gpsimd.dma_start(out=s["x5"][:], in_=x5d[b])
            # batch-0 memsets ride DVE (free at t=0); later batches use the
            # Pool queue so a memset waiting on its own batch's image can
            # never block the previous stream's PSUM copies on DVE
            meng = nc.vector if vk == 0 else nc.gpsimd
            for dx in range(3):
                meng.memset(qsh[dx][:, :, 0:1, :].bitcast(u32), 0)
            for dx in (0, 2):
                col = 0 if dx == 0 else HW - 1
                meng.memset(qsh[dx].bitcast(u8)[:, :, 1:1 + HC // HW, col:col + 1], 0)
            for dx in range(3):
                t = qsh[dx]
                meng.memset(t[:, :, HW + 1:HW + 2, :].bitcast(u32), 0)
                if dx != 1:
                    col = 0 if dx == 0 else HW - 1
                    meng.memset(
                        t.bitcast(u8)[:, :, 1 + HC // HW:HW + 1, col:col + 1], 0)

        def emit_stream(vk, hooks=(), defer_mtiles=False):
            s = state[vk]
            qsh, x8t = s["qsh"], s["x8"]
            ps_scc = pp_sc.tile([128, 512], f32, tag="pscc", name="pscc")
            ps_sc = [ps_scc[:, 0:DIM], ps_scc[:, DIM:512]]
            ps_g4 = pp_g4.tile([128, 512], f32, tag="g4", name="g4")
            s["sc"], s["scc"], s["g4"] = ps_sc, ps_scc, ps_g4
            ct8 = p_ct.tile([128, NST, DIM], f8, tag="ct8", name="ct8")
            flats = [qsh[dx].rearrange("p t a b -> p t (a b)") for dx in range(3)]
            qps = s["qps"] = []

            def emit_conv(j2):
                half = j2 % 2
                if half == 0:
                    s["pc"] = pp_pc.tile([128, 512], f32, tag="pc", name="pc")
                ps = s["pc"][:, DIM * half:DIM * (half + 1)]
                for dy in range(3):
                    for dx in range(3):
                        off = (2 * j2 + dy) * HW
                        nc.tensor.matmul(
                            ps, flats[dx][:, :, off:off + 128],
                            wdw8[:, :, (dy * 3 + dx) * DIM:(dy * 3 + dx + 1) * DIM],
                            start=(dy == 0 and dx == 0),
                            stop=(dy == 2 and dx == 2),
                            perf_mode=DR, skip_group_check=True)
                if half == 1:
                    # GPSIMD can't read PSUM on hw: alternate DVE/ACT
                    dst = ct8[:, j2 - 1:j2 + 1, :].rearrange("p a b -> p (a b)")
                    if (j2 // 2) % 2 == 0:
                        nc.vector.tensor_copy(dst, s["pc"][:])
                    else:
                        nc.scalar.copy(dst, s["pc"][:])

            def emit_mtile(j):
                s0 = (4 * j) // 3
                psik = pp_ik.tile([128, 512], f32, tag="pik", name="pik")
                # interp into [0:256] and kT into [256:512] of one PSUM bank
                # (per-element has_written drives overwrite-then-accumulate)
                nc.tensor.matmul(
                    psik[:, 0:DIM], smat8[:, j % 3], ct8[:, s0:s0 + 2, :],
                    start=True, stop=False, perf_mode=DR, skip_group_check=True)
                nc.tensor.matmul(
                    psik[:, DIM:512], x8t[:, :, 128 * j:128 * (j + 1)], wk8[:],
                    start=False, stop=True, perf_mode=DR, skip_group_check=True)
                if j % 2 == 0:
                    qps.append(p_qk.tile([128, 2, 512], f8, tag="qk8", name="qk8"))
                    nc.vector.tensor_copy(qps[-1][:, 0, :], psik[:])
                else:
                    nc.scalar.copy(qps[-1][:, 1, :], psik[:])

            def emit_scores(p):
                # lagged by a conv group so the qk8 copies are long done
                qp = qps[p]
                for rr in range(2):
                    nc.tensor.matmul(
                        ps_sc[rr], qp[:, :, 128 * rr:128 * (rr + 1)],
                        qp[:, :, DIM:512],
                        start=(p == 0 and rr == 0),
                        stop=(p == NMT // 2 - 1 and rr == 1),
                        perf_mode=DR, skip_group_check=True)
                # gram blocks for l2 norms (diag extracted in softmax)
                for g in range(4):
                    nc.tensor.matmul(
                        ps_g4[:, 128 * g:128 * (g + 1)],
                        qp[:, :, 128 * g:128 * (g + 1)],
                        qp[:, :, 128 * g:128 * (g + 1)],
                        start=(p == 0), stop=(p == NMT // 2 - 1),
                        perf_mode=DR, skip_group_check=True)

            # conv group jj; mtiles lag one group; scores lag mtiles so every
            # PE instruction's copy-dependencies resolved a whole group ago
            ndone = 0
            for jj in range(8):
                for q in range(4):
                    emit_conv(4 * jj + q)
                if jj < len(hooks):
                    hooks[jj]()
                if jj >= 1:
                    # pairs whose last mtile was emitted a full group ago
                    ready = max(0, (3 * jj - 3) // 2)
                    while ndone < ready:
                        emit_scores(ndone)
                        ndone += 1
                    for t in range(3):
                        emit_mtile(3 * (jj - 1) + t)
            if defer_mtiles:
                # hand the last three mtiles to the caller so it can
                # interleave them with fin tiles (no conv cover remains)
                s["tail_mtiles"] = [lambda t=t: emit_mtile(21 + t)
                                    for t in range(3)]
            else:
                for t in range(3):
                    emit_mtile(21 + t)

            def tail_scores(nd=ndone):
                for p in range(nd, NMT // 2):
                    emit_scores(p)

            # defer the last score pairs: their qk8 copies were just issued,
            # so the caller schedules them under later PE work
            s["tail_fn"] = tail_scores

        def softmax_parts(vk):
            """Softmax chain as three emission hooks, interleavable with the
            next batch's conv groups so the chain latency hides under them.
            Part 1 frees the pscc/g4 PSUM banks early (SBUF copies)."""
            s = state[vk]
            rqT, rZ = [], []

            def part1():
                s["tail_fn"]()  # this batch's deferred score pairs
                ps_g4 = s["g4"]
                # scores to SBUF, freeing the bank for the next stream
                scp = p_sm.tile([128, 512], f32, tag="scp", name="scp")
                s["scp"] = scp
                nc.scalar.copy(scp[:], s["scc"][:])
                # gram diagonals -> packed [1,512] row [nq0 nq1 nk0 nk1]
                geye = p_sm.tile([128, 512], f16, tag="geye", name="geye")
                nc.vector.tensor_mul(geye[:], ps_g4[:], eye4[:])
                ps_nqk = pp_pq.tile([1, 512], f32, tag="pq", name="pq")
                nc.tensor.matmul(ps_nqk[:], ones_c16[:], geye[:], start=True, stop=True)
                # 1/sqrt of the packed [nq | nk] row as exp(-ln(n)/2): keeps
                # every ACT function (copy/exp/ln) in one activation table,
                # avoiding two 1.3us table reloads per batch that sqrt causes
                rrow = p_sm.tile([1, 512], f32, tag="rrow", name="rrow", bufs=2)
                s["rrow"] = rrow
                nc.scalar.activation(rrow[:], ps_nqk[:], Ln)
                nc.scalar.activation(rrow[:], rrow[:], Exp, scale=-0.5)

            def part2():
                rrow = s["rrow"]
                # rnq back to per-partition columns (+ temperature)
                for rr in range(2):
                    pst = pp_pq.tile([128, 1], f32, tag="pq", name="pq")
                    nc.tensor.transpose(pst[:], rrow[:, 128 * rr:128 * (rr + 1)],
                                        ident[0:1, 0:1])
                    rqt = p_tn.tile([128, 1], f32, tag="rqt", name="rqt")
                    nc.vector.tensor_mul(rqt[:], pst[:], tmpv[rr][:])
                    rqT.append(rqt)
                # rnk broadcast down partitions via outer product
                psb = pp_pq.tile([128, DIM], f32, tag="pq", name="pq")
                nc.tensor.matmul(psb[:], ones_row[:], rrow[:, DIM:512],
                                 start=True, stop=True)
                rkb = p_sm.tile([128, DIM], f32, tag="rkb", name="rkb")
                s["rkb"] = rkb
                nc.scalar.copy(rkb[:], psb[:])

            def part3():
                scp, rkb = s["scp"], s["rkb"]
                E = [p_sm.tile([128, DIM], f32, tag="e", name="e") for _ in range(2)]
                # masked softmax, exp with fused row-sum; the all-SBUF chain
                # ops legally run on GPSIMD/Pool, freeing DVE for PSUM copies
                for rr in range(2):
                    sc = p_sm.tile([128, DIM], f32, tag="sc", name="sc")
                    # fused (scp * rq) * rkb in one DVE pass
                    nc.vector.scalar_tensor_tensor(
                        sc[:], scp[:, DIM * rr:DIM * (rr + 1)], rqT[rr][:],
                        rkb[:], op0=mybir.AluOpType.mult,
                        op1=mybir.AluOpType.mult)
                    nc.gpsimd.tensor_add(sc[:], sc[:], msk[rr][:])
                    z = p_tn.tile([128, 1], f32, tag="z", name="z")
                    nc.scalar.activation(E[rr][:], sc[:], Exp, accum_out=z[:])
                    rz = p_tn.tile([128, 1], f32, tag="rz", name="rz")
                    nc.vector.reciprocal(rz[:], z[:])
                    rZ.append(rz)
                # Ahat = E / Z (rows)
                Ahat = [p_sm.tile([128, DIM], f32r, tag="ah", name="ah", bufs=3)
                        for _ in range(2)]
                for rr in range(2):
                    nc.vector.tensor_scalar_mul(Ahat[rr][:], E[rr][:], rZ[rr][:])
                s["Ahat"] = Ahat

            return (part1, part2, part3)

        def emit_softmax(vk):
            for p in softmax_parts(vk):
                p()

        def out_head(vk):
            s = state[vk]
            Ahat = s["Ahat"]
            # m1t[d,o] = (W_po @ Ahat)^T ; wch[c,o] = (W_po @ Ahat @ W_v)^T
            m1t = [p_sm.tile([128, DIM], f16, tag="m1t", name="m1t") for _ in range(2)]
            for d in range(2):
                ps = pp_pq.tile([128, DIM], f32, tag="pq", name="pq")
                for k in range(C128):
                    nc.tensor.matmul(
                        ps[:], r(Ahat[k][:, 128 * d:128 * (d + 1)]), r(wp[k][:]),
                        start=(k == 0), stop=(k == C128 - 1))
                nc.scalar.copy(m1t[d][:], ps[:])
            # wch in e4m3 + e5m2 residual: the final matmul runs as fp8
            # DoubleRow with error feedback (wch8@x8 + wch8@dx + dwch@x8),
            # adding only ~0.2% output error but halving the PE cost
            wch8 = p_sm.tile([128, 2, DIM], f8, tag="wch8", name="wch8")
            dwch5 = p_sm.tile([128, 2, DIM], f8e5, tag="dwch5", name="dwch5")
            s["wch8"], s["dwch5"] = wch8, dwch5
            for cb in range(2):
                ps = pp_pq.tile([128, DIM], f32, tag="pq", name="pq")
                for d in range(2):
                    nc.tensor.matmul(
                        ps[:], wv2[d][:, 128 * cb:128 * (cb + 1)], m1t[d][:],
                        start=(d == 0), stop=(d == 1))
                nc.scalar.copy(wch8[:, cb, :], ps[:])
                nc.vector.tensor_sub(dwch5[:, cb, :], ps[:], wch8[:, cb, :])

        def fin_tile(vk, i, mixed=True):
            """One 1024-wide output tile of W_chain @ x (fp8 DoubleRow with
            error feedback). mixed=False keeps PSUM in the pq pool so tiles
            can interleave with a stream (whose psik owns the ik pool)."""
            s = state[vk]
            b = s["b"]
            x8t, x5t = s["x8"], s["x5"]
            wch8, dwch5 = s["wch8"], s["dwch5"]
            o, h = divmod(i, NM512 // 2)
            st = p_fin.tile([128, 1024], f16, tag="fin", name="fin")
            for half in range(2):
                n = 2 * h + half
                pool = pp_pq if (not mixed or (2 * i + half) % 2 == 0) else pp_ik
                tg = "pq" if pool is pp_pq else "pik"
                ps = pool.tile([128, 512], f32, tag=tg, name=tg)
                osl = slice(128 * o, 128 * (o + 1))
                nsl = slice(512 * n, 512 * (n + 1))
                nc.tensor.matmul(ps[:], wch8[:, :, osl], x8t[:, :, nsl],
                                 start=True, stop=False, perf_mode=DR)
                nc.tensor.matmul(ps[:], wch8[:, :, osl], x5t[:, :, nsl],
                                 start=False, stop=False, perf_mode=DR)
                nc.tensor.matmul(ps[:], dwch5[:, :, osl], x8t[:, :, nsl],
                                 start=False, stop=True, perf_mode=DR)
                if half == 0:
                    nc.scalar.copy(st[:, 0:512], ps[:])
                else:
                    nc.vector.tensor_copy(st[:, 512:1024], ps[:])
            deng = nc.sync if h % 2 == 0 else nc.gpsimd
            deng.dma_start(
                out=od[b, 128 * o:128 * (o + 1), 1024 * h:1024 * (h + 1)],
                in_=st[:])

        def fin_tile_split(vk, i):
            # last tile of the kernel: two half staging tiles with parallel
            # DMAs on separate queues shortens the post-matmul drain chain
            s = state[vk]
            b = s["b"]
            x8t, x5t = s["x8"], s["x5"]
            wch8, dwch5 = s["wch8"], s["dwch5"]
            o, h = divmod(i, NM512 // 2)
            for half in range(2):
                n = 2 * h + half
                pool = pp_pq if half == 0 else pp_ik
                tg = "pq" if pool is pp_pq else "pik"
                ps = pool.tile([128, 512], f32, tag=tg, name=tg)
                osl = slice(128 * o, 128 * (o + 1))
                nsl = slice(512 * n, 512 * (n + 1))
                nc.tensor.matmul(ps[:], wch8[:, :, osl], x8t[:, :, nsl],
                                 start=True, stop=False, perf_mode=DR)
                nc.tensor.matmul(ps[:], wch8[:, :, osl], x5t[:, :, nsl],
                                 start=False, stop=False, perf_mode=DR)
                nc.tensor.matmul(ps[:], dwch5[:, :, osl], x8t[:, :, nsl],
                                 start=False, stop=True, perf_mode=DR)
                st = p_fin.tile([128, 512], f16, tag="fin2", name="fin2")
                if half == 0:
                    nc.scalar.copy(st[:], ps[:])
                else:
                    nc.vector.tensor_copy(st[:], ps[:])
                deng = nc.sync if half == 0 else nc.gpsimd
                deng.dma_start(
                    out=od[b, 128 * o:128 * (o + 1), 512 * n:512 * (n + 1)],
                    in_=st[:])

        def emit_out(vk, hooks=(), tiles=None, split_last=False):
            s = state[vk]
            if "wch8" not in s:
                out_head(vk)
            if len(hooks) > 0:
                hooks[0]()
            idxs = list(range(2 * (NM512 // 2)) if tiles is None else tiles)
            for i in idxs:
                if split_last and i == idxs[-1]:
                    fin_tile_split(vk, i)
                else:
                    fin_tile(vk, i)
                if i + 1 < len(hooks):
                    hooks[i + 1]()

        # software pipeline: q1(b+1) fills the PE while batch b's softmax
        # chain runs *inside* stream(b+1) via hooks (its tiny PE ops slot
        # between conv groups; part 1 frees the score/gram PSUM banks before
        # stream(b+1)'s first scores matmul needs them)
        sq_ = list(range(n_batches)) if seq is None else list(seq)
        vis = [(i, b) for i, b in enumerate(sq_)]
        n = len(vis)
        emit_load_q1(0, vis[0][1], after_c0=early_consts)
        emit_stream(0)
        late_consts()
        for i in range(1, n):
            emit_load_q1(i, vis[i][1])
            # softmax(i-1) and batch i-1's out-head hide inside stream(i)'s
            # conv groups
            sm = softmax_parts(i - 1)
            emit_stream(i, hooks=(*sm, lambda vv=i - 1: out_head(vv)))
            if i >= 2:
                emit_out(i - 2)
        if n > 1:
            # tail: last softmax chain hides under out(n-2)'s fin stream
            emit_out(n - 2, hooks=softmax_parts(n - 1))
            out_head(n - 1)
            emit_out(n - 1, split_last=True)
        else:
            emit_softmax(0)
            emit_out(0)

    if split_waits:
        _split_waits(nc)
    return nc


def _get_nc():
    if "nc" not in _CACHE:
        _CACHE["nc"] = build_nc()
    return _CACHE["nc"]


def make_inputs(inputs):
    """Host-side prep: consts + per-core sharded activations."""
    consts = _host_consts(inputs["W_kv"], inputs["W_q"], inputs["W_dw"],
                          inputs["W_po"], inputs["temperature"])
    xf = np.asarray(inputs["x"], np.float32)
    x8f = _f8(xf)
    x5f = _f8e5(xf - np.float32(x8f))
    x8 = x8f.reshape(B, 2, 128, M).transpose(0, 2, 1, 3)
    x5 = x5f.reshape(B, 2, 128, M).transpose(0, 2, 1, 3)
    y8 = _f8(inputs["y"]).reshape(B, 2, 128, L).transpose(0, 2, 1, 3)
    in_maps = []
    for i in range(NCORES):
        m = dict(consts)
        m["x8_sh"] = np.ascontiguousarray(x8[BL * i:BL * (i + 1)])
        m["x5_sh"] = np.ascontiguousarray(x5[BL * i:BL * (i + 1)])
        m["y8_sh"] = np.ascontiguousarray(y8[BL * i:BL * (i + 1)])
        in_maps.append(m)
    return in_maps


def run(inputs, trace=False, trace_kwargs=None):
    from concourse.bass_utils import run_bass_kernel_spmd

    nc = _get_nc()
    in_maps = make_inputs(inputs)
    res = run_bass_kernel_spmd(
        nc, in_maps, core_ids=list(range(NCORES)), trace=trace,
        trace_kwargs=trace_kwargs or {})
    out = np.concatenate(
        [np.asarray(res.results[i]["out"], np.float32) for i in range(NCORES)],
        axis=0)
    return out, res


def kernel(**inputs) -> np.ndarray:
    out, _ = run(inputs, trace=False)
    return out


# revision 101
# speedup vs baseline: 2.7826x; 1.0100x over previous
"""Trainium2 Bass kernel for nn_Attention1 (channel attention transformer block).

Reference computation (per batch):
  kv = W_kv @ x ; k, v = split(kv)                    # pointwise conv over m=3072
  q  = conv3x3(W_q @ y, W_dw)                         # 1x1 then full 3x3, 64x64 image
  q  = linear_interp(snake(q.flatten(HW)), 4096->3072)
  q, k = l2norm over m ; attn = softmax(q @ k^T * temp) per 32-channel head
  out = W_po @ (attn @ v)

Sharding: data-parallel over batch, 16 batches / 8 cores = 2 per core. SPMD,
no collectives; per-core outputs are concatenated on host.

Per-core kernel layout strategy. All heavy matmuls run in fp8(e4m3) with
perf_mode=DoubleRow: operands are laid out [128 partitions, 2, cols] so one
matmul contracts 256 channels (the pair dim sums in the PE), doubling PE
throughput vs fp16. fp8 is safe for everything that feeds the l2-normalized
q/k (scale and elementwise quantization wash out to ~0.1% at score level);
the final W_chain @ x matmul stays fp16 since it writes the output directly.
  - q path   : y kept as fp8 chan-pair images, zero-padded 66x64, three
               horizontally shifted copies; 3x3 conv as 9 DoubleRow matmuls
               (image stationary), both 128-chan blocks contracted at once.
               conv outputs for two row-tiles share one PSUM bank, one copy
               into a contiguous fp8 ct arena (32 x [128,256] slots)
  - snake+interp : fused sparse matrix S applied via one DoubleRow matmul
               per m-tile (S blocks repeat with period 3; the two
               contributing 128x128 blocks ride the pair dim)
  - kT       : x chan-pair fp8 stationary, W_k^T pair fp8 moving -> kT
               (m on partitions), same PSUM bank as interp (one copy)
  - scores   : qk tiles copied to fp8 pair arenas (two m-tiles per pair);
               q@kT via DoubleRow contracting 256 m-rows per matmul
  - norms    : gram blocks qk8[s]^T qk8[s] accumulate in PSUM; diagonal
               extracted with an eye mask (DVE) + ones-matmul -> packed
               [1,512] row of |q|^2,|k|^2 (replaces explicit squares)
  - softmax  : per-head masking via additive -30 mask over the full 256-wide
               score rows; exp on ScalarE with fused row-sum (accum_out);
               1/Z folded into Ahat as a per-row scale
  - out      : W_po @ A @ W_v folded into a 256x256 chain (f32r/f16 small
               matmuls), then W_chain @ x streamed in fp16; output DMA'd
               as f16 and upcast on host
"""
import numpy as np

HEADS = 8
B, DIM, M = 16, 256, 3072
HW = 64
L = HW * HW          # 4096 flattened conv spatial size
NCORES = 8
BL = B // NCORES     # batches per core
C128 = DIM // 128    # channel 128-tiles (2)
NM512 = M // 512     # m-dim 512-tiles (6)
NMT = M // 128       # m-dim 128-tiles (24)
NST = L // 128       # conv-spatial 128-tiles (32)

_CACHE = {}


def _f8(a):
    import ml_dtypes
    return np.asarray(a, np.float32).astype(ml_dtypes.float8_e4m3)


def _f8e5(a):
    import ml_dtypes
    return np.asarray(a, np.float32).astype(ml_dtypes.float8_e5m2)


def _s_blocks():
    """Snake+interp as a sparse matrix; 6 distinct 128x128 blocks (period 3)."""
    mask = np.arange(L).reshape(HW, HW)
    mask[1::2] = mask[1::2][:, ::-1]
    mask = mask.reshape(-1)
    src = (np.arange(M) + 0.5) * (L / M) - 0.5
    src = np.maximum(src, 0.0)
    i0 = np.minimum(np.floor(src).astype(np.int64), L - 1)
    i1 = np.minimum(i0 + 1, L - 1)
    lam = (src - i0).astype(np.float32)
    S = np.zeros((L, M), np.float32)
    S[mask[i0], np.arange(M)] += (1 - lam)
    S[mask[i1], np.arange(M)] += lam
    blocks = np.zeros((6, 128, 128), np.float32)
    for j in range(3):
        s0 = (4 * j) // 3
        for t in range(2):
            blocks[j * 2 + t] = S[128 * (s0 + t):128 * (s0 + t + 1), 128 * j:128 * (j + 1)]
    # [128 l-part, 3 (j%3), 2 (pair t), 128 m]
    return blocks.transpose(1, 0, 2).reshape(128, 3, 2, 128).copy()


def _host_consts(W_kv, W_q, W_dw, W_po, temperature):
    c = np.arange(DIM)
    mask = np.where((c[:, None] // 32) == (c[None, :] // 32), 0.0, -30.0).astype(np.float32)
    tv = np.repeat(np.asarray(temperature, np.float32).reshape(HEADS), DIM // HEADS)
    # folded conv weights: (W_dw . W_q) -> [in-chan b, dy, dx, out o], then
    # chan-pair layout [128 p, 2 t, 9*256] with in-chan = p + 128 t
    fold = np.einsum("oayx,ab->byxo", np.asarray(W_dw, np.float32),
                     np.asarray(W_q, np.float32))
    wdw8 = fold.reshape(2, 128, 9 * DIM).transpose(1, 0, 2)
    # W_k^T chan-pair: [128 p, 2 t, 256 o] with in-chan = p + 128 t
    wk8 = W_kv[:DIM].T.reshape(2, 128, DIM).transpose(1, 0, 2)
    eye4 = np.concatenate([np.eye(128, dtype=np.float32)] * 4, axis=1)
    return {
        "w_k8": _f8(wk8),
        "w_v2": np.ascontiguousarray(W_kv[DIM:], np.float16),
        "w_poT": np.ascontiguousarray(W_po.T, np.float32),
        "w_dw8": _f8(wdw8),
        "s_mat8": _f8(_s_blocks()),
        "eye4": np.ascontiguousarray(eye4),
        "mask": np.ascontiguousarray(mask.reshape(2, 128, DIM)),
        "tempv": np.ascontiguousarray(tv.reshape(2, 128, 1)),
    }


def _make_tc_class():
    """TileContext subclass splitting the end-of-kernel drain waits.

    This container's walrus rejects >1 sem wait on CTRL-encoded instructions
    (Drain/NoOp). The stock Tile epilogue hangs every semaphore's final value
    on one Drain. Emit a chain of SP NoOps with one wait each instead, then a
    waitless drain: SP reaches it only after all sems hit their final values.
    """
    import bass_rust
    import concourse.mybir as mybir
    import concourse.tile as tile

    class SplitDrainTileContext(tile.TileContext):
        def _drain_and_barrier(self, tick_clock, wait_clock):
            probe = self.nc.sync.nop()
            wait_clock.add_sem_waits(
                probe.ins, bass_rust.ScopedClock({None: tick_clock.global_clock})
            )
            waits = list(probe.ins.sync_info.on_wait or [])
            probe.ins.sync_info.on_wait = waits[:1]
            for w in waits[1:]:
                n2 = self.nc.sync.nop()
                n2.ins.sync_info = mybir.SyncInfo(on_wait=[w], on_update=[])
            self.nc.sync.drain()
            self.nc.all_engine_barrier()
            assert self.sems is not None
            popped = self.nc._tile_sem_poison_stack.pop()
            assert popped is self._sem_poison
            self.nc.clear_and_free_semaphores(list(self.sems.allocated().values()))
            self.nc.all_engine_barrier()

    return SplitDrainTileContext


def _split_waits(nc):
    """Walrus in this container allows only one sem wait per instruction.
    Move extra waits onto same-engine NoOps inserted just before."""
    import concourse.mybir as mybir
    n = 0
    for f in nc.m.functions:
        for bb in f.blocks:
            out = []
            changed = False
            for inst in bb.instructions:
                si = inst.sync_info
                waits = list(si.on_wait) if si and si.on_wait else []
                if len(waits) > 1:
                    for w in waits[:-1]:
                        n += 1
                        nop = mybir.InstNoOp(name=f"I-sw{n}-{inst.name}", ins=[], outs=[])
                        nop.engine = inst.engine
                        nop.sync_info = mybir.SyncInfo(on_wait=[w], on_update=[])
                        out.append(nop)
                    si.on_wait = [waits[-1]]
                    changed = True
                out.append(inst)
            if changed:
                bb.instructions = out
    return n


def build_nc(split_waits=True, n_batches=BL, seq=None):
    from contextlib import ExitStack
    from collections import defaultdict
    import concourse.bass as bass
    import concourse.mybir as mybir
    from concourse.masks import make_identity

    f32 = mybir.dt.float32
    f32r = mybir.dt.float32r
    u32 = mybir.dt.uint32
    u8 = mybir.dt.uint8
    f16 = mybir.dt.float16
    f8 = mybir.dt.float8e4
    DR = mybir.MatmulPerfMode.DoubleRow
    Exp = mybir.ActivationFunctionType.Exp
    Ln = mybir.ActivationFunctionType.Ln

    def r(ap):
        return ap.bitcast(f32r)

    TC = _make_tc_class()
    nc = bass.Bass("TRN2", target_bir_lowering=False, debug=False)

    f8e5 = mybir.dt.float8e5
    x8d = nc.dram_tensor("x8_sh", [BL, 128, 2, M], f8, kind="ExternalInput").ap()
    x5d = nc.dram_tensor("x5_sh", [BL, 128, 2, M], f8e5, kind="ExternalInput").ap()
    y8d = nc.dram_tensor("y8_sh", [BL, 128, 2, L], f8, kind="ExternalInput").ap()
    wkd = nc.dram_tensor("w_k8", [128, 2, DIM], f8, kind="ExternalInput").ap()
    wvd = nc.dram_tensor("w_v2", [DIM, DIM], f16, kind="ExternalInput").ap()
    wpd = nc.dram_tensor("w_poT", [DIM, DIM], f32r, kind="ExternalInput").ap()
    wdd = nc.dram_tensor("w_dw8", [128, 2, 9 * DIM], f8, kind="ExternalInput").ap()
    sd = nc.dram_tensor("s_mat8", [128, 3, 2, 128], f8, kind="ExternalInput").ap()
    eyd = nc.dram_tensor("eye4", [128, 512], f32, kind="ExternalInput").ap()
    md = nc.dram_tensor("mask", [2, 128, DIM], f32, kind="ExternalInput").ap()
    td = nc.dram_tensor("tempv", [2, 128, 1], f32, kind="ExternalInput").ap()
    od = nc.dram_tensor("out", [BL, DIM, M], f16, kind="ExternalOutput").ap()

    with TC(nc) as tc, ExitStack() as ctx:
        P = lambda **kw: ctx.enter_context(tc.tile_pool(**kw))
        consts = P(name="consts", bufs=1)
        p_qp = P(name="p_qp", bufs=2)
        p_x = P(name="p_x", bufs=2)
        p_ct = P(name="p_ct", bufs=2)
        p_qk = P(name="p_qk", bufs=4)
        p_sm = P(name="p_sm", bufs=2)
        p_tn = P(name="p_tn", bufs=4)
        p_fin = P(name="p_fin", bufs=12)
        # global PSUM pools: 2+2+1+1+2 = 8 banks exactly
        pp_pc = P(name="pp_pc", bufs=2, space="PSUM")
        pp_ik = P(name="pp_ik", bufs=2, space="PSUM")
        pp_sc = P(name="pp_sc", bufs=1, space="PSUM")
        pp_g4 = P(name="pp_g4", bufs=1, space="PSUM")
        pp_pq = P(name="pp_pq", bufs=2, space="PSUM")

        # ---- constants; the conv weights are split across all three DMA
        # queues (behind each queue's first image chunk) so the first conv
        # tile can start ~2.5us in; softmax/out-chain consts are emitted
        # after batch 0's loads ----
        wdw8 = consts.tile([128, 2, 9 * DIM], f8, tag="wdw8", name="wdw8")
        smat8 = consts.tile([128, 3, 2, 128], f8, tag="smat8", name="smat8")
        wk8 = consts.tile([128, 2, DIM], f8, tag="wk8", name="wk8")

        def early_consts():
            for dy, eng in ((0, nc.scalar), (1, nc.sync), (2, nc.gpsimd)):
                c0, c1 = 3 * DIM * dy, 3 * DIM * (dy + 1)
                eng.dma_start(out=wdw8[:, :, c0:c1], in_=wdd[:, :, c0:c1])
            nc.scalar.dma_start(out=smat8[:], in_=sd)
            nc.scalar.dma_start(out=wk8[:], in_=wkd)
            # dummy activation primes the copy/exp/ln table while the image
            # DMAs land, so the first real ACT copy doesn't pay the ~1.3us
            # table load mid-stream
            nc.scalar.copy(scr[:], ones_row[:])
        wv2 = [consts.tile([128, DIM], f16, tag=f"wv2{k}", name=f"wv2{k}") for k in range(C128)]
        wp = [consts.tile([128, DIM], f32r, tag=f"wp{k}", name=f"wp{k}") for k in range(C128)]
        eye4 = consts.tile([128, 512], f32, tag="eye4", name="eye4")
        msk = [consts.tile([128, DIM], f32, tag=f"msk{k}", name=f"msk{k}") for k in range(2)]
        tmpv = [consts.tile([128, 1], f32, tag=f"tmpv{k}", name=f"tmpv{k}") for k in range(2)]

        def late_consts():
            for k in range(C128):
                sl = slice(128 * k, 128 * (k + 1))
                nc.sync.dma_start(out=wv2[k][:], in_=wvd[sl, :])
                nc.sync.dma_start(out=wp[k][:], in_=wpd[sl, :])
            nc.sync.dma_start(out=eye4[:], in_=eyd)
            for rr in range(2):
                nc.sync.dma_start(out=msk[rr][:], in_=md[rr])
                nc.sync.dma_start(out=tmpv[rr][:], in_=td[rr])
        ident = consts.tile([128, 128], f32, tag="ident", name="ident")
        make_identity(nc, ident[:])
        ones_row = consts.tile([1, 128], f32, tag="ones", name="ones")
        nc.vector.memset(ones_row[:], 1.0)
        scr = consts.tile([1, 128], f32, tag="scr", name="scr")
        ones_c16 = consts.tile([128, 1], f16, tag="ones16", name="ones16")
        nc.vector.memset(ones_c16[:], 1.0)

        state = defaultdict(dict)

        def emit_load_q1(vk, b, after_c0=None):
            s = state[vk]
            s["b"] = b
            s["x8"] = p_x.tile([128, 2, M], f8, tag="x8", name="x8")
            # W_q is folded into the conv weights on the host, so the conv
            # input is y itself: one DRAM DMA for the centered fp8 chan-pair
            # image (128, 2, 66, 64) with zeroed border rows, then two
            # on-chip shifted copies (dx-1 = -1/+1) built by SBUF->SBUF DMA;
            # the row-wrap artifacts land exactly on the edge columns the
            # memsets zero afterwards. Chunked x2 so early conv tiles start
            # before the whole image lands.
            qsh = [p_qp.tile([128, 2, HW + 2, HW], f8, tag=f"qsh{dx}",
                             name=f"qsh{dx}") for dx in range(3)]
            s["qsh"] = qsh
            tf = [qsh[dx].rearrange("p t a b -> p t (a b)") for dx in range(3)]
            # one shifted image copy per DMA queue, nothing queued ahead of
            # them: subtile tracking can't see through the pair-dim views, so
            # every conv waits for the LAST write to its qsh tile -- minimize
            # the makespan of the three copies instead of chunking cleverly
            HC = L // 2
            # batch 0's dx0 copy must ride the ACT hwdge queue to parallelize
            # the startup; later batches' dx0 goes on SP so ACT stays free
            # for PSUM copies during the busy stream window
            d0eng = nc.scalar if vk == 0 else nc.sync
            nc.sync.dma_start(out=tf[1][:, :, HW:HW + HC], in_=y8d[b][:, :, 0:HC])
            # qsh0[65+l] = y[l]
            d0eng.dma_start(out=tf[0][:, :, HW + 1:HW + 1 + HC],
                            in_=y8d[b][:, :, 0:HC])
            # qsh2[64+l] = y[l+1]  (tail col is memset)
            nc.gpsimd.dma_start(out=tf[2][:, :, HW:HW + HC],
                                in_=y8d[b][:, :, 1:HC + 1])
            nc.sync.dma_start(out=tf[1][:, :, HW + HC:HW + L],
                              in_=y8d[b][:, :, HC:L])
            d0eng.dma_start(out=tf[0][:, :, HW + 1 + HC:HW + 1 + L],
                            in_=y8d[b][:, :, HC:L])
            nc.gpsimd.dma_start(out=tf[2][:, :, HW + HC:HW + L - 1],
                                in_=y8d[b][:, :, HC + 1:L])
            if after_c0 is not None:
                after_c0()
            for cc in range(2):
                nc.sync.dma_start(
                    out=s["x8"][:, :, 1536 * cc:1536 * (cc + 1)],
                    in_=x8d[b][:, :, 1536 * cc:1536 * (cc + 1)])
            # e5m2 residual of x (for the fp8 error-feedback output matmul)
            s["x5"] = p_x.tile([128, 2, M], f8e5, tag="x5", name="x5")
            nc.gpsimd.dma_start(out=s["x5"][:], in_=x5d[b])
            # batch-0 memsets ride DVE (free at t=0); later batches use the
            # Pool queue so a memset waiting on its own batch's image can
            # never block the previous stream's PSUM copies on DVE
            meng = nc.vector if vk == 0 else nc.gpsimd
            for dx in range(3):
                meng.memset(qsh[dx][:, :, 0:1, :].bitcast(u32), 0)
            for dx in (0, 2):
                col = 0 if dx == 0 else HW - 1
                meng.memset(qsh[dx].bitcast(u8)[:, :, 1:1 + HC // HW, col:col + 1], 0)
            for dx in range(3):
                t = qsh[dx]
                meng.memset(t[:, :, HW + 1:HW + 2, :].bitcast(u32), 0)
                if dx != 1:
                    col = 0 if dx == 0 else HW - 1
                    meng.memset(
                        t.bitcast(u8)[:, :, 1 + HC // HW:HW + 1, col:col + 1], 0)

        def emit_stream(vk, hooks=(), defer_mtiles=False):
            s = state[vk]
            qsh, x8t = s["qsh"], s["x8"]
            ps_scc = pp_sc.tile([128, 512], f32, tag="pscc", name="pscc")
            ps_sc = [ps_scc[:, 0:DIM], ps_scc[:, DIM:512]]
            ps_g4 = pp_g4.tile([128, 512], f32, tag="g4", name="g4")
            s["sc"], s["scc"], s["g4"] = ps_sc, ps_scc, ps_g4
            ct8 = p_ct.tile([128, NST, DIM], f8, tag="ct8", name="ct8")
            flats = [qsh[dx].rearrange("p t a b -> p t (a b)") for dx in range(3)]
            qps = s["qps"] = []

            def emit_conv(j2):
                half = j2 % 2
                if half == 0:
                    s["pc"] = pp_pc.tile([128, 512], f32, tag="pc", name="pc")
                ps = s["pc"][:, DIM * half:DIM * (half + 1)]
                for dy in range(3):
                    for dx in range(3):
                        off = (2 * j2 + dy) * HW
                        nc.tensor.matmul(
                            ps, flats[dx][:, :, off:off + 128],
                            wdw8[:, :, (dy * 3 + dx) * DIM:(dy * 3 + dx + 1) * DIM],
                            start=(dy == 0 and dx == 0),
                            stop=(dy == 2 and dx == 2),
                            perf_mode=DR, skip_group_check=True)
                if half == 1:
                    # GPSIMD can't read PSUM on hw: alternate DVE/ACT
                    dst = ct8[:, j2 - 1:j2 + 1, :].rearrange("p a b -> p (a b)")
                    if (j2 // 2) % 2 == 0:
                        nc.vector.tensor_copy(dst, s["pc"][:])
                    else:
                        nc.scalar.copy(dst, s["pc"][:])

            def emit_mtile(j):
                s0 = (4 * j) // 3
                psik = pp_ik.tile([128, 512], f32, tag="pik", name="pik")
                # interp into [0:256] and kT into [256:512] of one PSUM bank
                # (per-element has_written drives overwrite-then-accumulate)
                nc.tensor.matmul(
                    psik[:, 0:DIM], smat8[:, j % 3], ct8[:, s0:s0 + 2, :],
                    start=True, stop=False, perf_mode=DR, skip_group_check=True)
                nc.tensor.matmul(
                    psik[:, DIM:512], x8t[:, :, 128 * j:128 * (j + 1)], wk8[:],
                    start=False, stop=True, perf_mode=DR, skip_group_check=True)
                if j % 2 == 0:
                    qps.append(p_qk.tile([128, 2, 512], f8, tag="qk8", name="qk8"))
                    nc.vector.tensor_copy(qps[-1][:, 0, :], psik[:])
                else:
                    nc.scalar.copy(qps[-1][:, 1, :], psik[:])

            def emit_scores(p):
                # lagged by a conv group so the qk8 copies are long done
                qp = qps[p]
                for rr in range(2):
                    nc.tensor.matmul(
                        ps_sc[rr], qp[:, :, 128 * rr:128 * (rr + 1)],
                        qp[:, :, DIM:512],
                        start=(p == 0 and rr == 0),
                        stop=(p == NMT // 2 - 1 and rr == 1),
                        perf_mode=DR, skip_group_check=True)
                # gram blocks for l2 norms (diag extracted in softmax)
                for g in range(4):
                    nc.tensor.matmul(
                        ps_g4[:, 128 * g:128 * (g + 1)],
                        qp[:, :, 128 * g:128 * (g + 1)],
                        qp[:, :, 128 * g:128 * (g + 1)],
                        start=(p == 0), stop=(p == NMT // 2 - 1),
                        perf_mode=DR, skip_group_check=True)

            # conv group jj; mtiles lag one group; scores lag mtiles so every
            # PE instruction's copy-dependencies resolved a whole group ago
            ndone = 0
            for jj in range(8):
                for q in range(4):
                    emit_conv(4 * jj + q)
                if jj < len(hooks):
                    hooks[jj]()
                if jj >= 1:
                    # pairs whose last mtile was emitted a full group ago
                    ready = max(0, (3 * jj - 3) // 2)
                    while ndone < ready:
                        emit_scores(ndone)
                        ndone += 1
                    for t in range(3):
                        emit_mtile(3 * (jj - 1) + t)
            if defer_mtiles:
                # hand the last three mtiles to the caller so it can
                # interleave them with fin tiles (no conv cover remains)
                s["tail_mtiles"] = [lambda t=t: emit_mtile(21 + t)
                                    for t in range(3)]
            else:
                for t in range(3):
                    emit_mtile(21 + t)

            def tail_scores(nd=ndone):
                for p in range(nd, NMT // 2):
                    emit_scores(p)

            # defer the last score pairs: their qk8 copies were just issued,
            # so the caller schedules them under later PE work
            s["tail_fn"] = tail_scores

        def softmax_parts(vk):
            """Softmax chain as three emission hooks, interleavable with the
            next batch's conv groups so the chain latency hides under them.
            Part 1 frees the pscc/g4 PSUM banks early (SBUF copies)."""
            s = state[vk]
            rqT, rZ = [], []

            def part1():
                s["tail_fn"]()  # this batch's deferred score pairs
                ps_g4 = s["g4"]
                # scores to SBUF, freeing the bank for the next stream
                scp = p_sm.tile([128, 512], f32, tag="scp", name="scp")
                s["scp"] = scp
                nc.scalar.copy(scp[:], s["scc"][:])
                # gram diagonals -> packed [1,512] row [nq0 nq1 nk0 nk1]
                geye = p_sm.tile([128, 512], f16, tag="geye", name="geye")
                nc.vector.tensor_mul(geye[:], ps_g4[:], eye4[:])
                ps_nqk = pp_pq.tile([1, 512], f32, tag="pq", name="pq")
                nc.tensor.matmul(ps_nqk[:], ones_c16[:], geye[:], start=True, stop=True)
                # 1/sqrt of the packed [nq | nk] row as exp(-ln(n)/2): keeps
                # every ACT function (copy/exp/ln) in one activation table,
                # avoiding two 1.3us table reloads per batch that sqrt causes
                rrow = p_sm.tile([1, 512], f32, tag="rrow", name="rrow", bufs=2)
                s["rrow"] = rrow
                nc.scalar.activation(rrow[:], ps_nqk[:], Ln)
                nc.scalar.activation(rrow[:], rrow[:], Exp, scale=-0.5)

            def part2():
                rrow = s["rrow"]
                # rnq back to per-partition columns (+ temperature)
                for rr in range(2):
                    pst = pp_pq.tile([128, 1], f32, tag="pq", name="pq")
                    nc.tensor.transpose(pst[:], rrow[:, 128 * rr:128 * (rr + 1)],
                                        ident[0:1, 0:1])
                    rqt = p_tn.tile([128, 1], f32, tag="rqt", name="rqt")
                    nc.vector.tensor_mul(rqt[:], pst[:], tmpv[rr][:])
                    rqT.append(rqt)
                # rnk broadcast down partitions via outer product
                psb = pp_pq.tile([128, DIM], f32, tag="pq", name="pq")
                nc.tensor.matmul(psb[:], ones_row[:], rrow[:, DIM:512],
                                 start=True, stop=True)
                rkb = p_sm.tile([128, DIM], f32, tag="rkb", name="rkb")
                s["rkb"] = rkb
                nc.scalar.copy(rkb[:], psb[:])

            def part3():
                scp, rkb = s["scp"], s["rkb"]
                E = [p_sm.tile([128, DIM], f32, tag="e", name="e") for _ in range(2)]
                # masked softmax, exp with fused row-sum; the all-SBUF chain
                # ops legally run on GPSIMD/Pool, freeing DVE for PSUM copies
                for rr in range(2):
                    sc = p_sm.tile([128, DIM], f32, tag="sc", name="sc")
                    # fused (scp * rq) * rkb in one DVE pass
                    nc.vector.scalar_tensor_tensor(
                        sc[:], scp[:, DIM * rr:DIM * (rr + 1)], rqT[rr][:],
                        rkb[:], op0=mybir.AluOpType.mult,
                        op1=mybir.AluOpType.mult)
                    nc.gpsimd.tensor_add(sc[:], sc[:], msk[rr][:])
                    z = p_tn.tile([128, 1], f32, tag="z", name="z")
                    nc.scalar.activation(E[rr][:], sc[:], Exp, accum_out=z[:])
                    rz = p_tn.tile([128, 1], f32, tag="rz", name="rz")
                    nc.vector.reciprocal(rz[:], z[:])
                    rZ.append(rz)
                # Ahat = E / Z (rows)
                Ahat = [p_sm.tile([128, DIM], f32r, tag="ah", name="ah", bufs=3)
                        for _ in range(2)]
                for rr in range(2):
                    nc.vector.tensor_scalar_mul(Ahat[rr][:], E[rr][:], rZ[rr][:])
                s["Ahat"] = Ahat

            return (part1, part2, part3)

        def emit_softmax(vk):
            for p in softmax_parts(vk):
                p()

        def out_head(vk):
            s = state[vk]
            Ahat = s["Ahat"]
            # m1t[d,o] = (W_po @ Ahat)^T ; wch[c,o] = (W_po @ Ahat @ W_v)^T
            m1t = [p_sm.tile([128, DIM], f16, tag="m1t", name="m1t") for _ in range(2)]
            for d in range(2):
                ps = pp_pq.tile([128, DIM], f32, tag="pq", name="pq")
                for k in range(C128):
                    nc.tensor.matmul(
                        ps[:], r(Ahat[k][:, 128 * d:128 * (d + 1)]), r(wp[k][:]),
                        start=(k == 0), stop=(k == C128 - 1))
                nc.scalar.copy(m1t[d][:], ps[:])
            # wch in e4m3 + e5m2 residual: the final matmul runs as fp8
            # DoubleRow with error feedback (wch8@x8 + wch8@dx + dwch@x8),
            # adding only ~0.2% output error but halving the PE cost
            wch8 = p_sm.tile([128, 2, DIM], f8, tag="wch8", name="wch8")
            dwch5 = p_sm.tile([128, 2, DIM], f8e5, tag="dwch5", name="dwch5")
            s["wch8"], s["dwch5"] = wch8, dwch5
            for cb in range(2):
                ps = pp_pq.tile([128, DIM], f32, tag="pq", name="pq")
                for d in range(2):
                    nc.tensor.matmul(
                        ps[:], wv2[d][:, 128 * cb:128 * (cb + 1)], m1t[d][:],
                        start=(d == 0), stop=(d == 1))
                nc.scalar.copy(wch8[:, cb, :], ps[:])
                nc.vector.tensor_sub(dwch5[:, cb, :], ps[:], wch8[:, cb, :])

        def fin_tile(vk, i, mixed=True):
            """One 1024-wide output tile of W_chain @ x (fp8 DoubleRow with
            error feedback). mixed=False keeps PSUM in the pq pool so tiles
            can interleave with a stream (whose psik owns the ik pool)."""
            s = state[vk]
            b = s["b"]
            x8t, x5t = s["x8"], s["x5"]
            wch8, dwch5 = s["wch8"], s["dwch5"]
            o, h = divmod(i, NM512 // 2)
            st = p_fin.tile([128, 1024], f16, tag="fin", name="fin")
            for half in range(2):
                n = 2 * h + half
                pool = pp_pq if (not mixed or (2 * i + half) % 2 == 0) else pp_ik
                tg = "pq" if pool is pp_pq else "pik"
                ps = pool.tile([128, 512], f32, tag=tg, name=tg)
                osl = slice(128 * o, 128 * (o + 1))
                nsl = slice(512 * n, 512 * (n + 1))
                nc.tensor.matmul(ps[:], wch8[:, :, osl], x8t[:, :, nsl],
                                 start=True, stop=False, perf_mode=DR)
                nc.tensor.matmul(ps[:], wch8[:, :, osl], x5t[:, :, nsl],
                                 start=False, stop=False, perf_mode=DR)
                nc.tensor.matmul(ps[:], dwch5[:, :, osl], x8t[:, :, nsl],
                                 start=False, stop=True, perf_mode=DR)
                if half == 0:
                    nc.scalar.copy(st[:, 0:512], ps[:])
                else:
                    nc.vector.tensor_copy(st[:, 512:1024], ps[:])
            deng = nc.sync if h % 2 == 0 else nc.gpsimd
            deng.dma_start(
                out=od[b, 128 * o:128 * (o + 1), 1024 * h:1024 * (h + 1)],
                in_=st[:])

        def fin_tile_split(vk, i):
            # last tile of the kernel: two half staging tiles with parallel
            # DMAs on separate queues shortens the post-matmul drain chain
            s = state[vk]
            b = s["b"]
            x8t, x5t = s["x8"], s["x5"]
            wch8, dwch5 = s["wch8"], s["dwch5"]
            o, h = divmod(i, NM512 // 2)
            for half in range(2):
                n = 2 * h + half
                pool = pp_pq if half == 0 else pp_ik
                tg = "pq" if pool is pp_pq else "pik"
                ps = pool.tile([128, 512], f32, tag=tg, name=tg)
                osl = slice(128 * o, 128 * (o + 1))
                nsl = slice(512 * n, 512 * (n + 1))
                nc.tensor.matmul(ps[:], wch8[:, :, osl], x8t[:, :, nsl],
                                 start=True, stop=False, perf_mode=DR)
                nc.tensor.matmul(ps[:], wch8[:, :, osl], x5t[:, :, nsl],
                                 start=False, stop=False, perf_mode=DR)
                nc.tensor.matmul(ps[:], dwch5[:, :, osl], x8t[:, :, nsl],
                                 start=False, stop=True, perf_mode=DR)
                st = p_fin.tile([128, 512], f16, tag="fin2", name="fin2")
                if half == 0:
                    nc.scalar.copy(st[:], ps[:])
                else:
                    nc.vector.tensor_copy(st[:], ps[:])
                deng = nc.sync if half == 0 else nc.gpsimd
                deng.dma_start(
                    out=od[b, 128 * o:128 * (o + 1), 512 * n:512 * (n + 1)],
                    in_=st[:])

        def emit_out(vk, hooks=(), tiles=None, split_last=False):
            s = state[vk]
            if "wch8" not in s:
                out_head(vk)
            if len(hooks) > 0:
                hooks[0]()
            idxs = list(range(2 * (NM512 // 2)) if tiles is None else tiles)
            for i in idxs:
                if split_last and i == idxs[-1]:
                    fin_tile_split(vk, i)
                else:
                    fin_tile(vk, i)
                if i + 1 < len(hooks):
                    hooks[i + 1]()

        # software pipeline: q1(b+1) fills the PE while batch b's softmax
        # chain runs *inside* stream(b+1) via hooks (its tiny PE ops slot
        # between conv groups; part 1 frees the score/gram PSUM banks before
        # stream(b+1)'s first scores matmul needs them)
        sq_ = list(range(n_batches)) if seq is None else list(seq)
        vis = [(i, b) for i, b in enumerate(sq_)]
        n = len(vis)
        emit_load_q1(0, vis[0][1], after_c0=early_consts)
        emit_stream(0)
        late_consts()
        for i in range(1, n):
            emit_load_q1(i, vis[i][1])
            # softmax(i-1) and batch i-1's out-head hide inside stream(i)'s
            # conv groups
            sm = softmax_parts(i - 1)
            emit_stream(i, hooks=(*sm, lambda vv=i - 1: out_head(vv)))
            if i >= 2:
                emit_out(i - 2)
        if n > 1:
            # tail: last softmax chain hides under out(n-2)'s fin stream
            emit_out(n - 2, hooks=softmax_parts(n - 1))
            out_head(n - 1)
            emit_out(n - 1, split_last=True)
        else:
            emit_softmax(0)
            emit_out(0)

    if split_waits:
        _split_waits(nc)
    return nc


def _get_nc():
    if "nc" not in _CACHE:
        _CACHE["nc"] = build_nc()
    return _CACHE["nc"]


def make_inputs(inputs):
    """Host-side prep: consts + per-core sharded activations."""
    consts = _host_consts(inputs["W_kv"], inputs["W_q"], inputs["W_dw"],
                          inputs["W_po"], inputs["temperature"])
    xf = np.asarray(inputs["x"], np.float32)
    x8f = _f8(xf)
    x5f = _f8e5(xf - np.float32(x8f))
    x8 = x8f.reshape(B, 2, 128, M).transpose(0, 2, 1, 3)
    x5 = x5f.reshape(B, 2, 128, M).transpose(0, 2, 1, 3)
    y8 = _f8(inputs["y"]).reshape(B, 2, 128, L).transpose(0, 2, 1, 3)
    in_maps = []
    for i in range(NCORES):
        m = dict(consts)
        m["x8_sh"] = np.ascontiguousarray(x8[BL * i:BL * (i + 1)])
        m["x5_sh"] = np.ascontiguousarray(x5[BL * i:BL * (i + 1)])
        m["y8_sh"] = np.ascontiguousarray(y8[BL * i:BL * (i + 1)])
        in_maps.append(m)
    return in_maps


def run(inputs, trace=False, trace_kwargs=None):
    from concourse.bass_utils import run_bass_kernel_spmd

    nc = _get_nc()
    in_maps = make_inputs(inputs)
    res = run_bass_kernel_spmd(
        nc, in_maps, core_ids=list(range(NCORES)), trace=trace,
        trace_kwargs=trace_kwargs or {})
    out = np.concatenate(
        [np.asarray(res.results[i]["out"], np.float32) for i in range(NCORES)],
        axis=0)
    return out, res


def kernel(**inputs) -> np.ndarray:
    out, _ = run(inputs, trace=False)
    return out
